# revision 19
# baseline (speedup 1.0000x reference)
"""FEELVOS fused kernel for TRN2, 8-core SPMD — fp16 rev3.

Sharding: the reference only returns logits for classes C-2, C-1, so only 4 of
the 8 fused (batch, class) items matter. 8 cores = 4 (b, c) pairs x 2 frame
halves (top/bottom 24 rows). Bottom-half cores receive row-flipped inputs and
row-flipped conv kernels so every core runs the identical program computing
"top 25 rows" of its (possibly flipped) frame; the host un-flips on gather.

rev5: ref-major matching. The distance matmul puts 128 REF pixels on the
PSUM partitions and all 1200 queries on the free axis, NEGATED (m = 2ab -
|a|^2 - |b|^2 = -d^2, via sign flips host/device), so the reduction over
refs is a MAX. 18 ref chunks are folded by elementwise max-merges into two
fp16 accumulators: even chunks merge straight from PSUM on DVE; odd chunks
are copied PSUM->fp16 by ACT and merged by GPSIMD (SBUF-only engine). This
splits the PSUM-drain floor (~1.1 ns/elem/lane, dtype-independent on any
single engine) across three engines. The final 128-partition max per query
goes through PE transposes + short DVE reduces.

All matmul operands fp16; PSUM fp32. K=104 carries |a|^2 (rows 100/101 of
the ref operand, negated hi+lo pair, vs ones) and |b|^2 (rows 102/103 of
the query operand, negated hi+lo, vs ones), so -d^2 comes out of the
matmul complete.

U-Net decoder convs read single concatenated-K tiles: skip connections are
written at partition offsets via matmul tile_position (enc1 -> cat1[32:48],
enc2 -> cat2[64:96]), halving decoder matmuls. Embedding/square matmuls are
interleaved between U-Net layers to keep the PE busy (HAM warm). The head
conv runs directly on the padded [7,2500] xt plane (9 shifted K=7 matmuls).
"""
import numpy as np

import concourse.bass as bass
import concourse.bacc as bacc
import concourse.tile as tile
from concourse import mybir
from concourse.bass_utils import run_bass_kernel_spmd
from concourse.masks import make_identity

F32 = mybir.dt.float32
F16 = mybir.dt.float16
AF = mybir.ActivationFunctionType
ALU = mybir.AluOpType
AX = mybir.AxisListType

H = W = 48
NREF = H * W                 # 2304 ref pixels (full frame)
QROWS = 25
Q = QROWS * W                # 1200 query pixels
QCH, NQC = 120, 10           # query chunking for the distance matmul
RECH = [512, 512, 512, 512, 256]  # 2304 column chunking (PSUM bank)
E3CH = [432, 384, 384]       # 1200 column chunking
_PROG = None


def _r3(ap, h, w):
    return ap.rearrange("c (h w) -> c h w", h=h, w=w)


# blob column layouts: (name, row0, nrows, cols), fp16. Row offsets place
# weights at the partition base their matmul's contraction rows need.
ASEGS = [("enc1s", 0, 27, 16), ("enc2", 32, 16, 288), ("bott", 64, 32, 576)]
BSEGS = [("dec2", 0, 96, 288), ("dec1", 0, 48, 144), ("out", 0, 16, 4),
         ("outc", 0, 16, 1), ("emb", 0, 10, 100), ("dshc", 0, 7, 9)]


def _offsets(segs):
    off, o = {}, 0
    for nm, _r0, _r, c in segs:
        off[nm] = o
        o += c
    return off, o


AOFF, ACOLS = _offsets(ASEGS)
BOFF, BCOLS = _offsets(BSEGS)
BIAS_COL = {"enc1": 0, "enc2": 1, "bott": 2, "dec2": 3, "dec1": 4,
            "out": 5, "dsh": 6, "outc": 7}


def _emit(nc, tc, ctx):
    # ------------------------------------------------------------- dram io
    bA = nc.dram_tensor("blobA", [96, ACOLS], F16, kind="ExternalInput").ap()
    bB = nc.dram_tensor("blobB", [96, BCOLS], F16, kind="ExternalInput").ap()
    bBias = nc.dram_tensor("blobBias", [96, 10], F32,
                           kind="ExternalInput").ap()
    bX1 = nc.dram_tensor("blobX1", [27, 39 * W], F16,
                         kind="ExternalInput").ap()
    bX2 = nc.dram_tensor("blobX2", [10, 2 * NREF], F16,
                         kind="ExternalInput").ap()
    bX3 = nc.dram_tensor("blobX3", [1, Q], F16, kind="ExternalInput").ap()
    out_d = nc.dram_tensor("out", [1, 24 * W], F32, kind="ExternalOutput").ap()

    # ------------------------------------------------------------- sbuf
    sb = ctx.enter_context(tc.tile_pool(name="sb", bufs=1))

    def st(name, p, f, dt=F16):
        return sb.tile([p, f], dt, tag=name, name=name)

    bloba = st("bloba", 96, ACOLS)
    blobb = st("blobb", 96, BCOLS)
    blobbias = st("blobbias", 96, 10, F32)
    im27 = st("im27", 27, 39 * W)        # enc1 im2col (host-built)
    im2c12 = st("im2c12", 10, 2 * NREF)  # emb im2col e1|e2 (host-built)

    def wseg(blob, off, segs, nm):
        r0, rows, cols = next((a, b, c) for n, a, b, c in segs if n == nm)
        return blob[r0:r0 + rows, off[nm]:off[nm] + cols]

    wt = {nm: wseg(bloba, AOFF, ASEGS, nm) for nm, _, _, _ in ASEGS}
    wt.update({nm: wseg(blobb, BOFF, BSEGS, nm) for nm, _, _, _ in BSEGS})
    bia = {nm: blobbias[0:r, c:c + 1]
           for nm, (r, c) in {"enc1": (16, 0), "enc2": (32, 1),
                              "bott": (64, 2), "dec2": (32, 3),
                              "dec1": (16, 4), "out": (4, 5),
                              "dsh": (1, 6), "outc": (1, 7)}.items()}
    bia["enc1@32"] = blobbias[32:48, 8:9]
    bia["enc2@64"] = blobbias[64:96, 9:10]

    # device-written padded planes (fp16)
    x3cp = st("x3cp", 1, 2500)
    cat1 = st("cat1", 48, 2500)   # rows 0..31 up(d2), rows 32..47 e1
    p1p = st("p1p", 48, 676)      # rows 32..47 used
    cat2 = st("cat2", 96, 676)    # rows 0..63 up(bt), rows 64..95 e2
    p2p = st("p2p", 96, 196)      # rows 64..95 used
    btp = st("btp", 64, 196)
    d2p = st("d2p", 32, 676)
    d1p = st("d1p", 16, 2500)
    xt = st("xt", 7, 2500)

    im2c3 = st("im2c3", 10, Q)           # emb im2col (e3)
    e1x = st("e1x", 104, NREF)           # 0..99 -2*e1, 100/101 |a|^2 hi/lo,
    e2x = st("e2x", 104, NREF)           #   102/103 ones
    e3x = st("e3x", 104, Q)              # 0..99 e3, 100/101 ones,
    esq1 = st("esq1", 100, NREF)         #   102/103 |b|^2 hi/lo
    esq2 = st("esq2", 100, NREF)
    esq3 = st("esq3", 100, Q)
    ident = st("ident", 128, 128)

    c025 = st("c025", 100, 1)
    c1 = st("c1", 100, 2)
    hib1 = st("hib1", 1, NREF)
    lob1 = st("lob1", 1, NREF)
    hib2 = st("hib2", 1, NREF)
    lob2 = st("lob2", 1, NREF)
    hib3 = st("hib3", 1, Q)
    lob3 = st("lob3", 1, Q)
    accDG = st("accDG", 128, Q)          # fp16 running-max accumulators
    accAG = st("accAG", 128, Q)
    accDL = st("accDL", 128, Q)
    accAL = st("accAL", 128, Q)
    accMG = st("accMG", 128, Q)
    accML = st("accML", 128, Q)
    out_sb = st("out_sb", 1, 24 * W, F32)

    small = ctx.enter_context(tc.tile_pool(name="small", bufs=8))
    tmp = ctx.enter_context(tc.tile_pool(name="tmp", bufs=2))
    scr = ctx.enter_context(tc.tile_pool(name="scr", bufs=3))

    # ------------------------------------------------------------- input dma
    # exact-row segment DMAs, hot-first
    def seg_dma(blobt, blobd, off, segs, nm):
        r0, rows, cols = next((a, b, c) for n, a, b, c in segs if n == nm)
        nc.sync.dma_start(blobt[r0:r0 + rows, off[nm]:off[nm] + cols],
                          blobd[r0:r0 + rows, off[nm]:off[nm] + cols])

    seg_dma(bloba, bA, AOFF, ASEGS, "enc1s")
    nc.sync.dma_start(blobbias[:], bBias)
    nc.sync.dma_start(im27[:, 0:960], bX1[:, 0:960])        # enc1 rows 0..19
    nc.sync.dma_start(im27[:, 960:39 * W], bX1[:, 960:39 * W])
    seg_dma(bloba, bA, AOFF, ASEGS, "enc2")
    seg_dma(blobb, bB, BOFF, BSEGS, "emb")
    nc.sync.dma_start(im2c12[:, 0:NREF], bX2[:, 0:NREF])
    seg_dma(bloba, bA, AOFF, ASEGS, "bott")
    nc.sync.dma_start(im2c12[:, NREF:2 * NREF], bX2[:, NREF:2 * NREF])
    seg_dma(blobb, bB, BOFF, BSEGS, "dec2")
    nc.sync.dma_start(blobb[0:48, BOFF["dec1"]:BCOLS],
                      bB[0:48, BOFF["dec1"]:BCOLS])  # dec1+out+outc+dshc

    # ------------------------------------------------------------- init
    make_identity(nc, ident[:])
    nc.gpsimd.memset(c025[:], 0.25)
    nc.gpsimd.memset(c1[:], 1.0)
    nc.gpsimd.memset(xt[:], 0.0)
    # engine partition starts must be 32-aligned; rows 96..99 / 0..8 are
    # overwritten later by the embconv ACT / shift DMAs; rows 100/101 of
    # e1x/e2x and 102/103 of e3x by the hi/lo DMAs.
    nc.vector.memset(e1x[96:104, :], 1.0)
    nc.vector.memset(e2x[96:104, :], 1.0)
    nc.vector.memset(e3x[96:104, :], 1.0)
    nc.vector.memset(im2c3[0:10, :], 1.0)

    xt3 = _r3(xt[:], 50, 50)
    x3cp3 = _r3(x3cp[:], 50, 50)

    def borders(eng, ap3, pw):
        eng.memset(ap3[:, 0:1, :], 0.0)
        eng.memset(ap3[:, pw - 1:pw, :], 0.0)
        eng.memset(ap3[:, 1:pw - 1, 0:1], 0.0)
        eng.memset(ap3[:, 1:pw - 1, pw - 1:pw], 0.0)

    cat13 = _r3(cat1[:], 50, 50)
    p1p3 = _r3(p1p[:], 26, 26)
    cat23 = _r3(cat2[:], 26, 26)
    p2p3 = _r3(p2p[:], 14, 14)
    btp3 = _r3(btp[:], 14, 14)
    d2p3 = _r3(d2p[:], 26, 26)
    d1p3 = _r3(d1p[:], 50, 50)

    borders(nc.gpsimd, x3cp3, 50)
    borders(nc.gpsimd, cat13, 50)
    borders(nc.gpsimd, p1p3[32:48], 26)
    borders(nc.vector, cat23, 26)
    borders(nc.vector, p2p3[64:96], 14)
    borders(nc.vector, btp3, 14)
    borders(nc.vector, d2p3, 26)
    borders(nc.gpsimd, d1p3, 50)
    # xt ch6 = x2 rows 0..24 straight from dram (after the xt memset)
    nc.gpsimd.dma_start(xt3[6:7, 1:26, 1:49], bX3)

    pconv = ctx.enter_context(tc.tile_pool(name="pconv", bufs=2, space="PSUM"))
    pmain = ctx.enter_context(tc.tile_pool(name="pmain", bufs=2, space="PSUM"))

    def preheat(n):
        # dummy matmuls on long-ready inputs: keep the PE HAM clock warm
        # across windows where real work is blocked on DMA/ACT chains.
        for _ in range(n):
            ph = pmain.tile([128, Q], F32, tag="main", name="mainps")
            nc.tensor.matmul(ph[0:100, 0:512], wt["emb"],
                             im2c12[0:10, 0:512], start=True, stop=True)

    # ------------------------------------------------------------ helpers
    def conv9(src3, wtile, cin, cout, row_chunks, w_, func, bias_ap, dst3,
              pbase=0, obase=0):
        tp = (pbase, obase) if (pbase or obase) else None
        s3 = src3[pbase:pbase + cin]
        r0 = 0
        for nr in row_chunks:
            ps = pconv.tile([obase + cout, nr * w_], F32, tag="conv",
                            name="convps")
            for s in range(9):
                dy, dx = s // 3, s % 3
                nc.tensor.matmul(ps[obase:obase + cout, :],
                                 wtile[:, s * cout:(s + 1) * cout],
                                 s3[:, r0 + dy:r0 + dy + nr, dx:dx + w_],
                                 start=(s == 0), stop=(s == 8),
                                 tile_position=tp)
            nc.scalar.activation(dst3[obase:obase + cout,
                                      1 + r0:1 + r0 + nr, 1:1 + w_],
                                 _r3(ps[obase:obase + cout, :], nr, w_),
                                 func, bias=bias_ap)
            r0 += nr

    def pool2(src3, dst3, orows, ocols, pbase, cch):
        t1 = tmp.tile([pbase + cch, orows * ocols], F16, tag="pool_a",
                      name="poolt1")
        t2 = tmp.tile([pbase + cch, orows * ocols], F16, tag="pool_b",
                      name="poolt2")
        s3 = src3[pbase:pbase + cch]
        v = [s3[:, 1 + a:1 + a + 2 * orows:2, 1 + b:1 + b + 2 * ocols:2]
             for a, b in ((0, 0), (1, 1), (0, 1), (1, 0))]
        t13 = _r3(t1[pbase:pbase + cch, :], orows, ocols)
        t23 = _r3(t2[pbase:pbase + cch, :], orows, ocols)
        nc.vector.tensor_max(t13, v[0], v[1])
        nc.vector.tensor_max(t23, v[2], v[3])
        nc.vector.tensor_max(dst3[pbase:pbase + cch, 1:1 + orows,
                                  1:1 + ocols], t13, t23)

    def up2(src3, sbase, dst3, dbase, cch, irows, icols):
        s = src3[sbase:sbase + cch, 1:1 + irows, 1:1 + icols]
        for a in (0, 1):
            for b in (0, 1):
                nc.vector.tensor_copy(
                    dst3[dbase:dbase + cch, 1 + a:1 + a + 2 * irows:2,
                         1 + b:1 + b + 2 * icols:2], s)

    def embconv(imbuf, chunks, dst, scale):
        off = 0
        for cw in chunks:
            ps = pconv.tile([100, cw], F32, tag="conv", name="convps")
            nc.tensor.matmul(ps[:], wt["emb"], imbuf[:, off:off + cw],
                             start=True, stop=True)
            nc.scalar.activation(dst[0:100, off:off + cw], ps[:],
                                 AF.Copy, scale=scale)
            off += cw

    def sqhilo(src, chunks, lhsT, esq, hib, lob, ex, row):
        """rows(row, row+1) of ex = NEGATED hi/lo fp16 pair of
        lhsT.T @ Square(src)."""
        n = sum(chunks)
        off = 0
        for cw in chunks:
            nc.scalar.activation(esq[:, off:off + cw],
                                 src[0:100, off:off + cw], AF.Square)
            ps = pconv.tile([1, cw], F32, tag="conv", name="sqps")
            nc.tensor.matmul(ps[:], lhsT, esq[:, off:off + cw],
                             start=True, stop=True)
            nc.scalar.activation(hib[0:1, off:off + cw], ps[:], AF.Copy,
                                 scale=-1.0)
            nc.vector.scalar_tensor_tensor(lob[0:1, off:off + cw], ps[:],
                                           -1.0, hib[0:1, off:off + cw],
                                           op0=ALU.mult, op1=ALU.subtract)
            nc.sync.dma_start(ex[row:row + 1, off:off + cw],
                              hib[0:1, off:off + cw])
            nc.sync.dma_start(ex[row + 1:row + 2, off:off + cw],
                              lob[0:1, off:off + cw])
            off += cw

    # --------------------------------------------- U-Net + emb interleave
    # enc1: im2col matmuls -> cat1[32:48] (tile_position col offset 32)
    r0 = 0
    for nr in (10, 10, 10, 8):
        ps = pconv.tile([48, nr * W], F32, tag="conv", name="convps")
        nc.tensor.matmul(ps[32:48, :], wt["enc1s"],
                         im27[:, r0 * W:(r0 + nr) * W],
                         start=True, stop=True, tile_position=(0, 32))
        nc.scalar.activation(cat13[32:48, 1 + r0:1 + r0 + nr, 1:1 + W],
                             _r3(ps[32:48, :], nr, W), AF.Relu,
                             bias=bia["enc1@32"])
        r0 += nr
    embconv(im2c12[0:10, 0:NREF], RECH, e1x[:], 2.0)       # PE filler
    pool2(cat13, p1p3, 19, 24, 32, 16)
    conv9(p1p3, wt["enc2"], 16, 32, [18], 24, AF.Relu, bia["enc2@64"],
          cat23, pbase=32, obase=64)
    embconv(im2c12[0:10, NREF:2 * NREF], RECH, e2x[:], 2.0)
    pool2(cat23, p2p3, 9, 12, 64, 32)
    conv9(p2p3, wt["bott"], 32, 64, [8], 12, AF.Relu, bia["bott"], btp3,
          pbase=64, obase=0)
    sqhilo(e1x[:], RECH, c025[:], esq1[:], hib1, lob1, e1x[:], 100)
    up2(btp3, 0, cat23, 0, 64, 8, 12)
    conv9(cat23, wt["dec2"], 96, 32, [14], 24, AF.Relu, bia["dec2"], d2p3)
    sqhilo(e2x[:], RECH, c025[:], esq2[:], hib2, lob2, e2x[:], 100)
    up2(d2p3, 0, cat13, 0, 32, 14, 24)
    conv9(cat13, wt["dec1"], 48, 16, [10, 10, 6], W, AF.Relu, bia["dec1"],
          d1p3)

    # 1x1 output conv -> xt[0:4] (all 4 channels) and x3cp (class-c channel)
    r0 = 0
    for nr in (10, 10, 6):
        rhs = d1p3[:, 1 + r0:1 + r0 + nr, 1:1 + W]
        psc = pconv.tile([1, nr * W], F32, tag="conv", name="convps")
        nc.tensor.matmul(psc[:], wt["outc"], rhs, start=True, stop=True)
        nc.scalar.activation(x3cp3[0:1, 1 + r0:1 + r0 + nr, 1:1 + W],
                             _r3(psc[:], nr, W), AF.Identity,
                             bias=bia["outc"])
        ps = pconv.tile([4, nr * W], F32, tag="conv", name="convps")
        nc.tensor.matmul(ps[:], wt["out"], rhs, start=True, stop=True)
        nc.scalar.activation(xt3[0:4, 1 + r0:1 + r0 + nr, 1:1 + W],
                             _r3(ps[:], nr, W), AF.Identity, bias=bia["out"])
        r0 += nr

    preheat(14)

    # ------------------------------------------------------- embedding 3
    for s in range(9):
        dy, dx = s // 3, s % 3
        eng = (nc.sync, nc.gpsimd, nc.scalar)[s % 3]
        eng.dma_start(im2c3[s:s + 1, 0:Q],
                      x3cp3[0:1, dy:dy + QROWS, dx:dx + W])
    embconv(im2c3[:], E3CH, e3x[:], 1.0)
    sqhilo(e3x[:], E3CH, c1[:, 0:1], esq3[:], hib3, lob3, e3x[:], 102)

    # ------------------------------------------------------- matching
    nc.vector.memset(accDG[:], -60000.0)
    nc.vector.memset(accAG[:], -60000.0)
    nc.vector.memset(accDL[:], -60000.0)
    nc.vector.memset(accAL[:], -60000.0)

    def match_chunks(ex, accD, accA, k_range):
        for k in k_range:
            lhsT = ex[:][:, k * 128:(k + 1) * 128]
            ps = pmain.tile([128, Q], F32, tag="main", name="mainps")
            for o, n in ((0, 512), (512, 512), (1024, 176)):
                nc.tensor.matmul(ps[:, o:o + n], lhsT,
                                 e3x[:, o:o + n], start=True, stop=True)
            if k % 3 == 0:          # DVE drains PSUM directly (1x rate)
                nc.vector.tensor_max(accD[:], ps[:], accD[:])
            else:                    # ACT copies; DVE merges all-fp16 (2x)
                sc = scr.tile([128, Q], F16, tag="scr", name="scrt")
                nc.scalar.copy(sc[:], ps[:])
                nc.vector.tensor_max(accA[:], sc[:], accA[:])

    def ref_finals(accD, accA, accM, qmax):
        nc.vector.tensor_max(accM[:], accD[:], accA[:])

    def ref_transposes(accM, qmax):
        for i in range(NQC):
            n = 128 if i < 9 else 48
            pst = pconv.tile([n, 128], F16, tag="conv", name="trps")
            nc.tensor.transpose(pst[:], accM[:, i * 128:i * 128 + n],
                                ident[:128, :128])
            nc.vector.tensor_reduce(qmax[0:n, i:i + 1], pst[:],
                                    axis=AX.X, op=ALU.max)

    def ref_plane(qmax, r, eng):
        tneg = small.tile([128, NQC], F32, tag="tneg", name="tneg")
        nc.vector.tensor_scalar(tneg[:], qmax[:], 0.0, 0.0,
                                op0=ALU.min, op1=ALU.min)
        gcol = small.tile([128, NQC], F16, tag="gcol", name="gcol")
        nc.scalar.activation(gcol[:], tneg[:], AF.Tanh, scale=-0.5)
        pst = pconv.tile([NQC, 128], F16, tag="conv", name="gmtps")
        nc.tensor.transpose(pst[:], gcol[:], ident[:128, :128])
        gcolT = small.tile([NQC, 128], F16, tag="gcolT", name="gcolT")
        nc.scalar.copy(gcolT[:], pst[:])
        gflat = small.tile([1, Q], F16, tag="gflat", name="gflat")
        eng.dma_start(gflat[0:1, 0:1152], gcolT[0:9, :])
        eng.dma_start(gflat[0:1, 1152:1200], gcolT[9:10, 0:48])
        eng.dma_start(xt3[4 + r:5 + r, 1:26, 1:49], gflat[:])

    qmaxG = small.tile([128, NQC], F32, tag="qmaxG", name="qmaxG")
    qmaxL = small.tile([128, NQC], F32, tag="qmaxL", name="qmaxL")

    match_chunks(e1x, accDG, accAG, range(18))     # gm
    ref_finals(accDG, accAG, accMG, qmaxG)         # DVE combine (overlaps lm)
    match_chunks(e2x, accDL, accAL, range(4))
    ref_transposes(accMG, qmaxG)                   # PE slots between lm MMs
    match_chunks(e2x, accDL, accAL, range(4, 18))
    ref_plane(qmaxG, 0, nc.gpsimd)
    ref_finals(accDL, accAL, accML, qmaxL)
    ref_transposes(accML, qmaxL)
    ref_plane(qmaxL, 1, nc.gpsimd)
    preheat(12)

    # ------------------------------------------------------- head conv
    r0 = 0
    for nr in (8, 8, 8):
        ps = pconv.tile([1, nr * W], F32, tag="conv", name="convps")
        for s in range(9):
            dy, dx = s // 3, s % 3
            nc.tensor.matmul(ps[:], wt["dshc"][:, s:s + 1],
                             xt3[:, r0 + dy:r0 + dy + nr, dx:dx + W],
                             start=(s == 0), stop=(s == 8))
        nc.scalar.activation(out_sb[0:1, r0 * W:(r0 + nr) * W],
                             _r3(ps[:], nr, W), AF.Identity, bias=bia["dsh"])
        r0 += nr
    nc.sync.dma_start(out_d, out_sb[:])


def build_program():
    import contextlib
    nc = bacc.Bacc("TRN2", target_bir_lowering=False, debug=False,
                   num_devices=8)
    with tile.TileContext(nc) as tc:
        with contextlib.ExitStack() as ctx:
            _emit(nc, tc, ctx)
    nc.compile()
    return nc


def _get_program():
    global _PROG
    if _PROG is None:
        _PROG = build_program()
    return _PROG


CORE_BC = [(0, 2), (0, 3), (1, 2), (1, 3)]


def _wT_flat(w):
    """[Cout, Cin, 3, 3] -> [Cin, 9*Cout]: col block s holds w[:, :, s//3, s%3].T"""
    cout, cin = w.shape[:2]
    out = np.zeros((cin, 9 * cout), np.float32)
    for s in range(9):
        out[:, s * cout:(s + 1) * cout] = w[:, :, s // 3, s % 3].T
    return out


def _pad50(img):
    out = np.zeros((50, 50), np.float32)
    out[1:49, 1:49] = img
    return out


def _im2col9(img, rows, ones_row=False):
    """padded 50x50 -> [9(+1), rows*48] rows ordered s=dy*3+dx."""
    p = _pad50(img)
    rws = [p[dy:dy + rows, dx:dx + W].ravel()
           for dy in range(3) for dx in range(3)]
    if ones_row:
        rws.append(np.ones(rows * W, np.float32))
    return np.stack(rws)


def _blobs(inp, flip, c):
    w = {k: (inp[k][:, :, ::-1, :] if flip else inp[k])
         for k in ["enc1_w", "enc2_w", "bott_w", "dec2_w", "dec1_w",
                   "emb_w", "dsh_w"]}
    seg = {}
    seg["enc1s"] = w["enc1_w"].reshape(16, 3, 9).transpose(2, 1, 0) \
                              .reshape(27, 16)
    seg["enc2"] = _wT_flat(w["enc2_w"])
    seg["bott"] = _wT_flat(w["bott_w"])
    seg["dec2"] = _wT_flat(w["dec2_w"])
    seg["dec1"] = _wT_flat(w["dec1_w"])
    seg["out"] = inp["out_w"][:, :, 0, 0].T
    seg["outc"] = inp["out_w"][c, :, 0, 0][:, None]
    seg["emb"] = np.vstack([w["emb_w"].reshape(100, 9).T,
                            inp["emb_b"][None, :]])
    seg["dshc"] = w["dsh_w"].reshape(7, 9)

    def pack(segs, ncols):
        blob = np.zeros((96, ncols), np.float16)
        off = 0
        for nm, r0, rows, cols in segs:
            blob[r0:r0 + rows, off:off + cols] = seg[nm].astype(np.float16)
            off += cols
        return blob

    blobbias = np.zeros((96, 10), np.float32)
    for nm, col in BIAS_COL.items():
        if nm == "outc":
            v = inp["out_b"][c:c + 1]
        else:
            v = inp[nm + "_b"]
        blobbias[0:len(v), col] = v
    blobbias[32:48, 8] = inp["enc1_b"]
    blobbias[64:96, 9] = inp["enc2_b"]
    return pack(ASEGS, ACOLS), pack(BSEGS, BCOLS), blobbias


def make_in_maps(inp):
    maps = []
    for k8 in range(8):
        n_idx, half = k8 // 2, k8 % 2
        b, c = CORE_BC[n_idx]
        x1c, x2c, x3b = inp["x1"][b, c], inp["x2"][b, c], inp["x3"][b]
        if half:
            x1c, x2c, x3b = x1c[::-1], x2c[::-1], x3b[:, ::-1]
        bx1 = np.zeros((27, 39 * W), np.float32)
        for ci in range(3):
            im9 = _im2col9(x3b[ci], 39)
            for s in range(9):
                bx1[s * 3 + ci] = im9[s]
        bx2 = np.concatenate([_im2col9(x1c, H, True),
                              _im2col9(x2c, H, True)], axis=1)
        bx3 = x2c[0:25, :].reshape(1, Q)
        blobA, blobB, blobbias = _blobs(inp, bool(half), c)
        maps.append({"blobA": blobA, "blobB": blobB,
                     "blobBias": blobbias,
                     "blobX1": np.ascontiguousarray(bx1.astype(np.float16)),
                     "blobX2": np.ascontiguousarray(bx2.astype(np.float16)),
                     "blobX3": np.ascontiguousarray(bx3.astype(np.float16))})
    return maps


def assemble(results):
    out = np.zeros((2, 2, H, W), np.float32)
    for k8, r in enumerate(results):
        n_idx, half = k8 // 2, k8 % 2
        b, c = CORE_BC[n_idx]
        y = r["out"].reshape(24, W)
        if half == 0:
            out[b, c - 2, 0:24] = y
        else:
            out[b, c - 2, 24:48] = y[::-1]
    return out


def kernel(**inputs):
    inp = {k: np.asarray(v) for k, v in inputs.items()}
    nc = _get_program()
    maps = make_in_maps(inp)
    res = run_bass_kernel_spmd(nc, maps, core_ids=list(range(8)), trace=False)
    return assemble(res.results)


# revision 20
# speedup vs baseline: 1.0510x; 1.0510x over previous
"""FEELVOS fused kernel for TRN2, 8-core SPMD — fp16 rev3.

Sharding: the reference only returns logits for classes C-2, C-1, so only 4 of
the 8 fused (batch, class) items matter. 8 cores = 4 (b, c) pairs x 2 frame
halves (top/bottom 24 rows). Bottom-half cores receive row-flipped inputs and
row-flipped conv kernels so every core runs the identical program computing
"top 25 rows" of its (possibly flipped) frame; the host un-flips on gather.

rev5: ref-major matching. The distance matmul puts 128 REF pixels on the
PSUM partitions and all 1200 queries on the free axis, NEGATED (m = 2ab -
|a|^2 - |b|^2 = -d^2, via sign flips host/device), so the reduction over
refs is a MAX. 18 ref chunks are folded by elementwise max-merges into two
fp16 accumulators: even chunks merge straight from PSUM on DVE; odd chunks
are copied PSUM->fp16 by ACT and merged by GPSIMD (SBUF-only engine). This
splits the PSUM-drain floor (~1.1 ns/elem/lane, dtype-independent on any
single engine) across three engines. The final 128-partition max per query
goes through PE transposes + short DVE reduces.

All matmul operands fp16; PSUM fp32. K=104 carries |a|^2 (rows 100/101 of
the ref operand, negated hi+lo pair, vs ones) and |b|^2 (rows 102/103 of
the query operand, negated hi+lo, vs ones), so -d^2 comes out of the
matmul complete.

U-Net decoder convs read single concatenated-K tiles: skip connections are
written at partition offsets via matmul tile_position (enc1 -> cat1[32:48],
enc2 -> cat2[64:96]), halving decoder matmuls. Embedding/square matmuls are
interleaved between U-Net layers to keep the PE busy (HAM warm). The head
conv runs directly on the padded [7,2500] xt plane (9 shifted K=7 matmuls).
"""
import numpy as np

import concourse.bass as bass
import concourse.bacc as bacc
import concourse.tile as tile
from concourse import mybir
from concourse.bass_utils import run_bass_kernel_spmd
from concourse.masks import make_identity

F32 = mybir.dt.float32
F16 = mybir.dt.float16
AF = mybir.ActivationFunctionType
ALU = mybir.AluOpType
AX = mybir.AxisListType

H = W = 48
NREF = H * W                 # 2304 ref pixels (full frame)
QROWS = 25
Q = QROWS * W                # 1200 query pixels
QCH, NQC = 120, 10           # query chunking for the distance matmul
RECH = [512, 512, 512, 512, 256]  # 2304 column chunking (PSUM bank)
E3CH = [432, 384, 384]       # 1200 column chunking
_PROG = None


def _r3(ap, h, w):
    return ap.rearrange("c (h w) -> c h w", h=h, w=w)


# blob column layouts: (name, row0, nrows, cols), fp16. Row offsets place
# weights at the partition base their matmul's contraction rows need.
ASEGS = [("enc1s", 0, 27, 16), ("enc2", 32, 16, 288), ("bott", 64, 32, 576)]
BSEGS = [("dec2", 0, 96, 288), ("dec1", 0, 48, 144), ("out", 0, 16, 4),
         ("outc", 0, 16, 1), ("emb", 0, 10, 100), ("dshc", 0, 7, 9)]


def _offsets(segs):
    off, o = {}, 0
    for nm, _r0, _r, c in segs:
        off[nm] = o
        o += c
    return off, o


AOFF, ACOLS = _offsets(ASEGS)
BOFF, BCOLS = _offsets(BSEGS)
BIAS_COL = {"enc1": 0, "enc2": 1, "bott": 2, "dec2": 3, "dec1": 4,
            "out": 5, "dsh": 6, "outc": 7}


def _emit(nc, tc, ctx):
    # ------------------------------------------------------------- dram io
    bA = nc.dram_tensor("blobA", [96, ACOLS], F16, kind="ExternalInput").ap()
    bB = nc.dram_tensor("blobB", [96, BCOLS], F16, kind="ExternalInput").ap()
    bBias = nc.dram_tensor("blobBias", [96, 10], F32,
                           kind="ExternalInput").ap()
    bX1 = nc.dram_tensor("blobX1", [27, 39 * W], F16,
                         kind="ExternalInput").ap()
    bX2 = nc.dram_tensor("blobX2", [10, 2 * NREF], F16,
                         kind="ExternalInput").ap()
    bX3 = nc.dram_tensor("blobX3", [1, Q], F16, kind="ExternalInput").ap()
    out_d = nc.dram_tensor("out", [1, 24 * W], F32, kind="ExternalOutput").ap()

    # ------------------------------------------------------------- sbuf
    sb = ctx.enter_context(tc.tile_pool(name="sb", bufs=1))

    def st(name, p, f, dt=F16):
        return sb.tile([p, f], dt, tag=name, name=name)

    bloba = st("bloba", 96, ACOLS)
    blobb = st("blobb", 96, BCOLS)
    blobbias = st("blobbias", 96, 10, F32)
    im27 = st("im27", 27, 39 * W)        # enc1 im2col (host-built)
    im2c12 = st("im2c12", 10, 2 * NREF)  # emb im2col e1|e2 (host-built)

    def wseg(blob, off, segs, nm):
        r0, rows, cols = next((a, b, c) for n, a, b, c in segs if n == nm)
        return blob[r0:r0 + rows, off[nm]:off[nm] + cols]

    wt = {nm: wseg(bloba, AOFF, ASEGS, nm) for nm, _, _, _ in ASEGS}
    wt.update({nm: wseg(blobb, BOFF, BSEGS, nm) for nm, _, _, _ in BSEGS})
    bia = {nm: blobbias[0:r, c:c + 1]
           for nm, (r, c) in {"enc1": (16, 0), "enc2": (32, 1),
                              "bott": (64, 2), "dec2": (32, 3),
                              "dec1": (16, 4), "out": (4, 5),
                              "dsh": (1, 6), "outc": (1, 7)}.items()}
    bia["enc1@32"] = blobbias[32:48, 8:9]
    bia["enc2@64"] = blobbias[64:96, 9:10]

    # device-written padded planes (fp16)
    x3cp = st("x3cp", 1, 2500)
    cat1 = st("cat1", 48, 2500)   # rows 0..31 up(d2), rows 32..47 e1
    p1p = st("p1p", 48, 676)      # rows 32..47 used
    cat2 = st("cat2", 96, 676)    # rows 0..63 up(bt), rows 64..95 e2
    p2p = st("p2p", 96, 196)      # rows 64..95 used
    btp = st("btp", 64, 196)
    d2p = st("d2p", 32, 676)
    d1p = st("d1p", 16, 2500)
    xt = st("xt", 7, 2500)

    im2c3 = st("im2c3", 10, Q)           # emb im2col (e3)
    e1x = st("e1x", 104, NREF)           # 0..99 -2*e1, 100/101 |a|^2 hi/lo,
    e2x = st("e2x", 104, NREF)           #   102/103 ones
    e3x = st("e3x", 104, Q)              # 0..99 e3, 100/101 ones,
    esq1 = st("esq1", 100, NREF)         #   102/103 |b|^2 hi/lo
    esq2 = st("esq2", 100, NREF)
    esq3 = st("esq3", 100, Q)
    ident = st("ident", 128, 128)

    c025 = st("c025", 100, 1)
    c1 = st("c1", 100, 2)
    hib1 = st("hib1", 1, NREF)
    lob1 = st("lob1", 1, NREF)
    hib2 = st("hib2", 1, NREF)
    lob2 = st("lob2", 1, NREF)
    hib3 = st("hib3", 1, Q)
    lob3 = st("lob3", 1, Q)
    accDG = st("accDG", 128, Q)          # fp16 running-max accumulators
    accAG = st("accAG", 128, Q)
    accDL = st("accDL", 128, Q)
    accAL = st("accAL", 128, Q)
    accMG = st("accMG", 128, Q)
    accML = st("accML", 128, Q)
    out_sb = st("out_sb", 1, 24 * W, F32)

    small = ctx.enter_context(tc.tile_pool(name="small", bufs=8))
    tmp = ctx.enter_context(tc.tile_pool(name="tmp", bufs=2))
    scr = ctx.enter_context(tc.tile_pool(name="scr", bufs=3))

    # ------------------------------------------------------------- input dma
    # exact-row segment DMAs, hot-first
    def seg_dma(blobt, blobd, off, segs, nm):
        r0, rows, cols = next((a, b, c) for n, a, b, c in segs if n == nm)
        nc.sync.dma_start(blobt[r0:r0 + rows, off[nm]:off[nm] + cols],
                          blobd[r0:r0 + rows, off[nm]:off[nm] + cols])

    seg_dma(bloba, bA, AOFF, ASEGS, "enc1s")
    nc.sync.dma_start(blobbias[:], bBias)
    nc.sync.dma_start(im27[:, 0:960], bX1[:, 0:960])        # enc1 rows 0..19
    seg_dma(bloba, bA, AOFF, ASEGS, "enc2")
    seg_dma(blobb, bB, BOFF, BSEGS, "emb")
    nc.sync.dma_start(im27[:, 960:39 * W], bX1[:, 960:39 * W])
    nc.sync.dma_start(im2c12[:, 0:NREF], bX2[:, 0:NREF])
    seg_dma(bloba, bA, AOFF, ASEGS, "bott")
    nc.sync.dma_start(im2c12[:, NREF:2 * NREF], bX2[:, NREF:2 * NREF])
    seg_dma(blobb, bB, BOFF, BSEGS, "dec2")
    nc.sync.dma_start(blobb[0:48, BOFF["dec1"]:BCOLS],
                      bB[0:48, BOFF["dec1"]:BCOLS])  # dec1+out+outc+dshc

    # ------------------------------------------------------------- init
    make_identity(nc, ident[:])
    nc.gpsimd.memset(c025[:], 0.25)
    nc.gpsimd.memset(c1[:], 1.0)
    nc.gpsimd.memset(xt[:], 0.0)
    # engine partition starts must be 32-aligned; rows 96..99 / 0..8 are
    # overwritten later by the embconv ACT / shift DMAs; rows 100/101 of
    # e1x/e2x and 102/103 of e3x by the hi/lo DMAs.
    nc.vector.memset(e1x[96:104, :], 1.0)
    nc.vector.memset(e2x[96:104, :], 1.0)
    nc.vector.memset(e3x[96:104, :], 1.0)
    nc.vector.memset(im2c3[0:10, :], 1.0)

    xt3 = _r3(xt[:], 50, 50)
    x3cp3 = _r3(x3cp[:], 50, 50)

    def borders(eng, ap3, pw):
        eng.memset(ap3[:, 0:1, :], 0.0)
        eng.memset(ap3[:, pw - 1:pw, :], 0.0)
        eng.memset(ap3[:, 1:pw - 1, 0:1], 0.0)
        eng.memset(ap3[:, 1:pw - 1, pw - 1:pw], 0.0)

    cat13 = _r3(cat1[:], 50, 50)
    p1p3 = _r3(p1p[:], 26, 26)
    cat23 = _r3(cat2[:], 26, 26)
    p2p3 = _r3(p2p[:], 14, 14)
    btp3 = _r3(btp[:], 14, 14)
    d2p3 = _r3(d2p[:], 26, 26)
    d1p3 = _r3(d1p[:], 50, 50)

    borders(nc.gpsimd, x3cp3, 50)
    borders(nc.gpsimd, cat13, 50)
    borders(nc.gpsimd, p1p3[32:48], 26)
    borders(nc.vector, cat23, 26)
    borders(nc.vector, p2p3[64:96], 14)
    borders(nc.vector, btp3, 14)
    borders(nc.vector, d2p3, 26)
    borders(nc.gpsimd, d1p3, 50)
    # xt ch6 = x2 rows 0..24 straight from dram (after the xt memset)
    nc.gpsimd.dma_start(xt3[6:7, 1:26, 1:49], bX3)

    pconv = ctx.enter_context(tc.tile_pool(name="pconv", bufs=2, space="PSUM"))
    pmain = ctx.enter_context(tc.tile_pool(name="pmain", bufs=2, space="PSUM"))


    # ------------------------------------------------------------ helpers
    def conv9(src3, wtile, cin, cout, row_chunks, w_, func, bias_ap, dst3,
              pbase=0, obase=0):
        tp = (pbase, obase) if (pbase or obase) else None
        s3 = src3[pbase:pbase + cin]
        r0 = 0
        for nr in row_chunks:
            ps = pconv.tile([obase + cout, nr * w_], F32, tag="conv",
                            name="convps")
            for s in range(9):
                dy, dx = s // 3, s % 3
                nc.tensor.matmul(ps[obase:obase + cout, :],
                                 wtile[:, s * cout:(s + 1) * cout],
                                 s3[:, r0 + dy:r0 + dy + nr, dx:dx + w_],
                                 start=(s == 0), stop=(s == 8),
                                 tile_position=tp)
            nc.scalar.activation(dst3[obase:obase + cout,
                                      1 + r0:1 + r0 + nr, 1:1 + w_],
                                 _r3(ps[obase:obase + cout, :], nr, w_),
                                 func, bias=bias_ap)
            r0 += nr

    def pool2(src3, dst3, orows, ocols, pbase, cch):
        t1 = tmp.tile([pbase + cch, orows * ocols], F16, tag="pool_a",
                      name="poolt1")
        t2 = tmp.tile([pbase + cch, orows * ocols], F16, tag="pool_b",
                      name="poolt2")
        s3 = src3[pbase:pbase + cch]
        v = [s3[:, 1 + a:1 + a + 2 * orows:2, 1 + b:1 + b + 2 * ocols:2]
             for a, b in ((0, 0), (1, 1), (0, 1), (1, 0))]
        t13 = _r3(t1[pbase:pbase + cch, :], orows, ocols)
        t23 = _r3(t2[pbase:pbase + cch, :], orows, ocols)
        nc.vector.tensor_max(t13, v[0], v[1])
        nc.vector.tensor_max(t23, v[2], v[3])
        nc.vector.tensor_max(dst3[pbase:pbase + cch, 1:1 + orows,
                                  1:1 + ocols], t13, t23)

    def up2(src3, sbase, dst3, dbase, cch, irows, icols):
        s = src3[sbase:sbase + cch, 1:1 + irows, 1:1 + icols]
        for a in (0, 1):
            for b in (0, 1):
                nc.vector.tensor_copy(
                    dst3[dbase:dbase + cch, 1 + a:1 + a + 2 * irows:2,
                         1 + b:1 + b + 2 * icols:2], s)

    def embconv(imbuf, chunks, dst, scale):
        off = 0
        for cw in chunks:
            ps = pconv.tile([100, cw], F32, tag="conv", name="convps")
            nc.tensor.matmul(ps[:], wt["emb"], imbuf[:, off:off + cw],
                             start=True, stop=True)
            nc.scalar.activation(dst[0:100, off:off + cw], ps[:],
                                 AF.Copy, scale=scale)
            off += cw

    def sqhilo(src, chunks, lhsT, esq, hib, lob, ex, row):
        """rows(row, row+1) of ex = NEGATED hi/lo fp16 pair of
        lhsT.T @ Square(src)."""
        n = sum(chunks)
        off = 0
        for cw in chunks:
            nc.scalar.activation(esq[:, off:off + cw],
                                 src[0:100, off:off + cw], AF.Square)
            ps = pconv.tile([1, cw], F32, tag="conv", name="sqps")
            nc.tensor.matmul(ps[:], lhsT, esq[:, off:off + cw],
                             start=True, stop=True)
            nc.scalar.activation(hib[0:1, off:off + cw], ps[:], AF.Copy,
                                 scale=-1.0)
            nc.vector.scalar_tensor_tensor(lob[0:1, off:off + cw], ps[:],
                                           -1.0, hib[0:1, off:off + cw],
                                           op0=ALU.mult, op1=ALU.subtract)
            off += cw
        nc.sync.dma_start(ex[row:row + 1, 0:n], hib[0:1, 0:n])
        nc.sync.dma_start(ex[row + 1:row + 2, 0:n], lob[0:1, 0:n])

    # --------------------------------------------- U-Net + emb interleave
    # enc1: im2col matmuls -> cat1[32:48] (tile_position col offset 32)
    r0 = 0
    for nr in (10, 10, 10, 8):
        ps = pconv.tile([48, nr * W], F32, tag="conv", name="convps")
        nc.tensor.matmul(ps[32:48, :], wt["enc1s"],
                         im27[:, r0 * W:(r0 + nr) * W],
                         start=True, stop=True, tile_position=(0, 32))
        nc.scalar.activation(cat13[32:48, 1 + r0:1 + r0 + nr, 1:1 + W],
                             _r3(ps[32:48, :], nr, W), AF.Relu,
                             bias=bia["enc1@32"])
        r0 += nr
    embconv(im2c12[0:10, 0:NREF], RECH, e1x[:], 2.0)       # PE filler
    pool2(cat13, p1p3, 19, 24, 32, 16)
    conv9(p1p3, wt["enc2"], 16, 32, [18], 24, AF.Relu, bia["enc2@64"],
          cat23, pbase=32, obase=64)
    embconv(im2c12[0:10, NREF:2 * NREF], RECH, e2x[:], 2.0)
    pool2(cat23, p2p3, 9, 12, 64, 32)
    conv9(p2p3, wt["bott"], 32, 64, [8], 12, AF.Relu, bia["bott"], btp3,
          pbase=64, obase=0)
    sqhilo(e1x[:], RECH, c025[:], esq1[:], hib1, lob1, e1x[:], 100)
    up2(btp3, 0, cat23, 0, 64, 8, 12)
    conv9(cat23, wt["dec2"], 96, 32, [14], 24, AF.Relu, bia["dec2"], d2p3)
    sqhilo(e2x[:], RECH, c025[:], esq2[:], hib2, lob2, e2x[:], 100)
    up2(d2p3, 0, cat13, 0, 32, 14, 24)
    conv9(cat13, wt["dec1"], 48, 16, [10, 10, 6], W, AF.Relu, bia["dec1"],
          d1p3)

    # 1x1 output conv -> xt[0:4] (all 4 channels) and x3cp (class-c channel)
    r0 = 0
    for nr in (10, 10, 6):
        rhs = d1p3[:, 1 + r0:1 + r0 + nr, 1:1 + W]
        psc = pconv.tile([1, nr * W], F32, tag="conv", name="convps")
        nc.tensor.matmul(psc[:], wt["outc"], rhs, start=True, stop=True)
        nc.scalar.activation(x3cp3[0:1, 1 + r0:1 + r0 + nr, 1:1 + W],
                             _r3(psc[:], nr, W), AF.Identity,
                             bias=bia["outc"])
        ps = pconv.tile([4, nr * W], F32, tag="conv", name="convps")
        nc.tensor.matmul(ps[:], wt["out"], rhs, start=True, stop=True)
        nc.scalar.activation(xt3[0:4, 1 + r0:1 + r0 + nr, 1:1 + W],
                             _r3(ps[:], nr, W), AF.Identity, bias=bia["out"])
        r0 += nr

    # ------------------------------------------------------- embedding 3
    for s in range(9):
        dy, dx = s // 3, s % 3
        eng = nc.sync if s % 2 == 0 else nc.gpsimd
        eng.dma_start(im2c3[s:s + 1, 0:Q],
                      x3cp3[0:1, dy:dy + QROWS, dx:dx + W])
    embconv(im2c3[:], E3CH, e3x[:], 1.0)
    sqhilo(e3x[:], E3CH, c1[:, 0:1], esq3[:], hib3, lob3, e3x[:], 102)

    # ------------------------------------------------------- matching
    nc.vector.memset(accDG[:], -60000.0)
    nc.vector.memset(accAG[:], -60000.0)
    nc.vector.memset(accDL[:], -60000.0)
    nc.vector.memset(accAL[:], -60000.0)

    def match_chunks(ex, accD, accA, k_range):
        for k in k_range:
            lhsT = ex[:][:, k * 128:(k + 1) * 128]
            ps = pmain.tile([128, Q], F32, tag="main", name="mainps")
            for o, n in ((0, 512), (512, 512), (1024, 176)):
                nc.tensor.matmul(ps[:, o:o + n], lhsT,
                                 e3x[:, o:o + n], start=True, stop=True)
            if k % 4 == 0:          # DVE drains PSUM directly (1x rate)
                nc.vector.tensor_max(accD[:], ps[:], accD[:])
            else:                    # ACT copies; DVE merges all-fp16 (2x)
                sc = scr.tile([128, Q], F16, tag="scr", name="scrt")
                nc.scalar.copy(sc[:], ps[:])
                nc.vector.tensor_max(accA[:], sc[:], accA[:])

    def ref_finals(accD, accA, accM, qmax):
        nc.vector.tensor_max(accM[:], accD[:], accA[:])

    def ref_transposes(accM, qmax):
        for i in range(NQC):
            n = 128 if i < 9 else 48
            pst = pconv.tile([n, 128], F16, tag="conv", name="trps")
            nc.tensor.transpose(pst[:], accM[:, i * 128:i * 128 + n],
                                ident[:128, :128])
            nc.vector.tensor_reduce(qmax[0:n, i:i + 1], pst[:],
                                    axis=AX.X, op=ALU.max)

    def ref_plane(qmax, r, eng):
        tneg = small.tile([128, NQC], F32, tag="tneg", name="tneg")
        nc.vector.tensor_scalar(tneg[:], qmax[:], 0.0, 0.0,
                                op0=ALU.min, op1=ALU.min)
        gcol = small.tile([128, NQC], F16, tag="gcol", name="gcol")
        nc.scalar.activation(gcol[:], tneg[:], AF.Tanh, scale=-0.5)
        pst = pconv.tile([NQC, 128], F16, tag="conv", name="gmtps")
        nc.tensor.transpose(pst[:], gcol[:], ident[:128, :128])
        gcolT = small.tile([NQC, 128], F16, tag="gcolT", name="gcolT")
        nc.scalar.copy(gcolT[:], pst[:])
        gflat = small.tile([1, Q], F16, tag="gflat", name="gflat")
        eng.dma_start(gflat[0:1, 0:1152], gcolT[0:9, :])
        eng.dma_start(gflat[0:1, 1152:1200], gcolT[9:10, 0:48])
        eng.dma_start(xt3[4 + r:5 + r, 1:26, 1:49], gflat[:])

    qmaxG = small.tile([128, NQC], F32, tag="qmaxG", name="qmaxG")
    qmaxL = small.tile([128, NQC], F32, tag="qmaxL", name="qmaxL")

    match_chunks(e1x, accDG, accAG, range(18))     # gm
    ref_finals(accDG, accAG, accMG, qmaxG)         # DVE combine (overlaps lm)
    match_chunks(e2x, accDL, accAL, range(4))
    ref_transposes(accMG, qmaxG)                   # PE slots between lm MMs
    match_chunks(e2x, accDL, accAL, range(4, 18))
    ref_plane(qmaxG, 0, nc.gpsimd)
    ref_finals(accDL, accAL, accML, qmaxL)
    ref_transposes(accML, qmaxL)
    ref_plane(qmaxL, 1, nc.gpsimd)

    # ------------------------------------------------------- head conv
    r0 = 0
    for nr in (8, 8, 8):
        ps = pconv.tile([1, nr * W], F32, tag="conv", name="convps")
        for s in range(9):
            dy, dx = s // 3, s % 3
            nc.tensor.matmul(ps[:], wt["dshc"][:, s:s + 1],
                             xt3[:, r0 + dy:r0 + dy + nr, dx:dx + W],
                             start=(s == 0), stop=(s == 8))
        nc.scalar.activation(out_sb[0:1, r0 * W:(r0 + nr) * W],
                             _r3(ps[:], nr, W), AF.Identity, bias=bia["dsh"])
        r0 += nr
    nc.sync.dma_start(out_d, out_sb[:])


def build_program():
    import contextlib
    nc = bacc.Bacc("TRN2", target_bir_lowering=False, debug=False,
                   num_devices=8)
    with tile.TileContext(nc) as tc:
        with contextlib.ExitStack() as ctx:
            _emit(nc, tc, ctx)
    nc.compile()
    return nc


def _get_program():
    global _PROG
    if _PROG is None:
        _PROG = build_program()
    return _PROG


CORE_BC = [(0, 2), (0, 3), (1, 2), (1, 3)]


def _wT_flat(w):
    """[Cout, Cin, 3, 3] -> [Cin, 9*Cout]: col block s holds w[:, :, s//3, s%3].T"""
    cout, cin = w.shape[:2]
    out = np.zeros((cin, 9 * cout), np.float32)
    for s in range(9):
        out[:, s * cout:(s + 1) * cout] = w[:, :, s // 3, s % 3].T
    return out


def _pad50(img):
    out = np.zeros((50, 50), np.float32)
    out[1:49, 1:49] = img
    return out


def _im2col9(img, rows, ones_row=False):
    """padded 50x50 -> [9(+1), rows*48] rows ordered s=dy*3+dx."""
    p = _pad50(img)
    rws = [p[dy:dy + rows, dx:dx + W].ravel()
           for dy in range(3) for dx in range(3)]
    if ones_row:
        rws.append(np.ones(rows * W, np.float32))
    return np.stack(rws)


def _blobs(inp, flip, c):
    w = {k: (inp[k][:, :, ::-1, :] if flip else inp[k])
         for k in ["enc1_w", "enc2_w", "bott_w", "dec2_w", "dec1_w",
                   "emb_w", "dsh_w"]}
    seg = {}
    seg["enc1s"] = w["enc1_w"].reshape(16, 3, 9).transpose(2, 1, 0) \
                              .reshape(27, 16)
    seg["enc2"] = _wT_flat(w["enc2_w"])
    seg["bott"] = _wT_flat(w["bott_w"])
    seg["dec2"] = _wT_flat(w["dec2_w"])
    seg["dec1"] = _wT_flat(w["dec1_w"])
    seg["out"] = inp["out_w"][:, :, 0, 0].T
    seg["outc"] = inp["out_w"][c, :, 0, 0][:, None]
    seg["emb"] = np.vstack([w["emb_w"].reshape(100, 9).T,
                            inp["emb_b"][None, :]])
    seg["dshc"] = w["dsh_w"].reshape(7, 9)

    def pack(segs, ncols):
        blob = np.zeros((96, ncols), np.float16)
        off = 0
        for nm, r0, rows, cols in segs:
            blob[r0:r0 + rows, off:off + cols] = seg[nm].astype(np.float16)
            off += cols
        return blob

    blobbias = np.zeros((96, 10), np.float32)
    for nm, col in BIAS_COL.items():
        if nm == "outc":
            v = inp["out_b"][c:c + 1]
        else:
            v = inp[nm + "_b"]
        blobbias[0:len(v), col] = v
    blobbias[32:48, 8] = inp["enc1_b"]
    blobbias[64:96, 9] = inp["enc2_b"]
    return pack(ASEGS, ACOLS), pack(BSEGS, BCOLS), blobbias


def make_in_maps(inp):
    maps = []
    for k8 in range(8):
        n_idx, half = k8 // 2, k8 % 2
        b, c = CORE_BC[n_idx]
        x1c, x2c, x3b = inp["x1"][b, c], inp["x2"][b, c], inp["x3"][b]
        if half:
            x1c, x2c, x3b = x1c[::-1], x2c[::-1], x3b[:, ::-1]
        bx1 = np.zeros((27, 39 * W), np.float32)
        for ci in range(3):
            im9 = _im2col9(x3b[ci], 39)
            for s in range(9):
                bx1[s * 3 + ci] = im9[s]
        bx2 = np.concatenate([_im2col9(x1c, H, True),
                              _im2col9(x2c, H, True)], axis=1)
        bx3 = x2c[0:25, :].reshape(1, Q)
        blobA, blobB, blobbias = _blobs(inp, bool(half), c)
        maps.append({"blobA": blobA, "blobB": blobB,
                     "blobBias": blobbias,
                     "blobX1": np.ascontiguousarray(bx1.astype(np.float16)),
                     "blobX2": np.ascontiguousarray(bx2.astype(np.float16)),
                     "blobX3": np.ascontiguousarray(bx3.astype(np.float16))})
    return maps


def assemble(results):
    out = np.zeros((2, 2, H, W), np.float32)
    for k8, r in enumerate(results):
        n_idx, half = k8 // 2, k8 % 2
        b, c = CORE_BC[n_idx]
        y = r["out"].reshape(24, W)
        if half == 0:
            out[b, c - 2, 0:24] = y
        else:
            out[b, c - 2, 24:48] = y[::-1]
    return out


def kernel(**inputs):
    inp = {k: np.asarray(v) for k, v in inputs.items()}
    nc = _get_program()
    maps = make_in_maps(inp)
    res = run_bass_kernel_spmd(nc, maps, core_ids=list(range(8)), trace=False)
    return assemble(res.results)


# revision 22
# speedup vs baseline: 1.0710x; 1.0190x over previous
"""FEELVOS fused kernel for TRN2, 8-core SPMD — fp16 rev3.

Sharding: the reference only returns logits for classes C-2, C-1, so only 4 of
the 8 fused (batch, class) items matter. 8 cores = 4 (b, c) pairs x 2 frame
halves (top/bottom 24 rows). Bottom-half cores receive row-flipped inputs and
row-flipped conv kernels so every core runs the identical program computing
"top 25 rows" of its (possibly flipped) frame; the host un-flips on gather.

rev5: ref-major matching. The distance matmul puts 128 REF pixels on the
PSUM partitions and all 1200 queries on the free axis, NEGATED (m = 2ab -
|a|^2 - |b|^2 = -d^2, via sign flips host/device), so the reduction over
refs is a MAX. 18 ref chunks are folded by elementwise max-merges into two
fp16 accumulators: even chunks merge straight from PSUM on DVE; odd chunks
are copied PSUM->fp16 by ACT and merged by GPSIMD (SBUF-only engine). This
splits the PSUM-drain floor (~1.1 ns/elem/lane, dtype-independent on any
single engine) across three engines. The final 128-partition max per query
goes through PE transposes + short DVE reduces.

All matmul operands fp16; PSUM fp32. K=104 carries |a|^2 (rows 100/101 of
the ref operand, negated hi+lo pair, vs ones) and |b|^2 (rows 102/103 of
the query operand, negated hi+lo, vs ones), so -d^2 comes out of the
matmul complete.

U-Net decoder convs read single concatenated-K tiles: skip connections are
written at partition offsets via matmul tile_position (enc1 -> cat1[32:48],
enc2 -> cat2[64:96]), halving decoder matmuls. Embedding/square matmuls are
interleaved between U-Net layers to keep the PE busy (HAM warm). The head
conv runs directly on the padded [7,2500] xt plane (9 shifted K=7 matmuls).
"""
import numpy as np

import concourse.bass as bass
import concourse.bacc as bacc
import concourse.tile as tile
from concourse import mybir
from concourse.bass_utils import run_bass_kernel_spmd
from concourse.masks import make_identity

F32 = mybir.dt.float32
F16 = mybir.dt.float16
AF = mybir.ActivationFunctionType
ALU = mybir.AluOpType
AX = mybir.AxisListType

H = W = 48
NREF = H * W                 # 2304 ref pixels (full frame)
QROWS = 25
Q = QROWS * W                # 1200 query pixels
QCH, NQC = 120, 10           # query chunking for the distance matmul
RECH = [512, 512, 512, 512, 256]  # 2304 column chunking (PSUM bank)
E3CH = [432, 384, 384]       # 1200 column chunking
_PROG = None


def _r3(ap, h, w):
    return ap.rearrange("c (h w) -> c h w", h=h, w=w)


# blob column layouts: (name, row0, nrows, cols), fp16. Row offsets place
# weights at the partition base their matmul's contraction rows need.
ASEGS = [("enc1s", 0, 27, 16), ("enc2", 32, 16, 288), ("bott", 64, 32, 576)]
BSEGS = [("dec2", 0, 96, 288), ("dec1", 0, 48, 144), ("out", 0, 16, 4),
         ("outc", 0, 16, 1), ("emb", 0, 10, 100), ("dshc", 0, 7, 9)]


def _offsets(segs):
    off, o = {}, 0
    for nm, _r0, _r, c in segs:
        off[nm] = o
        o += c
    return off, o


AOFF, ACOLS = _offsets(ASEGS)
BOFF, BCOLS = _offsets(BSEGS)
BIAS_COL = {"enc1": 0, "enc2": 1, "bott": 2, "dec2": 3, "dec1": 4,
            "out": 5, "dsh": 6, "outc": 7}


def _emit(nc, tc, ctx):
    # ------------------------------------------------------------- dram io
    bA = nc.dram_tensor("blobA", [96, ACOLS], F16, kind="ExternalInput").ap()
    bB = nc.dram_tensor("blobB", [96, BCOLS], F16, kind="ExternalInput").ap()
    bBias = nc.dram_tensor("blobBias", [96, 10], F32,
                           kind="ExternalInput").ap()
    bX1 = nc.dram_tensor("blobX1", [27, 39 * W], F16,
                         kind="ExternalInput").ap()
    bX2 = nc.dram_tensor("blobX2", [10, 2 * NREF], F16,
                         kind="ExternalInput").ap()
    bX3 = nc.dram_tensor("blobX3", [1, Q], F16, kind="ExternalInput").ap()
    out_d = nc.dram_tensor("out", [1, 24 * W], F32, kind="ExternalOutput").ap()

    # ------------------------------------------------------------- sbuf
    sb = ctx.enter_context(tc.tile_pool(name="sb", bufs=1))

    def st(name, p, f, dt=F16):
        return sb.tile([p, f], dt, tag=name, name=name)

    bloba = st("bloba", 96, ACOLS)
    blobb = st("blobb", 96, BCOLS)
    blobbias = st("blobbias", 96, 10, F32)
    im27 = st("im27", 27, 39 * W)        # enc1 im2col (host-built)
    im2c12 = st("im2c12", 10, 2 * NREF)  # emb im2col e1|e2 (host-built)

    def wseg(blob, off, segs, nm):
        r0, rows, cols = next((a, b, c) for n, a, b, c in segs if n == nm)
        return blob[r0:r0 + rows, off[nm]:off[nm] + cols]

    wt = {nm: wseg(bloba, AOFF, ASEGS, nm) for nm, _, _, _ in ASEGS}
    wt.update({nm: wseg(blobb, BOFF, BSEGS, nm) for nm, _, _, _ in BSEGS})
    bia = {nm: blobbias[0:r, c:c + 1]
           for nm, (r, c) in {"enc1": (16, 0), "enc2": (32, 1),
                              "bott": (64, 2), "dec2": (32, 3),
                              "dec1": (16, 4), "out": (4, 5),
                              "dsh": (1, 6), "outc": (1, 7)}.items()}
    bia["enc1@32"] = blobbias[32:48, 8:9]
    bia["enc2@64"] = blobbias[64:96, 9:10]

    # device-written padded planes (fp16)
    x3cp = st("x3cp", 1, 2500)
    cat1 = st("cat1", 48, 2500)   # rows 0..31 up(d2), rows 32..47 e1
    p1p = st("p1p", 48, 676)      # rows 32..47 used
    cat2 = st("cat2", 96, 676)    # rows 0..63 up(bt), rows 64..95 e2
    p2p = st("p2p", 96, 196)      # rows 64..95 used
    btp = st("btp", 64, 196)
    d2p = st("d2p", 32, 676)
    d1p = st("d1p", 16, 2500)
    xt = st("xt", 7, 2500)

    im2c3 = st("im2c3", 10, Q)           # emb im2col (e3)
    e1x = st("e1x", 102, NREF)           # 0..99 +2*e1, 100/101 -|a|^2 hi/lo
    e2x = st("e2x", 102, NREF)
    e3x = st("e3x", 102, Q)              # 0..99 e3, 100/101 ones
    esq1 = st("esq1", 100, NREF)
    esq2 = st("esq2", 100, NREF)
    esq3 = st("esq3", 100, Q)
    ident = st("ident", 128, 128)

    c025 = st("c025", 100, 1)
    c1 = st("c1", 100, 2)
    hib1 = st("hib1", 1, NREF)
    lob1 = st("lob1", 1, NREF)
    hib2 = st("hib2", 1, NREF)
    lob2 = st("lob2", 1, NREF)
    b2T = st("b2T", 128, NQC, F32)       # |b|^2 in transposed query layout
    accDG = st("accDG", 128, Q)          # fp16 running-max accumulators
    accAG = st("accAG", 128, Q)
    accDL = st("accDL", 128, Q)
    accAL = st("accAL", 128, Q)
    accMG = st("accMG", 128, Q)
    accML = st("accML", 128, Q)
    out_sb = st("out_sb", 1, 24 * W, F32)

    small = ctx.enter_context(tc.tile_pool(name="small", bufs=8))
    tmp = ctx.enter_context(tc.tile_pool(name="tmp", bufs=2))
    scr = ctx.enter_context(tc.tile_pool(name="scr", bufs=3))

    # ------------------------------------------------------------- input dma
    # exact-row segment DMAs, hot-first
    def seg_dma(blobt, blobd, off, segs, nm):
        r0, rows, cols = next((a, b, c) for n, a, b, c in segs if n == nm)
        nc.sync.dma_start(blobt[r0:r0 + rows, off[nm]:off[nm] + cols],
                          blobd[r0:r0 + rows, off[nm]:off[nm] + cols])

    seg_dma(bloba, bA, AOFF, ASEGS, "enc1s")
    nc.sync.dma_start(blobbias[:], bBias)
    nc.sync.dma_start(im27[:, 0:960], bX1[:, 0:960])        # enc1 rows 0..19
    seg_dma(bloba, bA, AOFF, ASEGS, "enc2")
    seg_dma(blobb, bB, BOFF, BSEGS, "emb")
    nc.sync.dma_start(im27[:, 960:39 * W], bX1[:, 960:39 * W])
    nc.sync.dma_start(im2c12[:, 0:NREF // 2], bX2[:, 0:NREF // 2])
    nc.sync.dma_start(im2c12[:, NREF // 2:NREF], bX2[:, NREF // 2:NREF])
    seg_dma(bloba, bA, AOFF, ASEGS, "bott")
    nc.sync.dma_start(im2c12[:, NREF:NREF + NREF // 2],
                      bX2[:, NREF:NREF + NREF // 2])
    nc.sync.dma_start(im2c12[:, NREF + NREF // 2:2 * NREF],
                      bX2[:, NREF + NREF // 2:2 * NREF])
    seg_dma(blobb, bB, BOFF, BSEGS, "dec2")
    nc.sync.dma_start(blobb[0:48, BOFF["dec1"]:BCOLS],
                      bB[0:48, BOFF["dec1"]:BCOLS])  # dec1+out+outc+dshc

    # ------------------------------------------------------------- init
    make_identity(nc, ident[:])
    nc.gpsimd.memset(c025[:], 0.25)
    nc.gpsimd.memset(c1[:], 1.0)
    nc.gpsimd.memset(xt[:], 0.0)
    # engine partition starts must be 32-aligned; rows 96..99 / 0..8 are
    # overwritten later by the embconv ACT / shift DMAs; rows 100/101 of
    # e1x/e2x and 102/103 of e3x by the hi/lo DMAs.
    nc.vector.memset(e1x[96:102, :], 1.0)
    nc.vector.memset(e2x[96:102, :], 1.0)
    nc.vector.memset(e3x[96:102, :], 1.0)
    nc.vector.memset(im2c3[0:10, :], 1.0)

    xt3 = _r3(xt[:], 50, 50)
    x3cp3 = _r3(x3cp[:], 50, 50)

    def borders(eng, ap3, pw):
        eng.memset(ap3[:, 0:1, :], 0.0)
        eng.memset(ap3[:, pw - 1:pw, :], 0.0)
        eng.memset(ap3[:, 1:pw - 1, 0:1], 0.0)
        eng.memset(ap3[:, 1:pw - 1, pw - 1:pw], 0.0)

    cat13 = _r3(cat1[:], 50, 50)
    p1p3 = _r3(p1p[:], 26, 26)
    cat23 = _r3(cat2[:], 26, 26)
    p2p3 = _r3(p2p[:], 14, 14)
    btp3 = _r3(btp[:], 14, 14)
    d2p3 = _r3(d2p[:], 26, 26)
    d1p3 = _r3(d1p[:], 50, 50)

    borders(nc.gpsimd, x3cp3, 50)
    borders(nc.gpsimd, cat13, 50)
    borders(nc.gpsimd, p1p3[32:48], 26)
    borders(nc.vector, cat23, 26)
    borders(nc.vector, p2p3[64:96], 14)
    borders(nc.vector, btp3, 14)
    borders(nc.vector, d2p3, 26)
    borders(nc.gpsimd, d1p3, 50)
    # xt ch6 = x2 rows 0..24 straight from dram (after the xt memset)
    nc.gpsimd.dma_start(xt3[6:7, 1:26, 1:49], bX3)

    pconv = ctx.enter_context(tc.tile_pool(name="pconv", bufs=2, space="PSUM"))
    pmain = ctx.enter_context(tc.tile_pool(name="pmain", bufs=2, space="PSUM"))


    # ------------------------------------------------------------ helpers
    def conv9(src3, wtile, cin, cout, row_chunks, w_, func, bias_ap, dst3,
              pbase=0, obase=0):
        tp = (pbase, obase) if (pbase or obase) else None
        s3 = src3[pbase:pbase + cin]
        r0 = 0
        for nr in row_chunks:
            ps = pconv.tile([obase + cout, nr * w_], F32, tag="conv",
                            name="convps")
            for s in range(9):
                dy, dx = s // 3, s % 3
                nc.tensor.matmul(ps[obase:obase + cout, :],
                                 wtile[:, s * cout:(s + 1) * cout],
                                 s3[:, r0 + dy:r0 + dy + nr, dx:dx + w_],
                                 start=(s == 0), stop=(s == 8),
                                 tile_position=tp)
            nc.scalar.activation(dst3[obase:obase + cout,
                                      1 + r0:1 + r0 + nr, 1:1 + w_],
                                 _r3(ps[obase:obase + cout, :], nr, w_),
                                 func, bias=bias_ap)
            r0 += nr

    def pool2(src3, dst3, orows, ocols, pbase, cch):
        t1 = tmp.tile([pbase + cch, orows * ocols], F16, tag="pool_a",
                      name="poolt1")
        t2 = tmp.tile([pbase + cch, orows * ocols], F16, tag="pool_b",
                      name="poolt2")
        s3 = src3[pbase:pbase + cch]
        v = [s3[:, 1 + a:1 + a + 2 * orows:2, 1 + b:1 + b + 2 * ocols:2]
             for a, b in ((0, 0), (1, 1), (0, 1), (1, 0))]
        t13 = _r3(t1[pbase:pbase + cch, :], orows, ocols)
        t23 = _r3(t2[pbase:pbase + cch, :], orows, ocols)
        nc.vector.tensor_max(t13, v[0], v[1])
        nc.vector.tensor_max(t23, v[2], v[3])
        nc.vector.tensor_max(dst3[pbase:pbase + cch, 1:1 + orows,
                                  1:1 + ocols], t13, t23)

    def up2(src3, sbase, dst3, dbase, cch, irows, icols):
        s = src3[sbase:sbase + cch, 1:1 + irows, 1:1 + icols]
        for a in (0, 1):
            for b in (0, 1):
                nc.vector.tensor_copy(
                    dst3[dbase:dbase + cch, 1 + a:1 + a + 2 * irows:2,
                         1 + b:1 + b + 2 * icols:2], s)

    def embconv(imbuf, chunks, dst, scale):
        off = 0
        for cw in chunks:
            ps = pconv.tile([100, cw], F32, tag="conv", name="convps")
            nc.tensor.matmul(ps[:], wt["emb"], imbuf[:, off:off + cw],
                             start=True, stop=True)
            nc.scalar.activation(dst[0:100, off:off + cw], ps[:],
                                 AF.Copy, scale=scale)
            off += cw

    def sqhilo(src, chunks, lhsT, esq, hib, lob, ex, row):
        """rows(row, row+1) of ex = NEGATED hi/lo fp16 pair of
        lhsT.T @ Square(src)."""
        n = sum(chunks)
        off = 0
        for cw in chunks:
            nc.scalar.activation(esq[:, off:off + cw],
                                 src[0:100, off:off + cw], AF.Square)
            ps = pconv.tile([1, cw], F32, tag="conv", name="sqps")
            nc.tensor.matmul(ps[:], lhsT, esq[:, off:off + cw],
                             start=True, stop=True)
            nc.scalar.activation(hib[0:1, off:off + cw], ps[:], AF.Copy,
                                 scale=-1.0)
            nc.vector.scalar_tensor_tensor(lob[0:1, off:off + cw], ps[:],
                                           -1.0, hib[0:1, off:off + cw],
                                           op0=ALU.mult, op1=ALU.subtract)
            off += cw
        nc.sync.dma_start(ex[row:row + 1, 0:n], hib[0:1, 0:n])
        nc.sync.dma_start(ex[row + 1:row + 2, 0:n], lob[0:1, 0:n])

    # --------------------------------------------- U-Net + emb interleave
    # enc1: im2col matmuls -> cat1[32:48] (tile_position col offset 32)
    r0 = 0
    for nr in (10, 10, 10, 8):
        ps = pconv.tile([48, nr * W], F32, tag="conv", name="convps")
        nc.tensor.matmul(ps[32:48, :], wt["enc1s"],
                         im27[:, r0 * W:(r0 + nr) * W],
                         start=True, stop=True, tile_position=(0, 32))
        nc.scalar.activation(cat13[32:48, 1 + r0:1 + r0 + nr, 1:1 + W],
                             _r3(ps[32:48, :], nr, W), AF.Relu,
                             bias=bia["enc1@32"])
        r0 += nr
    embconv(im2c12[0:10, 0:NREF], RECH, e1x[:], 2.0)       # PE filler
    pool2(cat13, p1p3, 19, 24, 32, 16)
    conv9(p1p3, wt["enc2"], 16, 32, [18], 24, AF.Relu, bia["enc2@64"],
          cat23, pbase=32, obase=64)
    embconv(im2c12[0:10, NREF:2 * NREF], RECH, e2x[:], 2.0)
    pool2(cat23, p2p3, 9, 12, 64, 32)
    conv9(p2p3, wt["bott"], 32, 64, [8], 12, AF.Relu, bia["bott"], btp3,
          pbase=64, obase=0)
    sqhilo(e1x[:], RECH, c025[:], esq1[:], hib1, lob1, e1x[:], 100)
    up2(btp3, 0, cat23, 0, 64, 8, 12)
    conv9(cat23, wt["dec2"], 96, 32, [14], 24, AF.Relu, bia["dec2"], d2p3)
    sqhilo(e2x[:], RECH, c025[:], esq2[:], hib2, lob2, e2x[:], 100)
    up2(d2p3, 0, cat13, 0, 32, 14, 24)
    conv9(cat13, wt["dec1"], 48, 16, [10, 10, 6], W, AF.Relu, bia["dec1"],
          d1p3)

    # 1x1 output conv -> xt[0:4] (all 4 channels) and x3cp (class-c channel)
    r0 = 0
    for nr in (10, 10, 6):
        rhs = d1p3[:, 1 + r0:1 + r0 + nr, 1:1 + W]
        psc = pconv.tile([1, nr * W], F32, tag="conv", name="convps")
        nc.tensor.matmul(psc[:], wt["outc"], rhs, start=True, stop=True)
        nc.scalar.activation(x3cp3[0:1, 1 + r0:1 + r0 + nr, 1:1 + W],
                             _r3(psc[:], nr, W), AF.Identity,
                             bias=bia["outc"])
        ps = pconv.tile([4, nr * W], F32, tag="conv", name="convps")
        nc.tensor.matmul(ps[:], wt["out"], rhs, start=True, stop=True)
        nc.scalar.activation(xt3[0:4, 1 + r0:1 + r0 + nr, 1:1 + W],
                             _r3(ps[:], nr, W), AF.Identity, bias=bia["out"])
        r0 += nr

    # ------------------------------------------------------- embedding 3
    for s in range(9):
        dy, dx = s // 3, s % 3
        eng = nc.sync if s % 2 == 0 else nc.gpsimd
        eng.dma_start(im2c3[s:s + 1, 0:Q],
                      x3cp3[0:1, dy:dy + QROWS, dx:dx + W])
    embconv(im2c3[:], E3CH, e3x[:], 1.0)

    # ------------------------------------------------------- matching
    nc.vector.memset(accDG[:], -60000.0)
    nc.vector.memset(accAG[:], -60000.0)
    nc.vector.memset(accDL[:], -60000.0)
    nc.vector.memset(accAL[:], -60000.0)

    def match_chunks(ex, accD, accA, k_range):
        for k in k_range:
            lhsT = ex[:][:, k * 128:(k + 1) * 128]
            ps = pmain.tile([128, Q], F32, tag="main", name="mainps")
            for o, n in ((0, 512), (512, 512), (1024, 176)):
                nc.tensor.matmul(ps[:, o:o + n], lhsT,
                                 e3x[0:102, o:o + n], start=True, stop=True)
            if k % 4 == 0:          # DVE drains PSUM directly (1x rate)
                nc.vector.tensor_max(accD[:], ps[:], accD[:])
            else:                    # ACT copies; DVE merges all-fp16 (2x)
                sc = scr.tile([128, Q], F16, tag="scr", name="scrt")
                nc.scalar.copy(sc[:], ps[:])
                nc.vector.tensor_max(accA[:], sc[:], accA[:])

    def ref_finals(accD, accA, accM, qmax):
        nc.vector.tensor_max(accM[:], accD[:], accA[:])

    def ref_transposes(accM, qmax):
        for i in range(NQC):
            n = 128 if i < 9 else 48
            pst = pconv.tile([n, 128], F16, tag="conv", name="trps")
            nc.tensor.transpose(pst[:], accM[:, i * 128:i * 128 + n],
                                ident[:128, :128])
            nc.vector.tensor_reduce(qmax[0:n, i:i + 1], pst[:],
                                    axis=AX.X, op=ALU.max)

    def ref_plane(qmax, r, eng):
        dsub = small.tile([128, NQC], F32, tag="dsub", name="dsub")
        nc.vector.scalar_tensor_tensor(dsub[:], qmax[:], 1.0, b2T[:],
                                       op0=ALU.mult, op1=ALU.subtract)
        tneg = small.tile([128, NQC], F32, tag="tneg", name="tneg")
        nc.vector.tensor_scalar(tneg[:], dsub[:], 0.0, 0.0,
                                op0=ALU.min, op1=ALU.min)
        gcol = small.tile([128, NQC], F16, tag="gcol", name="gcol")
        nc.scalar.activation(gcol[:], tneg[:], AF.Tanh, scale=-0.5)
        pst = pconv.tile([NQC, 128], F16, tag="conv", name="gmtps")
        nc.tensor.transpose(pst[:], gcol[:], ident[:128, :128])
        gcolT = small.tile([NQC, 128], F16, tag="gcolT", name="gcolT")
        nc.scalar.copy(gcolT[:], pst[:])
        gflat = small.tile([1, Q], F16, tag="gflat", name="gflat")
        eng.dma_start(gflat[0:1, 0:1152], gcolT[0:9, :])
        eng.dma_start(gflat[0:1, 1152:1200], gcolT[9:10, 0:48])
        eng.dma_start(xt3[4 + r:5 + r, 1:26, 1:49], gflat[:])

    qmaxG = small.tile([128, NQC], F32, tag="qmaxG", name="qmaxG")
    qmaxL = small.tile([128, NQC], F32, tag="qmaxL", name="qmaxL")

    match_chunks(e1x, accDG, accAG, range(3))      # gm starts immediately
    # |b|^2 in transposed layout (commutes with the ref-max): Square + 10
    # small matmuls slot between gm chunks; only the qmax tail needs them.
    off = 0
    for cw in E3CH:
        nc.scalar.activation(esq3[:, off:off + cw], e3x[0:100, off:off + cw],
                             AF.Square)
        off += cw
    nc.gpsimd.memset(b2T[:], 0.0)
    for i in range(NQC):
        n = 128 if i < 9 else 48
        b2ps = pconv.tile([128, 2], F32, tag="conv", name="b2ps")
        nc.tensor.matmul(b2ps[0:n, :], esq3[:, i * 128:i * 128 + n], c1[:],
                         start=True, stop=True)
        nc.scalar.copy(b2T[0:n, i:i + 1], b2ps[0:n, 0:1])
    match_chunks(e1x, accDG, accAG, range(3, 18))
    ref_finals(accDG, accAG, accMG, qmaxG)         # DVE combine (overlaps lm)
    match_chunks(e2x, accDL, accAL, range(4))
    ref_transposes(accMG, qmaxG)                   # PE slots between lm MMs
    match_chunks(e2x, accDL, accAL, range(4, 18))
    ref_plane(qmaxG, 0, nc.gpsimd)
    ref_finals(accDL, accAL, accML, qmaxL)
    ref_transposes(accML, qmaxL)
    ref_plane(qmaxL, 1, nc.gpsimd)

    # ------------------------------------------------------- head conv
    r0 = 0
    for nr in (8, 8, 8):
        ps = pconv.tile([1, nr * W], F32, tag="conv", name="convps")
        for s in range(9):
            dy, dx = s // 3, s % 3
            nc.tensor.matmul(ps[:], wt["dshc"][:, s:s + 1],
                             xt3[:, r0 + dy:r0 + dy + nr, dx:dx + W],
                             start=(s == 0), stop=(s == 8))
        nc.scalar.activation(out_sb[0:1, r0 * W:(r0 + nr) * W],
                             _r3(ps[:], nr, W), AF.Identity, bias=bia["dsh"])
        r0 += nr
    nc.sync.dma_start(out_d, out_sb[:])


def build_program():
    import contextlib
    nc = bacc.Bacc("TRN2", target_bir_lowering=False, debug=False,
                   num_devices=8)
    with tile.TileContext(nc) as tc:
        with contextlib.ExitStack() as ctx:
            _emit(nc, tc, ctx)
    nc.compile()
    return nc


def _get_program():
    global _PROG
    if _PROG is None:
        _PROG = build_program()
    return _PROG


CORE_BC = [(0, 2), (0, 3), (1, 2), (1, 3)]


def _wT_flat(w):
    """[Cout, Cin, 3, 3] -> [Cin, 9*Cout]: col block s holds w[:, :, s//3, s%3].T"""
    cout, cin = w.shape[:2]
    out = np.zeros((cin, 9 * cout), np.float32)
    for s in range(9):
        out[:, s * cout:(s + 1) * cout] = w[:, :, s // 3, s % 3].T
    return out


def _pad50(img):
    out = np.zeros((50, 50), np.float32)
    out[1:49, 1:49] = img
    return out


def _im2col9(img, rows, ones_row=False):
    """padded 50x50 -> [9(+1), rows*48] rows ordered s=dy*3+dx."""
    p = _pad50(img)
    rws = [p[dy:dy + rows, dx:dx + W].ravel()
           for dy in range(3) for dx in range(3)]
    if ones_row:
        rws.append(np.ones(rows * W, np.float32))
    return np.stack(rws)


def _blobs(inp, flip, c):
    w = {k: (inp[k][:, :, ::-1, :] if flip else inp[k])
         for k in ["enc1_w", "enc2_w", "bott_w", "dec2_w", "dec1_w",
                   "emb_w", "dsh_w"]}
    seg = {}
    seg["enc1s"] = w["enc1_w"].reshape(16, 3, 9).transpose(2, 1, 0) \
                              .reshape(27, 16)
    seg["enc2"] = _wT_flat(w["enc2_w"])
    seg["bott"] = _wT_flat(w["bott_w"])
    seg["dec2"] = _wT_flat(w["dec2_w"])
    seg["dec1"] = _wT_flat(w["dec1_w"])
    seg["out"] = inp["out_w"][:, :, 0, 0].T
    seg["outc"] = inp["out_w"][c, :, 0, 0][:, None]
    seg["emb"] = np.vstack([w["emb_w"].reshape(100, 9).T,
                            inp["emb_b"][None, :]])
    seg["dshc"] = w["dsh_w"].reshape(7, 9)

    def pack(segs, ncols):
        blob = np.zeros((96, ncols), np.float16)
        off = 0
        for nm, r0, rows, cols in segs:
            blob[r0:r0 + rows, off:off + cols] = seg[nm].astype(np.float16)
            off += cols
        return blob

    blobbias = np.zeros((96, 10), np.float32)
    for nm, col in BIAS_COL.items():
        if nm == "outc":
            v = inp["out_b"][c:c + 1]
        else:
            v = inp[nm + "_b"]
        blobbias[0:len(v), col] = v
    blobbias[32:48, 8] = inp["enc1_b"]
    blobbias[64:96, 9] = inp["enc2_b"]
    return pack(ASEGS, ACOLS), pack(BSEGS, BCOLS), blobbias


def make_in_maps(inp):
    maps = []
    for k8 in range(8):
        n_idx, half = k8 // 2, k8 % 2
        b, c = CORE_BC[n_idx]
        x1c, x2c, x3b = inp["x1"][b, c], inp["x2"][b, c], inp["x3"][b]
        if half:
            x1c, x2c, x3b = x1c[::-1], x2c[::-1], x3b[:, ::-1]
        bx1 = np.zeros((27, 39 * W), np.float32)
        for ci in range(3):
            im9 = _im2col9(x3b[ci], 39)
            for s in range(9):
                bx1[s * 3 + ci] = im9[s]
        bx2 = np.concatenate([_im2col9(x1c, H, True),
                              _im2col9(x2c, H, True)], axis=1)
        bx3 = x2c[0:25, :].reshape(1, Q)
        blobA, blobB, blobbias = _blobs(inp, bool(half), c)
        maps.append({"blobA": blobA, "blobB": blobB,
                     "blobBias": blobbias,
                     "blobX1": np.ascontiguousarray(bx1.astype(np.float16)),
                     "blobX2": np.ascontiguousarray(bx2.astype(np.float16)),
                     "blobX3": np.ascontiguousarray(bx3.astype(np.float16))})
    return maps


def assemble(results):
    out = np.zeros((2, 2, H, W), np.float32)
    for k8, r in enumerate(results):
        n_idx, half = k8 // 2, k8 % 2
        b, c = CORE_BC[n_idx]
        y = r["out"].reshape(24, W)
        if half == 0:
            out[b, c - 2, 0:24] = y
        else:
            out[b, c - 2, 24:48] = y[::-1]
    return out


def kernel(**inputs):
    inp = {k: np.asarray(v) for k, v in inputs.items()}
    nc = _get_program()
    maps = make_in_maps(inp)
    res = run_bass_kernel_spmd(nc, maps, core_ids=list(range(8)), trace=False)
    return assemble(res.results)


# revision 23
# speedup vs baseline: 1.0831x; 1.0113x over previous
"""FEELVOS fused kernel for TRN2, 8-core SPMD — fp16 rev3.

Sharding: the reference only returns logits for classes C-2, C-1, so only 4 of
the 8 fused (batch, class) items matter. 8 cores = 4 (b, c) pairs x 2 frame
halves (top/bottom 24 rows). Bottom-half cores receive row-flipped inputs and
row-flipped conv kernels so every core runs the identical program computing
"top 25 rows" of its (possibly flipped) frame; the host un-flips on gather.

rev5: ref-major matching. The distance matmul puts 128 REF pixels on the
PSUM partitions and all 1200 queries on the free axis, NEGATED (m = 2ab -
|a|^2 - |b|^2 = -d^2, via sign flips host/device), so the reduction over
refs is a MAX. 18 ref chunks are folded by elementwise max-merges into two
fp16 accumulators: even chunks merge straight from PSUM on DVE; odd chunks
are copied PSUM->fp16 by ACT and merged by GPSIMD (SBUF-only engine). This
splits the PSUM-drain floor (~1.1 ns/elem/lane, dtype-independent on any
single engine) across three engines. The final 128-partition max per query
goes through PE transposes + short DVE reduces.

All matmul operands fp16; PSUM fp32. K=104 carries |a|^2 (rows 100/101 of
the ref operand, negated hi+lo pair, vs ones) and |b|^2 (rows 102/103 of
the query operand, negated hi+lo, vs ones), so -d^2 comes out of the
matmul complete.

U-Net decoder convs read single concatenated-K tiles: skip connections are
written at partition offsets via matmul tile_position (enc1 -> cat1[32:48],
enc2 -> cat2[64:96]), halving decoder matmuls. Embedding/square matmuls are
interleaved between U-Net layers to keep the PE busy (HAM warm). The head
conv runs directly on the padded [7,2500] xt plane (9 shifted K=7 matmuls).
"""
import numpy as np

import concourse.bass as bass
import concourse.bacc as bacc
import concourse.tile as tile
from concourse import mybir
from concourse.bass_utils import run_bass_kernel_spmd
from concourse.masks import make_identity

F32 = mybir.dt.float32
F16 = mybir.dt.float16
AF = mybir.ActivationFunctionType
ALU = mybir.AluOpType
AX = mybir.AxisListType

H = W = 48
NREF = H * W                 # 2304 ref pixels (full frame)
QROWS = 25
Q = QROWS * W                # 1200 query pixels
QCH, NQC = 120, 10           # query chunking for the distance matmul
RECH = [512, 512, 512, 512, 256]  # 2304 column chunking (PSUM bank)
E3CH = [432, 384, 384]       # 1200 column chunking
_PROG = None


def _r3(ap, h, w):
    return ap.rearrange("c (h w) -> c h w", h=h, w=w)


# blob column layouts: (name, row0, nrows, cols), fp16. Row offsets place
# weights at the partition base their matmul's contraction rows need.
ASEGS = [("enc1s", 0, 27, 16), ("enc2", 32, 16, 288), ("bott", 64, 32, 576)]
BSEGS = [("dec2", 0, 96, 288), ("dec1", 0, 48, 144), ("out", 0, 16, 4),
         ("outc", 0, 16, 1), ("emb", 0, 10, 100), ("dshc", 0, 7, 9)]


def _offsets(segs):
    off, o = {}, 0
    for nm, _r0, _r, c in segs:
        off[nm] = o
        o += c
    return off, o


AOFF, ACOLS = _offsets(ASEGS)
BOFF, BCOLS = _offsets(BSEGS)
BIAS_COL = {"enc1": 0, "enc2": 1, "bott": 2, "dec2": 3, "dec1": 4,
            "out": 5, "dsh": 6, "outc": 7}


def _emit(nc, tc, ctx):
    # ------------------------------------------------------------- dram io
    bA = nc.dram_tensor("blobA", [96, ACOLS], F16, kind="ExternalInput").ap()
    bB = nc.dram_tensor("blobB", [96, BCOLS], F16, kind="ExternalInput").ap()
    bBias = nc.dram_tensor("blobBias", [96, 10], F32,
                           kind="ExternalInput").ap()
    bX1 = nc.dram_tensor("blobX1", [27, 39 * W], F16,
                         kind="ExternalInput").ap()
    bX2 = nc.dram_tensor("blobX2", [10, 2 * NREF], F16,
                         kind="ExternalInput").ap()
    bX3 = nc.dram_tensor("blobX3", [1, Q], F16, kind="ExternalInput").ap()
    out_d = nc.dram_tensor("out", [1, 24 * W], F32, kind="ExternalOutput").ap()

    # ------------------------------------------------------------- sbuf
    sb = ctx.enter_context(tc.tile_pool(name="sb", bufs=1))

    def st(name, p, f, dt=F16):
        return sb.tile([p, f], dt, tag=name, name=name)

    bloba = st("bloba", 96, ACOLS)
    blobb = st("blobb", 96, BCOLS)
    blobbias = st("blobbias", 96, 10, F32)
    im27 = st("im27", 27, 39 * W)        # enc1 im2col (host-built)
    im2c12 = st("im2c12", 10, 2 * NREF)  # emb im2col e1|e2 (host-built)

    def wseg(blob, off, segs, nm):
        r0, rows, cols = next((a, b, c) for n, a, b, c in segs if n == nm)
        return blob[r0:r0 + rows, off[nm]:off[nm] + cols]

    wt = {nm: wseg(bloba, AOFF, ASEGS, nm) for nm, _, _, _ in ASEGS}
    wt.update({nm: wseg(blobb, BOFF, BSEGS, nm) for nm, _, _, _ in BSEGS})
    bia = {nm: blobbias[0:r, c:c + 1]
           for nm, (r, c) in {"enc1": (16, 0), "enc2": (32, 1),
                              "bott": (64, 2), "dec2": (32, 3),
                              "dec1": (16, 4), "out": (4, 5),
                              "dsh": (1, 6), "outc": (1, 7)}.items()}
    bia["enc1@32"] = blobbias[32:48, 8:9]
    bia["enc2@64"] = blobbias[64:96, 9:10]

    # device-written padded planes (fp16)
    x3cp = st("x3cp", 1, 2500)
    cat1 = st("cat1", 48, 2500)   # rows 0..31 up(d2), rows 32..47 e1
    p1p = st("p1p", 48, 676)      # rows 32..47 used
    cat2 = st("cat2", 96, 676)    # rows 0..63 up(bt), rows 64..95 e2
    p2p = st("p2p", 96, 196)      # rows 64..95 used
    btp = st("btp", 64, 196)
    d2p = st("d2p", 32, 676)
    d1p = st("d1p", 16, 2500)
    xt = st("xt", 7, 2500)

    im2c3 = st("im2c3", 10, Q)           # emb im2col (e3)
    e1x = st("e1x", 102, NREF)           # 0..99 +2*e1, 100/101 -|a|^2 hi/lo
    e2x = st("e2x", 102, NREF)
    e3x = st("e3x", 102, Q)              # 0..99 e3, 100/101 ones
    esq1 = st("esq1", 100, NREF)
    esq2 = st("esq2", 100, NREF)
    esq3 = st("esq3", 100, Q)
    ident = st("ident", 128, 128)

    c025 = st("c025", 100, 1)
    c1 = st("c1", 100, 2)
    hib1 = st("hib1", 1, NREF)
    lob1 = st("lob1", 1, NREF)
    hib2 = st("hib2", 1, NREF)
    lob2 = st("lob2", 1, NREF)
    b2T = st("b2T", 128, NQC, F32)       # |b|^2 in transposed query layout
    accDG = st("accDG", 128, Q)          # fp16 running-max accumulators
    accAG = st("accAG", 128, Q)
    accDL = st("accDL", 128, Q)
    accAL = st("accAL", 128, Q)
    accMG = st("accMG", 128, Q)
    accML = st("accML", 128, Q)
    out_sb = st("out_sb", 1, 24 * W, F32)

    small = ctx.enter_context(tc.tile_pool(name="small", bufs=8))
    tmp = ctx.enter_context(tc.tile_pool(name="tmp", bufs=2))
    scr = ctx.enter_context(tc.tile_pool(name="scr", bufs=3))

    # ------------------------------------------------------------- input dma
    # exact-row segment DMAs, hot-first
    def seg_dma(eng, blobt, blobd, off, segs, nm):
        r0, rows, cols = next((a, b, c) for n, a, b, c in segs if n == nm)
        eng.dma_start(blobt[r0:r0 + rows, off[nm]:off[nm] + cols],
                      blobd[r0:r0 + rows, off[nm]:off[nm] + cols])

    seg_dma(nc.sync, bloba, bA, AOFF, ASEGS, "enc1s")
    nc.gpsimd.dma_start(blobbias[:], bBias)
    nc.sync.dma_start(im27[:, 0:960], bX1[:, 0:960])        # enc1 rows 0..19
    seg_dma(nc.gpsimd, bloba, bA, AOFF, ASEGS, "enc2")
    seg_dma(nc.gpsimd, blobb, bB, BOFF, BSEGS, "emb")
    nc.sync.dma_start(im27[:, 960:39 * W], bX1[:, 960:39 * W])
    nc.sync.dma_start(im2c12[:, 0:NREF // 2], bX2[:, 0:NREF // 2])
    nc.gpsimd.dma_start(im2c12[:, NREF // 2:NREF], bX2[:, NREF // 2:NREF])
    seg_dma(nc.gpsimd, bloba, bA, AOFF, ASEGS, "bott")
    nc.sync.dma_start(im2c12[:, NREF:NREF + NREF // 2],
                      bX2[:, NREF:NREF + NREF // 2])
    nc.gpsimd.dma_start(im2c12[:, NREF + NREF // 2:2 * NREF],
                        bX2[:, NREF + NREF // 2:2 * NREF])
    seg_dma(nc.sync, blobb, bB, BOFF, BSEGS, "dec2")
    nc.sync.dma_start(blobb[0:48, BOFF["dec1"]:BCOLS],
                      bB[0:48, BOFF["dec1"]:BCOLS])  # dec1+out+outc+dshc

    # ------------------------------------------------------------- init
    make_identity(nc, ident[:])
    nc.gpsimd.memset(c025[:], 0.25)
    nc.gpsimd.memset(c1[:], 1.0)
    nc.gpsimd.memset(xt[:], 0.0)
    # engine partition starts must be 32-aligned; rows 96..99 / 0..8 are
    # overwritten later by the embconv ACT / shift DMAs; rows 100/101 of
    # e1x/e2x and 102/103 of e3x by the hi/lo DMAs.
    nc.vector.memset(e1x[96:102, :], 1.0)
    nc.vector.memset(e2x[96:102, :], 1.0)
    nc.vector.memset(e3x[96:102, :], 1.0)
    nc.vector.memset(im2c3[0:10, :], 1.0)

    xt3 = _r3(xt[:], 50, 50)
    x3cp3 = _r3(x3cp[:], 50, 50)

    def borders(eng, ap3, pw):
        eng.memset(ap3[:, 0:1, :], 0.0)
        eng.memset(ap3[:, pw - 1:pw, :], 0.0)
        eng.memset(ap3[:, 1:pw - 1, 0:1], 0.0)
        eng.memset(ap3[:, 1:pw - 1, pw - 1:pw], 0.0)

    cat13 = _r3(cat1[:], 50, 50)
    p1p3 = _r3(p1p[:], 26, 26)
    cat23 = _r3(cat2[:], 26, 26)
    p2p3 = _r3(p2p[:], 14, 14)
    btp3 = _r3(btp[:], 14, 14)
    d2p3 = _r3(d2p[:], 26, 26)
    d1p3 = _r3(d1p[:], 50, 50)

    borders(nc.gpsimd, x3cp3, 50)
    borders(nc.gpsimd, cat13, 50)
    borders(nc.gpsimd, p1p3[32:48], 26)
    borders(nc.vector, cat23, 26)
    borders(nc.vector, p2p3[64:96], 14)
    borders(nc.vector, btp3, 14)
    borders(nc.vector, d2p3, 26)
    borders(nc.gpsimd, d1p3, 50)
    # xt ch6 = x2 rows 0..24 straight from dram (after the xt memset)
    nc.gpsimd.dma_start(xt3[6:7, 1:26, 1:49], bX3)

    pconv = ctx.enter_context(tc.tile_pool(name="pconv", bufs=2, space="PSUM"))
    pmain = ctx.enter_context(tc.tile_pool(name="pmain", bufs=2, space="PSUM"))


    # ------------------------------------------------------------ helpers
    def conv9(src3, wtile, cin, cout, row_chunks, w_, func, bias_ap, dst3,
              pbase=0, obase=0):
        tp = (pbase, obase) if (pbase or obase) else None
        s3 = src3[pbase:pbase + cin]
        r0 = 0
        for nr in row_chunks:
            ps = pconv.tile([obase + cout, nr * w_], F32, tag="conv",
                            name="convps")
            for s in range(9):
                dy, dx = s // 3, s % 3
                nc.tensor.matmul(ps[obase:obase + cout, :],
                                 wtile[:, s * cout:(s + 1) * cout],
                                 s3[:, r0 + dy:r0 + dy + nr, dx:dx + w_],
                                 start=(s == 0), stop=(s == 8),
                                 tile_position=tp)
            nc.scalar.activation(dst3[obase:obase + cout,
                                      1 + r0:1 + r0 + nr, 1:1 + w_],
                                 _r3(ps[obase:obase + cout, :], nr, w_),
                                 func, bias=bias_ap)
            r0 += nr

    def pool2(src3, dst3, orows, ocols, pbase, cch):
        t1 = tmp.tile([pbase + cch, orows * ocols], F16, tag="pool_a",
                      name="poolt1")
        t2 = tmp.tile([pbase + cch, orows * ocols], F16, tag="pool_b",
                      name="poolt2")
        s3 = src3[pbase:pbase + cch]
        v = [s3[:, 1 + a:1 + a + 2 * orows:2, 1 + b:1 + b + 2 * ocols:2]
             for a, b in ((0, 0), (1, 1), (0, 1), (1, 0))]
        t13 = _r3(t1[pbase:pbase + cch, :], orows, ocols)
        t23 = _r3(t2[pbase:pbase + cch, :], orows, ocols)
        nc.vector.tensor_max(t13, v[0], v[1])
        nc.vector.tensor_max(t23, v[2], v[3])
        nc.vector.tensor_max(dst3[pbase:pbase + cch, 1:1 + orows,
                                  1:1 + ocols], t13, t23)

    def up2(src3, sbase, dst3, dbase, cch, irows, icols):
        s = src3[sbase:sbase + cch, 1:1 + irows, 1:1 + icols]
        for a in (0, 1):
            for b in (0, 1):
                nc.vector.tensor_copy(
                    dst3[dbase:dbase + cch, 1 + a:1 + a + 2 * irows:2,
                         1 + b:1 + b + 2 * icols:2], s)

    def embconv(imbuf, chunks, dst, scale):
        off = 0
        for cw in chunks:
            ps = pconv.tile([100, cw], F32, tag="conv", name="convps")
            nc.tensor.matmul(ps[:], wt["emb"], imbuf[:, off:off + cw],
                             start=True, stop=True)
            nc.scalar.activation(dst[0:100, off:off + cw], ps[:],
                                 AF.Copy, scale=scale)
            off += cw

    def sqhilo(src, chunks, lhsT, esq, hib, lob, ex, row):
        """rows(row, row+1) of ex = NEGATED hi/lo fp16 pair of
        lhsT.T @ Square(src)."""
        n = sum(chunks)
        off = 0
        for cw in chunks:
            nc.scalar.activation(esq[:, off:off + cw],
                                 src[0:100, off:off + cw], AF.Square)
            ps = pconv.tile([1, cw], F32, tag="conv", name="sqps")
            nc.tensor.matmul(ps[:], lhsT, esq[:, off:off + cw],
                             start=True, stop=True)
            nc.scalar.activation(hib[0:1, off:off + cw], ps[:], AF.Copy,
                                 scale=-1.0)
            nc.vector.scalar_tensor_tensor(lob[0:1, off:off + cw], ps[:],
                                           -1.0, hib[0:1, off:off + cw],
                                           op0=ALU.mult, op1=ALU.subtract)
            off += cw
        nc.sync.dma_start(ex[row:row + 1, 0:n], hib[0:1, 0:n])
        nc.sync.dma_start(ex[row + 1:row + 2, 0:n], lob[0:1, 0:n])

    # --------------------------------------------- U-Net + emb interleave
    # enc1: im2col matmuls -> cat1[32:48] (tile_position col offset 32)
    r0 = 0
    for nr in (10, 10, 10, 8):
        ps = pconv.tile([48, nr * W], F32, tag="conv", name="convps")
        nc.tensor.matmul(ps[32:48, :], wt["enc1s"],
                         im27[:, r0 * W:(r0 + nr) * W],
                         start=True, stop=True, tile_position=(0, 32))
        nc.scalar.activation(cat13[32:48, 1 + r0:1 + r0 + nr, 1:1 + W],
                             _r3(ps[32:48, :], nr, W), AF.Relu,
                             bias=bia["enc1@32"])
        r0 += nr
    embconv(im2c12[0:10, 0:NREF], RECH, e1x[:], 2.0)       # PE filler
    pool2(cat13, p1p3, 19, 24, 32, 16)
    conv9(p1p3, wt["enc2"], 16, 32, [18], 24, AF.Relu, bia["enc2@64"],
          cat23, pbase=32, obase=64)
    embconv(im2c12[0:10, NREF:2 * NREF], RECH, e2x[:], 2.0)
    pool2(cat23, p2p3, 9, 12, 64, 32)
    conv9(p2p3, wt["bott"], 32, 64, [8], 12, AF.Relu, bia["bott"], btp3,
          pbase=64, obase=0)
    sqhilo(e1x[:], RECH, c025[:], esq1[:], hib1, lob1, e1x[:], 100)
    up2(btp3, 0, cat23, 0, 64, 8, 12)
    conv9(cat23, wt["dec2"], 96, 32, [14], 24, AF.Relu, bia["dec2"], d2p3)
    sqhilo(e2x[:], RECH, c025[:], esq2[:], hib2, lob2, e2x[:], 100)
    up2(d2p3, 0, cat13, 0, 32, 14, 24)
    # dec1 with the 1x1 output convs interleaved per row chunk so x3cp (and
    # the e3 shift DMAs it gates) completes as early as possible
    r0 = 0
    for nr in (10, 10, 6):
        psd = pconv.tile([16, nr * W], F32, tag="conv", name="convps")
        for s in range(9):
            dy, dx = s // 3, s % 3
            nc.tensor.matmul(psd[:], wt["dec1"][:, s * 16:(s + 1) * 16],
                             cat13[:, r0 + dy:r0 + dy + nr, dx:dx + W],
                             start=(s == 0), stop=(s == 8))
        nc.scalar.activation(d1p3[:, 1 + r0:1 + r0 + nr, 1:1 + W],
                             _r3(psd[:], nr, W), AF.Relu, bias=bia["dec1"])
        rhs = d1p3[:, 1 + r0:1 + r0 + nr, 1:1 + W]
        psc = pconv.tile([1, nr * W], F32, tag="conv", name="convps")
        nc.tensor.matmul(psc[:], wt["outc"], rhs, start=True, stop=True)
        nc.scalar.activation(x3cp3[0:1, 1 + r0:1 + r0 + nr, 1:1 + W],
                             _r3(psc[:], nr, W), AF.Identity,
                             bias=bia["outc"])
        ps = pconv.tile([4, nr * W], F32, tag="conv", name="convps")
        nc.tensor.matmul(ps[:], wt["out"], rhs, start=True, stop=True)
        nc.scalar.activation(xt3[0:4, 1 + r0:1 + r0 + nr, 1:1 + W],
                             _r3(ps[:], nr, W), AF.Identity, bias=bia["out"])
        r0 += nr

    # ------------------------------------------------------- embedding 3
    for s in range(9):
        dy, dx = s // 3, s % 3
        eng = nc.sync if s % 2 == 0 else nc.gpsimd
        eng.dma_start(im2c3[s:s + 1, 0:Q],
                      x3cp3[0:1, dy:dy + QROWS, dx:dx + W])
    embconv(im2c3[:], E3CH, e3x[:], 1.0)

    # ------------------------------------------------------- matching
    nc.vector.memset(accDG[:], -60000.0)
    nc.vector.memset(accAG[:], -60000.0)
    nc.vector.memset(accDL[:], -60000.0)
    nc.vector.memset(accAL[:], -60000.0)

    def match_chunks(ex, accD, accA, k_range):
        for k in k_range:
            lhsT = ex[:][:, k * 128:(k + 1) * 128]
            ps = pmain.tile([128, Q], F32, tag="main", name="mainps")
            for o, n in ((0, 512), (512, 512), (1024, 176)):
                nc.tensor.matmul(ps[:, o:o + n], lhsT,
                                 e3x[0:102, o:o + n], start=True, stop=True)
            if k % 4 == 0:          # DVE drains PSUM directly (1x rate)
                nc.vector.tensor_max(accD[:], ps[:], accD[:])
            else:                    # ACT copies; DVE merges all-fp16 (2x)
                sc = scr.tile([128, Q], F16, tag="scr", name="scrt")
                nc.scalar.copy(sc[:], ps[:])
                nc.vector.tensor_max(accA[:], sc[:], accA[:])

    def ref_finals(accD, accA, accM, qmax):
        nc.vector.tensor_max(accM[:], accD[:], accA[:])

    def ref_transposes(accM, qmax):
        for i in range(NQC):
            n = 128 if i < 9 else 48
            pst = pconv.tile([n, 128], F16, tag="conv", name="trps")
            nc.tensor.transpose(pst[:], accM[:, i * 128:i * 128 + n],
                                ident[:128, :128])
            nc.vector.tensor_reduce(qmax[0:n, i:i + 1], pst[:],
                                    axis=AX.X, op=ALU.max)

    def ref_plane(qmax, r, eng):
        dsub = small.tile([128, NQC], F32, tag="dsub", name="dsub")
        nc.vector.scalar_tensor_tensor(dsub[:], qmax[:], 1.0, b2T[:],
                                       op0=ALU.mult, op1=ALU.subtract)
        tneg = small.tile([128, NQC], F32, tag="tneg", name="tneg")
        nc.vector.tensor_scalar(tneg[:], dsub[:], 0.0, 0.0,
                                op0=ALU.min, op1=ALU.min)
        gcol = small.tile([128, NQC], F16, tag="gcol", name="gcol")
        nc.scalar.activation(gcol[:], tneg[:], AF.Tanh, scale=-0.5)
        pst = pconv.tile([NQC, 128], F16, tag="conv", name="gmtps")
        nc.tensor.transpose(pst[:], gcol[:], ident[:128, :128])
        gcolT = small.tile([NQC, 128], F16, tag="gcolT", name="gcolT")
        nc.scalar.copy(gcolT[:], pst[:])
        gflat = small.tile([1, Q], F16, tag="gflat", name="gflat")
        eng.dma_start(gflat[0:1, 0:1152], gcolT[0:9, :])
        eng.dma_start(gflat[0:1, 1152:1200], gcolT[9:10, 0:48])
        eng.dma_start(xt3[4 + r:5 + r, 1:26, 1:49], gflat[:])

    qmaxG = small.tile([128, NQC], F32, tag="qmaxG", name="qmaxG")
    qmaxL = small.tile([128, NQC], F32, tag="qmaxL", name="qmaxL")

    match_chunks(e1x, accDG, accAG, range(3))      # gm starts immediately
    # |b|^2 in transposed layout (commutes with the ref-max): Square + 10
    # small matmuls slot between gm chunks; only the qmax tail needs them.
    off = 0
    for cw in E3CH:
        nc.scalar.activation(esq3[:, off:off + cw], e3x[0:100, off:off + cw],
                             AF.Square)
        off += cw
    nc.gpsimd.memset(b2T[:], 0.0)
    for i in range(NQC):
        n = 128 if i < 9 else 48
        b2ps = pconv.tile([128, 2], F32, tag="conv", name="b2ps")
        nc.tensor.matmul(b2ps[0:n, :], esq3[:, i * 128:i * 128 + n], c1[:],
                         start=True, stop=True)
        nc.scalar.copy(b2T[0:n, i:i + 1], b2ps[0:n, 0:1])
    match_chunks(e1x, accDG, accAG, range(3, 18))
    ref_finals(accDG, accAG, accMG, qmaxG)         # DVE combine (overlaps lm)
    match_chunks(e2x, accDL, accAL, range(4))
    ref_transposes(accMG, qmaxG)                   # PE slots between lm MMs
    match_chunks(e2x, accDL, accAL, range(4, 18))
    ref_plane(qmaxG, 0, nc.gpsimd)
    ref_finals(accDL, accAL, accML, qmaxL)
    ref_transposes(accML, qmaxL)
    ref_plane(qmaxL, 1, nc.gpsimd)

    # ------------------------------------------------------- head conv
    r0 = 0
    for nr in (8, 8, 8):
        ps = pconv.tile([1, nr * W], F32, tag="conv", name="convps")
        for s in range(9):
            dy, dx = s // 3, s % 3
            nc.tensor.matmul(ps[:], wt["dshc"][:, s:s + 1],
                             xt3[:, r0 + dy:r0 + dy + nr, dx:dx + W],
                             start=(s == 0), stop=(s == 8))
        nc.scalar.activation(out_sb[0:1, r0 * W:(r0 + nr) * W],
                             _r3(ps[:], nr, W), AF.Identity, bias=bia["dsh"])
        r0 += nr
    nc.sync.dma_start(out_d, out_sb[:])


def build_program():
    import contextlib
    nc = bacc.Bacc("TRN2", target_bir_lowering=False, debug=False,
                   num_devices=8)
    with tile.TileContext(nc) as tc:
        with contextlib.ExitStack() as ctx:
            _emit(nc, tc, ctx)
    nc.compile()
    return nc


def _get_program():
    global _PROG
    if _PROG is None:
        _PROG = build_program()
    return _PROG


CORE_BC = [(0, 2), (0, 3), (1, 2), (1, 3)]


def _wT_flat(w):
    """[Cout, Cin, 3, 3] -> [Cin, 9*Cout]: col block s holds w[:, :, s//3, s%3].T"""
    cout, cin = w.shape[:2]
    out = np.zeros((cin, 9 * cout), np.float32)
    for s in range(9):
        out[:, s * cout:(s + 1) * cout] = w[:, :, s // 3, s % 3].T
    return out


def _pad50(img):
    out = np.zeros((50, 50), np.float32)
    out[1:49, 1:49] = img
    return out


def _im2col9(img, rows, ones_row=False):
    """padded 50x50 -> [9(+1), rows*48] rows ordered s=dy*3+dx."""
    p = _pad50(img)
    rws = [p[dy:dy + rows, dx:dx + W].ravel()
           for dy in range(3) for dx in range(3)]
    if ones_row:
        rws.append(np.ones(rows * W, np.float32))
    return np.stack(rws)


def _blobs(inp, flip, c):
    w = {k: (inp[k][:, :, ::-1, :] if flip else inp[k])
         for k in ["enc1_w", "enc2_w", "bott_w", "dec2_w", "dec1_w",
                   "emb_w", "dsh_w"]}
    seg = {}
    seg["enc1s"] = w["enc1_w"].reshape(16, 3, 9).transpose(2, 1, 0) \
                              .reshape(27, 16)
    seg["enc2"] = _wT_flat(w["enc2_w"])
    seg["bott"] = _wT_flat(w["bott_w"])
    seg["dec2"] = _wT_flat(w["dec2_w"])
    seg["dec1"] = _wT_flat(w["dec1_w"])
    seg["out"] = inp["out_w"][:, :, 0, 0].T
    seg["outc"] = inp["out_w"][c, :, 0, 0][:, None]
    seg["emb"] = np.vstack([w["emb_w"].reshape(100, 9).T,
                            inp["emb_b"][None, :]])
    seg["dshc"] = w["dsh_w"].reshape(7, 9)

    def pack(segs, ncols):
        blob = np.zeros((96, ncols), np.float16)
        off = 0
        for nm, r0, rows, cols in segs:
            blob[r0:r0 + rows, off:off + cols] = seg[nm].astype(np.float16)
            off += cols
        return blob

    blobbias = np.zeros((96, 10), np.float32)
    for nm, col in BIAS_COL.items():
        if nm == "outc":
            v = inp["out_b"][c:c + 1]
        else:
            v = inp[nm + "_b"]
        blobbias[0:len(v), col] = v
    blobbias[32:48, 8] = inp["enc1_b"]
    blobbias[64:96, 9] = inp["enc2_b"]
    return pack(ASEGS, ACOLS), pack(BSEGS, BCOLS), blobbias


def make_in_maps(inp):
    maps = []
    for k8 in range(8):
        n_idx, half = k8 // 2, k8 % 2
        b, c = CORE_BC[n_idx]
        x1c, x2c, x3b = inp["x1"][b, c], inp["x2"][b, c], inp["x3"][b]
        if half:
            x1c, x2c, x3b = x1c[::-1], x2c[::-1], x3b[:, ::-1]
        bx1 = np.zeros((27, 39 * W), np.float32)
        for ci in range(3):
            im9 = _im2col9(x3b[ci], 39)
            for s in range(9):
                bx1[s * 3 + ci] = im9[s]
        bx2 = np.concatenate([_im2col9(x1c, H, True),
                              _im2col9(x2c, H, True)], axis=1)
        bx3 = x2c[0:25, :].reshape(1, Q)
        blobA, blobB, blobbias = _blobs(inp, bool(half), c)
        maps.append({"blobA": blobA, "blobB": blobB,
                     "blobBias": blobbias,
                     "blobX1": np.ascontiguousarray(bx1.astype(np.float16)),
                     "blobX2": np.ascontiguousarray(bx2.astype(np.float16)),
                     "blobX3": np.ascontiguousarray(bx3.astype(np.float16))})
    return maps


def assemble(results):
    out = np.zeros((2, 2, H, W), np.float32)
    for k8, r in enumerate(results):
        n_idx, half = k8 // 2, k8 % 2
        b, c = CORE_BC[n_idx]
        y = r["out"].reshape(24, W)
        if half == 0:
            out[b, c - 2, 0:24] = y
        else:
            out[b, c - 2, 24:48] = y[::-1]
    return out


def kernel(**inputs):
    inp = {k: np.asarray(v) for k, v in inputs.items()}
    nc = _get_program()
    maps = make_in_maps(inp)
    res = run_bass_kernel_spmd(nc, maps, core_ids=list(range(8)), trace=False)
    return assemble(res.results)


# revision 24
# speedup vs baseline: 1.0852x; 1.0020x over previous
"""FEELVOS fused kernel for TRN2, 8-core SPMD — fp16 rev3.

Sharding: the reference only returns logits for classes C-2, C-1, so only 4 of
the 8 fused (batch, class) items matter. 8 cores = 4 (b, c) pairs x 2 frame
halves (top/bottom 24 rows). Bottom-half cores receive row-flipped inputs and
row-flipped conv kernels so every core runs the identical program computing
"top 25 rows" of its (possibly flipped) frame; the host un-flips on gather.

rev5: ref-major matching. The distance matmul puts 128 REF pixels on the
PSUM partitions and all 1200 queries on the free axis, NEGATED (m = 2ab -
|a|^2 - |b|^2 = -d^2, via sign flips host/device), so the reduction over
refs is a MAX. 18 ref chunks are folded by elementwise max-merges into two
fp16 accumulators: even chunks merge straight from PSUM on DVE; odd chunks
are copied PSUM->fp16 by ACT and merged by GPSIMD (SBUF-only engine). This
splits the PSUM-drain floor (~1.1 ns/elem/lane, dtype-independent on any
single engine) across three engines. The final 128-partition max per query
goes through PE transposes + short DVE reduces.

All matmul operands fp16; PSUM fp32. K=104 carries |a|^2 (rows 100/101 of
the ref operand, negated hi+lo pair, vs ones) and |b|^2 (rows 102/103 of
the query operand, negated hi+lo, vs ones), so -d^2 comes out of the
matmul complete.

U-Net decoder convs read single concatenated-K tiles: skip connections are
written at partition offsets via matmul tile_position (enc1 -> cat1[32:48],
enc2 -> cat2[64:96]), halving decoder matmuls. Embedding/square matmuls are
interleaved between U-Net layers to keep the PE busy (HAM warm). The head
conv runs directly on the padded [7,2500] xt plane (9 shifted K=7 matmuls).
"""
import numpy as np

import concourse.bass as bass
import concourse.bacc as bacc
import concourse.tile as tile
from concourse import mybir
from concourse.bass_utils import run_bass_kernel_spmd
from concourse.masks import make_identity

F32 = mybir.dt.float32
F16 = mybir.dt.float16
AF = mybir.ActivationFunctionType
ALU = mybir.AluOpType
AX = mybir.AxisListType

H = W = 48
NREF = H * W                 # 2304 ref pixels (full frame)
QROWS = 25
Q = QROWS * W                # 1200 query pixels
QCH, NQC = 120, 10           # query chunking for the distance matmul
RECH = [512, 512, 512, 512, 256]  # 2304 column chunking (PSUM bank)
E3CH = [432, 384, 384]       # 1200 column chunking
_PROG = None


def _r3(ap, h, w):
    return ap.rearrange("c (h w) -> c h w", h=h, w=w)


# blob column layouts: (name, row0, nrows, cols), fp16. Row offsets place
# weights at the partition base their matmul's contraction rows need.
ASEGS = [("enc1s", 0, 27, 16), ("enc2", 32, 16, 288), ("bott", 64, 32, 576)]
BSEGS = [("dec2", 0, 96, 288), ("dec1", 0, 48, 144), ("out", 0, 16, 4),
         ("outc", 0, 16, 1), ("emb", 0, 10, 100), ("dshc", 0, 7, 9)]


def _offsets(segs):
    off, o = {}, 0
    for nm, _r0, _r, c in segs:
        off[nm] = o
        o += c
    return off, o


AOFF, ACOLS = _offsets(ASEGS)
BOFF, BCOLS = _offsets(BSEGS)
BIAS_COL = {"enc1": 0, "enc2": 1, "bott": 2, "dec2": 3, "dec1": 4,
            "out": 5, "dsh": 6, "outc": 7}


def _emit(nc, tc, ctx):
    # ------------------------------------------------------------- dram io
    bA = nc.dram_tensor("blobA", [96, ACOLS], F16, kind="ExternalInput").ap()
    bB = nc.dram_tensor("blobB", [96, BCOLS], F16, kind="ExternalInput").ap()
    bBias = nc.dram_tensor("blobBias", [96, 10], F32,
                           kind="ExternalInput").ap()
    bX1 = nc.dram_tensor("blobX1", [27, 39 * W], F16,
                         kind="ExternalInput").ap()
    bX2 = nc.dram_tensor("blobX2", [10, 2 * NREF], F16,
                         kind="ExternalInput").ap()
    bX3 = nc.dram_tensor("blobX3", [1, Q], F16, kind="ExternalInput").ap()
    out_d = nc.dram_tensor("out", [1, 24 * W], F32, kind="ExternalOutput").ap()

    # ------------------------------------------------------------- sbuf
    sb = ctx.enter_context(tc.tile_pool(name="sb", bufs=1))

    def st(name, p, f, dt=F16):
        return sb.tile([p, f], dt, tag=name, name=name)

    bloba = st("bloba", 96, ACOLS)
    blobb = st("blobb", 96, BCOLS)
    blobbias = st("blobbias", 96, 10, F32)
    im27 = st("im27", 27, 39 * W)        # enc1 im2col (host-built)
    im2c12 = st("im2c12", 10, 2 * NREF)  # emb im2col e1|e2 (host-built)

    def wseg(blob, off, segs, nm):
        r0, rows, cols = next((a, b, c) for n, a, b, c in segs if n == nm)
        return blob[r0:r0 + rows, off[nm]:off[nm] + cols]

    wt = {nm: wseg(bloba, AOFF, ASEGS, nm) for nm, _, _, _ in ASEGS}
    wt.update({nm: wseg(blobb, BOFF, BSEGS, nm) for nm, _, _, _ in BSEGS})
    bia = {nm: blobbias[0:r, c:c + 1]
           for nm, (r, c) in {"enc1": (16, 0), "enc2": (32, 1),
                              "bott": (64, 2), "dec2": (32, 3),
                              "dec1": (16, 4), "out": (4, 5),
                              "dsh": (1, 6), "outc": (1, 7)}.items()}
    bia["enc1@32"] = blobbias[32:48, 8:9]
    bia["enc2@64"] = blobbias[64:96, 9:10]

    # device-written padded planes (fp16)
    x3cp = st("x3cp", 1, 2500)
    cat1 = st("cat1", 48, 2500)   # rows 0..31 up(d2), rows 32..47 e1
    p1p = st("p1p", 48, 676)      # rows 32..47 used
    cat2 = st("cat2", 96, 676)    # rows 0..63 up(bt), rows 64..95 e2
    p2p = st("p2p", 96, 196)      # rows 64..95 used
    btp = st("btp", 64, 196)
    d2p = st("d2p", 32, 676)
    d1p = st("d1p", 16, 2500)
    xt = st("xt", 7, 2500)

    im2c3 = st("im2c3", 10, Q)           # emb im2col (e3)
    e1x = st("e1x", 102, NREF)           # 0..99 +2*e1, 100/101 -|a|^2 hi/lo
    e2x = st("e2x", 102, NREF)
    e3x = st("e3x", 102, Q)              # 0..99 e3, 100/101 ones
    esq1 = st("esq1", 100, NREF)
    esq2 = st("esq2", 100, NREF)
    esq3 = st("esq3", 100, Q)
    ident = st("ident", 128, 128)

    c025 = st("c025", 100, 1)
    c1 = st("c1", 100, 2)
    hib1 = st("hib1", 1, NREF)
    lob1 = st("lob1", 1, NREF)
    hib2 = st("hib2", 1, NREF)
    lob2 = st("lob2", 1, NREF)
    b2T = st("b2T", 128, NQC, F32)       # |b|^2 in transposed query layout
    accDG = st("accDG", 128, Q)          # fp16 running-max accumulators
    accAG = st("accAG", 128, Q)
    accDL = st("accDL", 128, Q)
    accAL = st("accAL", 128, Q)
    accMG = st("accMG", 128, Q)
    accML = st("accML", 128, Q)
    out_sb = st("out_sb", 1, 24 * W, F32)

    small = ctx.enter_context(tc.tile_pool(name="small", bufs=8))
    tmp = ctx.enter_context(tc.tile_pool(name="tmp", bufs=2))
    scr = ctx.enter_context(tc.tile_pool(name="scr", bufs=3))

    # ------------------------------------------------------------- input dma
    # exact-row segment DMAs, hot-first
    def seg_dma(eng, blobt, blobd, off, segs, nm):
        r0, rows, cols = next((a, b, c) for n, a, b, c in segs if n == nm)
        eng.dma_start(blobt[r0:r0 + rows, off[nm]:off[nm] + cols],
                      blobd[r0:r0 + rows, off[nm]:off[nm] + cols])

    seg_dma(nc.sync, bloba, bA, AOFF, ASEGS, "enc1s")
    nc.gpsimd.dma_start(blobbias[:], bBias)
    nc.sync.dma_start(im27[:, 0:960], bX1[:, 0:960])        # enc1 rows 0..19
    seg_dma(nc.gpsimd, bloba, bA, AOFF, ASEGS, "enc2")
    seg_dma(nc.gpsimd, blobb, bB, BOFF, BSEGS, "emb")
    nc.sync.dma_start(im27[:, 960:39 * W], bX1[:, 960:39 * W])
    nc.sync.dma_start(im2c12[:, 0:NREF // 2], bX2[:, 0:NREF // 2])
    nc.gpsimd.dma_start(im2c12[:, NREF // 2:NREF], bX2[:, NREF // 2:NREF])
    seg_dma(nc.gpsimd, bloba, bA, AOFF, ASEGS, "bott")
    nc.sync.dma_start(im2c12[:, NREF:NREF + NREF // 2],
                      bX2[:, NREF:NREF + NREF // 2])
    nc.gpsimd.dma_start(im2c12[:, NREF + NREF // 2:2 * NREF],
                        bX2[:, NREF + NREF // 2:2 * NREF])
    seg_dma(nc.sync, blobb, bB, BOFF, BSEGS, "dec2")
    nc.sync.dma_start(blobb[0:48, BOFF["dec1"]:BCOLS],
                      bB[0:48, BOFF["dec1"]:BCOLS])  # dec1+out+outc+dshc

    # ------------------------------------------------------------- init
    make_identity(nc, ident[:])
    nc.gpsimd.memset(c025[:], 0.25)
    nc.gpsimd.memset(c1[:], 1.0)
    nc.gpsimd.memset(xt[:], 0.0)
    # engine partition starts must be 32-aligned; rows 96..99 / 0..8 are
    # overwritten later by the embconv ACT / shift DMAs; rows 100/101 of
    # e1x/e2x and 102/103 of e3x by the hi/lo DMAs.
    nc.vector.memset(e1x[96:102, :], 1.0)
    nc.vector.memset(e2x[96:102, :], 1.0)
    nc.vector.memset(e3x[96:102, :], 1.0)
    nc.vector.memset(im2c3[0:10, :], 1.0)

    xt3 = _r3(xt[:], 50, 50)
    x3cp3 = _r3(x3cp[:], 50, 50)

    def borders(eng, ap3, pw):
        eng.memset(ap3[:, 0:1, :], 0.0)
        eng.memset(ap3[:, pw - 1:pw, :], 0.0)
        eng.memset(ap3[:, 1:pw - 1, 0:1], 0.0)
        eng.memset(ap3[:, 1:pw - 1, pw - 1:pw], 0.0)

    cat13 = _r3(cat1[:], 50, 50)
    p1p3 = _r3(p1p[:], 26, 26)
    cat23 = _r3(cat2[:], 26, 26)
    p2p3 = _r3(p2p[:], 14, 14)
    btp3 = _r3(btp[:], 14, 14)
    d2p3 = _r3(d2p[:], 26, 26)
    d1p3 = _r3(d1p[:], 50, 50)

    borders(nc.gpsimd, x3cp3, 50)
    borders(nc.gpsimd, cat13, 50)
    borders(nc.gpsimd, p1p3[32:48], 26)
    borders(nc.vector, cat23, 26)
    borders(nc.vector, p2p3[64:96], 14)
    borders(nc.vector, btp3, 14)
    borders(nc.vector, d2p3, 26)
    borders(nc.gpsimd, d1p3, 50)
    # xt ch6 = x2 rows 0..24 straight from dram (after the xt memset)
    nc.gpsimd.dma_start(xt3[6:7, 1:26, 1:49], bX3)

    pconv = ctx.enter_context(tc.tile_pool(name="pconv", bufs=2, space="PSUM"))
    pmain = ctx.enter_context(tc.tile_pool(name="pmain", bufs=2, space="PSUM"))


    # ------------------------------------------------------------ helpers
    def conv9(src3, wtile, cin, cout, row_chunks, w_, func, bias_ap, dst3,
              pbase=0, obase=0):
        tp = (pbase, obase) if (pbase or obase) else None
        s3 = src3[pbase:pbase + cin]
        r0 = 0
        for nr in row_chunks:
            ps = pconv.tile([obase + cout, nr * w_], F32, tag="conv",
                            name="convps")
            for s in range(9):
                dy, dx = s // 3, s % 3
                nc.tensor.matmul(ps[obase:obase + cout, :],
                                 wtile[:, s * cout:(s + 1) * cout],
                                 s3[:, r0 + dy:r0 + dy + nr, dx:dx + w_],
                                 start=(s == 0), stop=(s == 8),
                                 tile_position=tp)
            nc.scalar.activation(dst3[obase:obase + cout,
                                      1 + r0:1 + r0 + nr, 1:1 + w_],
                                 _r3(ps[obase:obase + cout, :], nr, w_),
                                 func, bias=bias_ap)
            r0 += nr

    def pool2(src3, dst3, orows, ocols, pbase, cch):
        t1 = tmp.tile([pbase + cch, orows * ocols], F16, tag="pool_a",
                      name="poolt1")
        t2 = tmp.tile([pbase + cch, orows * ocols], F16, tag="pool_b",
                      name="poolt2")
        s3 = src3[pbase:pbase + cch]
        v = [s3[:, 1 + a:1 + a + 2 * orows:2, 1 + b:1 + b + 2 * ocols:2]
             for a, b in ((0, 0), (1, 1), (0, 1), (1, 0))]
        t13 = _r3(t1[pbase:pbase + cch, :], orows, ocols)
        t23 = _r3(t2[pbase:pbase + cch, :], orows, ocols)
        nc.vector.tensor_max(t13, v[0], v[1])
        nc.vector.tensor_max(t23, v[2], v[3])
        nc.vector.tensor_max(dst3[pbase:pbase + cch, 1:1 + orows,
                                  1:1 + ocols], t13, t23)

    def up2(src3, sbase, dst3, dbase, cch, irows, icols):
        s = src3[sbase:sbase + cch, 1:1 + irows, 1:1 + icols]
        for a in (0, 1):
            for b in (0, 1):
                nc.vector.tensor_copy(
                    dst3[dbase:dbase + cch, 1 + a:1 + a + 2 * irows:2,
                         1 + b:1 + b + 2 * icols:2], s)

    def embconv(imbuf, chunks, dst, scale):
        off = 0
        for cw in chunks:
            ps = pconv.tile([100, cw], F32, tag="conv", name="convps")
            nc.tensor.matmul(ps[:], wt["emb"], imbuf[:, off:off + cw],
                             start=True, stop=True)
            nc.scalar.activation(dst[0:100, off:off + cw], ps[:],
                                 AF.Copy, scale=scale)
            off += cw

    def sqhilo(src, chunks, lhsT, esq, hib, lob, ex, row):
        """rows(row, row+1) of ex = NEGATED hi/lo fp16 pair of
        lhsT.T @ Square(src)."""
        n = sum(chunks)
        off = 0
        for cw in chunks:
            nc.scalar.activation(esq[:, off:off + cw],
                                 src[0:100, off:off + cw], AF.Square)
            ps = pconv.tile([1, cw], F32, tag="conv", name="sqps")
            nc.tensor.matmul(ps[:], lhsT, esq[:, off:off + cw],
                             start=True, stop=True)
            nc.scalar.activation(hib[0:1, off:off + cw], ps[:], AF.Copy,
                                 scale=-1.0)
            nc.vector.scalar_tensor_tensor(lob[0:1, off:off + cw], ps[:],
                                           -1.0, hib[0:1, off:off + cw],
                                           op0=ALU.mult, op1=ALU.subtract)
            off += cw
        nc.sync.dma_start(ex[row:row + 1, 0:n], hib[0:1, 0:n])
        nc.sync.dma_start(ex[row + 1:row + 2, 0:n], lob[0:1, 0:n])

    # --------------------------------------------- U-Net + emb interleave
    # enc1: im2col matmuls -> cat1[32:48] (tile_position col offset 32)
    r0 = 0
    for nr in (10, 10, 10, 8):
        ps = pconv.tile([48, nr * W], F32, tag="conv", name="convps")
        nc.tensor.matmul(ps[32:48, :], wt["enc1s"],
                         im27[:, r0 * W:(r0 + nr) * W],
                         start=True, stop=True, tile_position=(0, 32))
        nc.scalar.activation(cat13[32:48, 1 + r0:1 + r0 + nr, 1:1 + W],
                             _r3(ps[32:48, :], nr, W), AF.Relu,
                             bias=bia["enc1@32"])
        r0 += nr
    embconv(im2c12[0:10, 0:NREF], RECH, e1x[:], 2.0)       # PE filler
    pool2(cat13, p1p3, 19, 24, 32, 16)
    conv9(p1p3, wt["enc2"], 16, 32, [18], 24, AF.Relu, bia["enc2@64"],
          cat23, pbase=32, obase=64)
    embconv(im2c12[0:10, NREF:2 * NREF], RECH, e2x[:], 2.0)
    pool2(cat23, p2p3, 9, 12, 64, 32)
    conv9(p2p3, wt["bott"], 32, 64, [8], 12, AF.Relu, bia["bott"], btp3,
          pbase=64, obase=0)
    sqhilo(e1x[:], RECH, c025[:], esq1[:], hib1, lob1, e1x[:], 100)
    up2(btp3, 0, cat23, 0, 64, 8, 12)
    conv9(cat23, wt["dec2"], 96, 32, [14], 24, AF.Relu, bia["dec2"], d2p3)
    sqhilo(e2x[:], RECH, c025[:], esq2[:], hib2, lob2, e2x[:], 100)
    up2(d2p3, 0, cat13, 0, 32, 14, 24)
    # dec1 with the 1x1 output convs interleaved per row chunk so x3cp (and
    # the e3 shift DMAs it gates) completes as early as possible
    r0 = 0
    for nr in (10, 10, 6):
        psd = pconv.tile([16, nr * W], F32, tag="conv", name="convps")
        for s in range(9):
            dy, dx = s // 3, s % 3
            nc.tensor.matmul(psd[:], wt["dec1"][:, s * 16:(s + 1) * 16],
                             cat13[:, r0 + dy:r0 + dy + nr, dx:dx + W],
                             start=(s == 0), stop=(s == 8))
        nc.scalar.activation(d1p3[:, 1 + r0:1 + r0 + nr, 1:1 + W],
                             _r3(psd[:], nr, W), AF.Relu, bias=bia["dec1"])
        rhs = d1p3[:, 1 + r0:1 + r0 + nr, 1:1 + W]
        psc = pconv.tile([1, nr * W], F32, tag="conv", name="convps")
        nc.tensor.matmul(psc[:], wt["outc"], rhs, start=True, stop=True)
        nc.scalar.activation(x3cp3[0:1, 1 + r0:1 + r0 + nr, 1:1 + W],
                             _r3(psc[:], nr, W), AF.Identity,
                             bias=bia["outc"])
        ps = pconv.tile([4, nr * W], F32, tag="conv", name="convps")
        nc.tensor.matmul(ps[:], wt["out"], rhs, start=True, stop=True)
        nc.scalar.activation(xt3[0:4, 1 + r0:1 + r0 + nr, 1:1 + W],
                             _r3(ps[:], nr, W), AF.Identity, bias=bia["out"])
        r0 += nr

    # ------------------------------------------------------- embedding 3
    # shift DMAs split by query-row chunk: each piece only needs the x3cp
    # rows its outc chunk has produced, so they stream during dec1.
    for r0, r1 in ((0, 9), (9, 18), (18, 25)):
        for s in range(9):
            dy, dx = s // 3, s % 3
            eng = nc.sync if s % 2 == 0 else nc.gpsimd
            eng.dma_start(im2c3[s:s + 1, r0 * W:r1 * W],
                          x3cp3[0:1, r0 + dy:r1 + dy, dx:dx + W])
    embconv(im2c3[:], E3CH, e3x[:], 1.0)

    # ------------------------------------------------------- matching
    nc.vector.memset(accDG[:], -60000.0)
    nc.vector.memset(accAG[:], -60000.0)
    nc.vector.memset(accDL[:], -60000.0)
    nc.vector.memset(accAL[:], -60000.0)

    def match_chunks(ex, accD, accA, k_range):
        for k in k_range:
            lhsT = ex[:][:, k * 128:(k + 1) * 128]
            ps = pmain.tile([128, Q], F32, tag="main", name="mainps")
            for o, n in ((0, 512), (512, 512), (1024, 176)):
                nc.tensor.matmul(ps[:, o:o + n], lhsT,
                                 e3x[0:102, o:o + n], start=True, stop=True)
            if k % 4 == 0:          # DVE drains PSUM directly (1x rate)
                nc.vector.tensor_max(accD[:], ps[:], accD[:])
            else:                    # ACT copies; DVE merges all-fp16 (2x)
                sc = scr.tile([128, Q], F16, tag="scr", name="scrt")
                nc.scalar.copy(sc[:], ps[:])
                nc.vector.tensor_max(accA[:], sc[:], accA[:])

    def ref_finals(accD, accA, accM, qmax):
        nc.vector.tensor_max(accM[:], accD[:], accA[:])

    def ref_transposes(accM, qmax):
        for i in range(NQC):
            n = 128 if i < 9 else 48
            pst = pconv.tile([n, 128], F16, tag="conv", name="trps")
            nc.tensor.transpose(pst[:], accM[:, i * 128:i * 128 + n],
                                ident[:128, :128])
            nc.vector.tensor_reduce(qmax[0:n, i:i + 1], pst[:],
                                    axis=AX.X, op=ALU.max)

    def ref_plane(qmax, r, eng):
        dsub = small.tile([128, NQC], F32, tag="dsub", name="dsub")
        nc.vector.scalar_tensor_tensor(dsub[:], qmax[:], 1.0, b2T[:],
                                       op0=ALU.mult, op1=ALU.subtract)
        tneg = small.tile([128, NQC], F32, tag="tneg", name="tneg")
        nc.vector.tensor_scalar(tneg[:], dsub[:], 0.0, 0.0,
                                op0=ALU.min, op1=ALU.min)
        gcol = small.tile([128, NQC], F16, tag="gcol", name="gcol")
        nc.scalar.activation(gcol[:], tneg[:], AF.Tanh, scale=-0.5)
        pst = pconv.tile([NQC, 128], F16, tag="conv", name="gmtps")
        nc.tensor.transpose(pst[:], gcol[:], ident[:128, :128])
        gcolT = small.tile([NQC, 128], F16, tag="gcolT", name="gcolT")
        nc.scalar.copy(gcolT[:], pst[:])
        gflat = small.tile([1, Q], F16, tag="gflat", name="gflat")
        eng.dma_start(gflat[0:1, 0:1152], gcolT[0:9, :])
        eng.dma_start(gflat[0:1, 1152:1200], gcolT[9:10, 0:48])
        eng.dma_start(xt3[4 + r:5 + r, 1:26, 1:49], gflat[:])

    qmaxG = small.tile([128, NQC], F32, tag="qmaxG", name="qmaxG")
    qmaxL = small.tile([128, NQC], F32, tag="qmaxL", name="qmaxL")

    match_chunks(e1x, accDG, accAG, range(3))      # gm starts immediately
    # |b|^2 in transposed layout (commutes with the ref-max): Square + 10
    # small matmuls slot between gm chunks; only the qmax tail needs them.
    off = 0
    for cw in E3CH:
        nc.scalar.activation(esq3[:, off:off + cw], e3x[0:100, off:off + cw],
                             AF.Square)
        off += cw
    nc.gpsimd.memset(b2T[:], 0.0)
    for i in range(NQC):
        n = 128 if i < 9 else 48
        b2ps = pconv.tile([128, 2], F32, tag="conv", name="b2ps")
        nc.tensor.matmul(b2ps[0:n, :], esq3[:, i * 128:i * 128 + n], c1[:],
                         start=True, stop=True)
        nc.scalar.copy(b2T[0:n, i:i + 1], b2ps[0:n, 0:1])
    match_chunks(e1x, accDG, accAG, range(3, 18))
    ref_finals(accDG, accAG, accMG, qmaxG)         # DVE combine (overlaps lm)
    match_chunks(e2x, accDL, accAL, range(4))
    ref_transposes(accMG, qmaxG)                   # PE slots between lm MMs
    match_chunks(e2x, accDL, accAL, range(4, 18))
    ref_plane(qmaxG, 0, nc.gpsimd)
    ref_finals(accDL, accAL, accML, qmaxL)
    ref_transposes(accML, qmaxL)
    ref_plane(qmaxL, 1, nc.gpsimd)

    # ------------------------------------------------------- head conv
    r0 = 0
    for nr in (8, 8, 8):
        ps = pconv.tile([1, nr * W], F32, tag="conv", name="convps")
        for s in range(9):
            dy, dx = s // 3, s % 3
            nc.tensor.matmul(ps[:], wt["dshc"][:, s:s + 1],
                             xt3[:, r0 + dy:r0 + dy + nr, dx:dx + W],
                             start=(s == 0), stop=(s == 8))
        nc.scalar.activation(out_sb[0:1, r0 * W:(r0 + nr) * W],
                             _r3(ps[:], nr, W), AF.Identity, bias=bia["dsh"])
        r0 += nr
    nc.sync.dma_start(out_d, out_sb[:])


def build_program():
    import contextlib
    nc = bacc.Bacc("TRN2", target_bir_lowering=False, debug=False,
                   num_devices=8)
    with tile.TileContext(nc) as tc:
        with contextlib.ExitStack() as ctx:
            _emit(nc, tc, ctx)
    nc.compile()
    return nc


def _get_program():
    global _PROG
    if _PROG is None:
        _PROG = build_program()
    return _PROG


CORE_BC = [(0, 2), (0, 3), (1, 2), (1, 3)]


def _wT_flat(w):
    """[Cout, Cin, 3, 3] -> [Cin, 9*Cout]: col block s holds w[:, :, s//3, s%3].T"""
    cout, cin = w.shape[:2]
    out = np.zeros((cin, 9 * cout), np.float32)
    for s in range(9):
        out[:, s * cout:(s + 1) * cout] = w[:, :, s // 3, s % 3].T
    return out


def _pad50(img):
    out = np.zeros((50, 50), np.float32)
    out[1:49, 1:49] = img
    return out


def _im2col9(img, rows, ones_row=False):
    """padded 50x50 -> [9(+1), rows*48] rows ordered s=dy*3+dx."""
    p = _pad50(img)
    rws = [p[dy:dy + rows, dx:dx + W].ravel()
           for dy in range(3) for dx in range(3)]
    if ones_row:
        rws.append(np.ones(rows * W, np.float32))
    return np.stack(rws)


def _blobs(inp, flip, c):
    w = {k: (inp[k][:, :, ::-1, :] if flip else inp[k])
         for k in ["enc1_w", "enc2_w", "bott_w", "dec2_w", "dec1_w",
                   "emb_w", "dsh_w"]}
    seg = {}
    seg["enc1s"] = w["enc1_w"].reshape(16, 3, 9).transpose(2, 1, 0) \
                              .reshape(27, 16)
    seg["enc2"] = _wT_flat(w["enc2_w"])
    seg["bott"] = _wT_flat(w["bott_w"])
    seg["dec2"] = _wT_flat(w["dec2_w"])
    seg["dec1"] = _wT_flat(w["dec1_w"])
    seg["out"] = inp["out_w"][:, :, 0, 0].T
    seg["outc"] = inp["out_w"][c, :, 0, 0][:, None]
    seg["emb"] = np.vstack([w["emb_w"].reshape(100, 9).T,
                            inp["emb_b"][None, :]])
    seg["dshc"] = w["dsh_w"].reshape(7, 9)

    def pack(segs, ncols):
        blob = np.zeros((96, ncols), np.float16)
        off = 0
        for nm, r0, rows, cols in segs:
            blob[r0:r0 + rows, off:off + cols] = seg[nm].astype(np.float16)
            off += cols
        return blob

    blobbias = np.zeros((96, 10), np.float32)
    for nm, col in BIAS_COL.items():
        if nm == "outc":
            v = inp["out_b"][c:c + 1]
        else:
            v = inp[nm + "_b"]
        blobbias[0:len(v), col] = v
    blobbias[32:48, 8] = inp["enc1_b"]
    blobbias[64:96, 9] = inp["enc2_b"]
    return pack(ASEGS, ACOLS), pack(BSEGS, BCOLS), blobbias


def make_in_maps(inp):
    maps = []
    for k8 in range(8):
        n_idx, half = k8 // 2, k8 % 2
        b, c = CORE_BC[n_idx]
        x1c, x2c, x3b = inp["x1"][b, c], inp["x2"][b, c], inp["x3"][b]
        if half:
            x1c, x2c, x3b = x1c[::-1], x2c[::-1], x3b[:, ::-1]
        bx1 = np.zeros((27, 39 * W), np.float32)
        for ci in range(3):
            im9 = _im2col9(x3b[ci], 39)
            for s in range(9):
                bx1[s * 3 + ci] = im9[s]
        bx2 = np.concatenate([_im2col9(x1c, H, True),
                              _im2col9(x2c, H, True)], axis=1)
        bx3 = x2c[0:25, :].reshape(1, Q)
        blobA, blobB, blobbias = _blobs(inp, bool(half), c)
        maps.append({"blobA": blobA, "blobB": blobB,
                     "blobBias": blobbias,
                     "blobX1": np.ascontiguousarray(bx1.astype(np.float16)),
                     "blobX2": np.ascontiguousarray(bx2.astype(np.float16)),
                     "blobX3": np.ascontiguousarray(bx3.astype(np.float16))})
    return maps


def assemble(results):
    out = np.zeros((2, 2, H, W), np.float32)
    for k8, r in enumerate(results):
        n_idx, half = k8 // 2, k8 % 2
        b, c = CORE_BC[n_idx]
        y = r["out"].reshape(24, W)
        if half == 0:
            out[b, c - 2, 0:24] = y
        else:
            out[b, c - 2, 24:48] = y[::-1]
    return out


def kernel(**inputs):
    inp = {k: np.asarray(v) for k, v in inputs.items()}
    nc = _get_program()
    maps = make_in_maps(inp)
    res = run_bass_kernel_spmd(nc, maps, core_ids=list(range(8)), trace=False)
    return assemble(res.results)


# revision 25
# speedup vs baseline: 1.1035x; 1.0169x over previous
"""FEELVOS fused kernel for TRN2, 8-core SPMD — fp16 rev3.

Sharding: the reference only returns logits for classes C-2, C-1, so only 4 of
the 8 fused (batch, class) items matter. 8 cores = 4 (b, c) pairs x 2 frame
halves (top/bottom 24 rows). Bottom-half cores receive row-flipped inputs and
row-flipped conv kernels so every core runs the identical program computing
"top 25 rows" of its (possibly flipped) frame; the host un-flips on gather.

rev5: ref-major matching. The distance matmul puts 128 REF pixels on the
PSUM partitions and all 1200 queries on the free axis, NEGATED (m = 2ab -
|a|^2 - |b|^2 = -d^2, via sign flips host/device), so the reduction over
refs is a MAX. 18 ref chunks are folded by elementwise max-merges into two
fp16 accumulators: even chunks merge straight from PSUM on DVE; odd chunks
are copied PSUM->fp16 by ACT and merged by GPSIMD (SBUF-only engine). This
splits the PSUM-drain floor (~1.1 ns/elem/lane, dtype-independent on any
single engine) across three engines. The final 128-partition max per query
goes through PE transposes + short DVE reduces.

All matmul operands fp16; PSUM fp32. K=104 carries |a|^2 (rows 100/101 of
the ref operand, negated hi+lo pair, vs ones) and |b|^2 (rows 102/103 of
the query operand, negated hi+lo, vs ones), so -d^2 comes out of the
matmul complete.

U-Net decoder convs read single concatenated-K tiles: skip connections are
written at partition offsets via matmul tile_position (enc1 -> cat1[32:48],
enc2 -> cat2[64:96]), halving decoder matmuls. Embedding/square matmuls are
interleaved between U-Net layers to keep the PE busy (HAM warm). The head
conv runs directly on the padded [7,2500] xt plane (9 shifted K=7 matmuls).
"""
import numpy as np

import concourse.bass as bass
import concourse.bacc as bacc
import concourse.tile as tile
from concourse import mybir
from concourse.bass_utils import run_bass_kernel_spmd
from concourse.masks import make_identity

F32 = mybir.dt.float32
F16 = mybir.dt.float16
AF = mybir.ActivationFunctionType
ALU = mybir.AluOpType
AX = mybir.AxisListType

H = W = 48
NREF = H * W                 # 2304 ref pixels (full frame)
QROWS = 25
Q = QROWS * W                # 1200 query pixels
QCH, NQC = 120, 10           # query chunking for the distance matmul
RECH = [512, 512, 512, 512, 256]  # 2304 column chunking (PSUM bank)
E3CH = [432, 384, 384]       # 1200 column chunking
_PROG = None


def _r3(ap, h, w):
    return ap.rearrange("c (h w) -> c h w", h=h, w=w)


# blob column layouts: (name, row0, nrows, cols), fp16. Row offsets place
# weights at the partition base their matmul's contraction rows need.
ASEGS = [("enc1s", 0, 27, 16), ("enc2", 32, 16, 288), ("bott", 64, 32, 576)]
BSEGS = [("dec2", 0, 96, 288), ("dec1", 0, 48, 144), ("out", 0, 16, 4),
         ("outc", 0, 16, 1), ("emb", 0, 10, 100), ("dshc", 0, 7, 9)]


def _offsets(segs):
    off, o = {}, 0
    for nm, _r0, _r, c in segs:
        off[nm] = o
        o += c
    return off, o


AOFF, ACOLS = _offsets(ASEGS)
BOFF, BCOLS = _offsets(BSEGS)
BIAS_COL = {"enc1": 0, "enc2": 1, "bott": 2, "dec2": 3, "dec1": 4,
            "out": 5, "dsh": 6, "outc": 7}


def _emit(nc, tc, ctx):
    # ------------------------------------------------------------- dram io
    bA = nc.dram_tensor("blobA", [96, ACOLS], F16, kind="ExternalInput").ap()
    bB = nc.dram_tensor("blobB", [96, BCOLS], F16, kind="ExternalInput").ap()
    bBias = nc.dram_tensor("blobBias", [96, 10], F32,
                           kind="ExternalInput").ap()
    bX1 = nc.dram_tensor("blobX1", [27, 39 * W], F16,
                         kind="ExternalInput").ap()
    bX2 = nc.dram_tensor("blobX2", [10, 2 * NREF], F16,
                         kind="ExternalInput").ap()
    bX3 = nc.dram_tensor("blobX3", [1, Q], F16, kind="ExternalInput").ap()
    out_d = nc.dram_tensor("out", [1, 24 * W], F32, kind="ExternalOutput").ap()

    # ------------------------------------------------------------- sbuf
    sb = ctx.enter_context(tc.tile_pool(name="sb", bufs=1))

    def st(name, p, f, dt=F16):
        return sb.tile([p, f], dt, tag=name, name=name)

    bloba = st("bloba", 96, ACOLS)
    blobb = st("blobb", 96, BCOLS)
    blobbias = st("blobbias", 96, 10, F32)
    im27 = st("im27", 27, 39 * W)        # enc1 im2col (host-built)
    im2c12 = st("im2c12", 10, 2 * NREF)  # emb im2col e1|e2 (host-built)

    def wseg(blob, off, segs, nm):
        r0, rows, cols = next((a, b, c) for n, a, b, c in segs if n == nm)
        return blob[r0:r0 + rows, off[nm]:off[nm] + cols]

    wt = {nm: wseg(bloba, AOFF, ASEGS, nm) for nm, _, _, _ in ASEGS}
    wt.update({nm: wseg(blobb, BOFF, BSEGS, nm) for nm, _, _, _ in BSEGS})
    bia = {nm: blobbias[0:r, c:c + 1]
           for nm, (r, c) in {"enc1": (16, 0), "enc2": (32, 1),
                              "bott": (64, 2), "dec2": (32, 3),
                              "dec1": (16, 4), "out": (4, 5),
                              "dsh": (1, 6), "outc": (1, 7)}.items()}
    bia["enc1@32"] = blobbias[32:48, 8:9]
    bia["enc2@64"] = blobbias[64:96, 9:10]

    # device-written padded planes (fp16)
    x3cp = st("x3cp", 1, 2500)
    cat1 = st("cat1", 48, 2500)   # rows 0..31 up(d2), rows 32..47 e1
    p1p = st("p1p", 48, 676)      # rows 32..47 used
    cat2 = st("cat2", 96, 676)    # rows 0..63 up(bt), rows 64..95 e2
    p2p = st("p2p", 96, 196)      # rows 64..95 used
    btp = st("btp", 64, 196)
    d2p = st("d2p", 32, 676)
    d1p = st("d1p", 16, 2500)
    xt = st("xt", 7, 2500)

    im2c3 = st("im2c3", 10, Q)           # emb im2col (e3)
    e1x = st("e1x", 102, NREF)           # 0..99 +2*e1, 100/101 -|a|^2 hi/lo
    e2x = st("e2x", 102, NREF)
    e3x = st("e3x", 102, Q)              # 0..99 e3, 100/101 ones
    esq1 = st("esq1", 100, NREF)
    esq2 = st("esq2", 100, NREF)
    esq3 = st("esq3", 100, Q)
    ident = st("ident", 128, 128)

    c025 = st("c025", 100, 1)
    c1 = st("c1", 100, 2)
    hib1 = st("hib1", 1, NREF)
    lob1 = st("lob1", 1, NREF)
    hib2 = st("hib2", 1, NREF)
    lob2 = st("lob2", 1, NREF)
    b2T = st("b2T", 128, NQC, F32)       # |b|^2 in transposed query layout
    accDG = st("accDG", 128, Q)          # fp16 running-max accumulators
    accAG = st("accAG", 128, Q)
    accDL = st("accDL", 128, Q)
    accAL = st("accAL", 128, Q)
    accMG = st("accMG", 128, Q)
    accML = st("accML", 128, Q)
    out_sb = st("out_sb", 1, 24 * W, F32)

    small = ctx.enter_context(tc.tile_pool(name="small", bufs=8))
    tmp = ctx.enter_context(tc.tile_pool(name="tmp", bufs=2))
    scr = ctx.enter_context(tc.tile_pool(name="scr", bufs=3))

    # ------------------------------------------------------------- input dma
    # exact-row segment DMAs, hot-first
    def seg_dma(eng, blobt, blobd, off, segs, nm):
        r0, rows, cols = next((a, b, c) for n, a, b, c in segs if n == nm)
        eng.dma_start(blobt[r0:r0 + rows, off[nm]:off[nm] + cols],
                      blobd[r0:r0 + rows, off[nm]:off[nm] + cols])

    seg_dma(nc.sync, bloba, bA, AOFF, ASEGS, "enc1s")
    nc.gpsimd.dma_start(blobbias[:], bBias)
    nc.sync.dma_start(im27[:, 0:960], bX1[:, 0:960])        # enc1 rows 0..19
    seg_dma(nc.gpsimd, bloba, bA, AOFF, ASEGS, "enc2")
    seg_dma(nc.gpsimd, blobb, bB, BOFF, BSEGS, "emb")
    nc.sync.dma_start(im27[:, 960:39 * W], bX1[:, 960:39 * W])
    nc.sync.dma_start(im2c12[:, 0:NREF // 2], bX2[:, 0:NREF // 2])
    nc.gpsimd.dma_start(im2c12[:, NREF // 2:NREF], bX2[:, NREF // 2:NREF])
    seg_dma(nc.gpsimd, bloba, bA, AOFF, ASEGS, "bott")
    nc.sync.dma_start(im2c12[:, NREF:NREF + NREF // 2],
                      bX2[:, NREF:NREF + NREF // 2])
    nc.gpsimd.dma_start(im2c12[:, NREF + NREF // 2:2 * NREF],
                        bX2[:, NREF + NREF // 2:2 * NREF])
    seg_dma(nc.sync, blobb, bB, BOFF, BSEGS, "dec2")
    nc.sync.dma_start(blobb[0:48, BOFF["dec1"]:BCOLS],
                      bB[0:48, BOFF["dec1"]:BCOLS])  # dec1+out+outc+dshc

    # ------------------------------------------------------------- init
    make_identity(nc, ident[:])
    nc.gpsimd.memset(c025[:], 0.25)
    nc.gpsimd.memset(c1[:], 1.0)
    nc.gpsimd.memset(xt[:], 0.0)
    # engine partition starts must be 32-aligned; rows 96..99 / 0..8 are
    # overwritten later by the embconv ACT / shift DMAs; rows 100/101 of
    # e1x/e2x and 102/103 of e3x by the hi/lo DMAs.
    nc.vector.memset(e1x[96:102, :], 1.0)
    nc.vector.memset(e2x[96:102, :], 1.0)
    nc.vector.memset(e3x[96:102, :], 1.0)
    nc.vector.memset(im2c3[0:10, :], 1.0)

    xt3 = _r3(xt[:], 50, 50)
    x3cp3 = _r3(x3cp[:], 50, 50)

    def borders(eng, ap3, pw):
        eng.memset(ap3[:, 0:1, :], 0.0)
        eng.memset(ap3[:, pw - 1:pw, :], 0.0)
        eng.memset(ap3[:, 1:pw - 1, 0:1], 0.0)
        eng.memset(ap3[:, 1:pw - 1, pw - 1:pw], 0.0)

    cat13 = _r3(cat1[:], 50, 50)
    p1p3 = _r3(p1p[:], 26, 26)
    cat23 = _r3(cat2[:], 26, 26)
    p2p3 = _r3(p2p[:], 14, 14)
    btp3 = _r3(btp[:], 14, 14)
    d2p3 = _r3(d2p[:], 26, 26)
    d1p3 = _r3(d1p[:], 50, 50)

    borders(nc.gpsimd, x3cp3, 50)
    borders(nc.gpsimd, cat13, 50)
    borders(nc.gpsimd, p1p3[32:48], 26)
    borders(nc.vector, cat23, 26)
    borders(nc.vector, p2p3[64:96], 14)
    borders(nc.vector, btp3, 14)
    borders(nc.vector, d2p3, 26)
    borders(nc.gpsimd, d1p3, 50)
    # xt ch6 = x2 rows 0..24 straight from dram (after the xt memset)
    nc.gpsimd.dma_start(xt3[6:7, 1:26, 1:49], bX3)

    pconv = ctx.enter_context(tc.tile_pool(name="pconv", bufs=2, space="PSUM"))
    pmain = ctx.enter_context(tc.tile_pool(name="pmain", bufs=2, space="PSUM"))


    # ------------------------------------------------------------ helpers
    def conv9(src3, wtile, cin, cout, row_chunks, w_, func, bias_ap, dst3,
              pbase=0, obase=0):
        tp = (pbase, obase) if (pbase or obase) else None
        s3 = src3[pbase:pbase + cin]
        r0 = 0
        for nr in row_chunks:
            ps = pconv.tile([obase + cout, nr * w_], F32, tag="conv",
                            name="convps")
            for s in range(9):
                dy, dx = s // 3, s % 3
                nc.tensor.matmul(ps[obase:obase + cout, :],
                                 wtile[:, s * cout:(s + 1) * cout],
                                 s3[:, r0 + dy:r0 + dy + nr, dx:dx + w_],
                                 start=(s == 0), stop=(s == 8),
                                 tile_position=tp)
            nc.scalar.activation(dst3[obase:obase + cout,
                                      1 + r0:1 + r0 + nr, 1:1 + w_],
                                 _r3(ps[obase:obase + cout, :], nr, w_),
                                 func, bias=bias_ap)
            r0 += nr

    def pool2(src3, dst3, orows, ocols, pbase, cch):
        t1 = tmp.tile([pbase + cch, orows * ocols], F16, tag="pool_a",
                      name="poolt1")
        t2 = tmp.tile([pbase + cch, orows * ocols], F16, tag="pool_b",
                      name="poolt2")
        s3 = src3[pbase:pbase + cch]
        v = [s3[:, 1 + a:1 + a + 2 * orows:2, 1 + b:1 + b + 2 * ocols:2]
             for a, b in ((0, 0), (1, 1), (0, 1), (1, 0))]
        t13 = _r3(t1[pbase:pbase + cch, :], orows, ocols)
        t23 = _r3(t2[pbase:pbase + cch, :], orows, ocols)
        nc.vector.tensor_max(t13, v[0], v[1])
        nc.vector.tensor_max(t23, v[2], v[3])
        nc.vector.tensor_max(dst3[pbase:pbase + cch, 1:1 + orows,
                                  1:1 + ocols], t13, t23)

    def up2(src3, sbase, dst3, dbase, cch, irows, icols):
        s = src3[sbase:sbase + cch, 1:1 + irows, 1:1 + icols]
        for a in (0, 1):
            for b in (0, 1):
                nc.vector.tensor_copy(
                    dst3[dbase:dbase + cch, 1 + a:1 + a + 2 * irows:2,
                         1 + b:1 + b + 2 * icols:2], s)

    def embconv(imbuf, chunks, dst, scale):
        off = 0
        for cw in chunks:
            ps = pconv.tile([100, cw], F32, tag="conv", name="convps")
            nc.tensor.matmul(ps[:], wt["emb"], imbuf[:, off:off + cw],
                             start=True, stop=True)
            nc.scalar.activation(dst[0:100, off:off + cw], ps[:],
                                 AF.Copy, scale=scale)
            off += cw

    def sqhilo(src, chunks, lhsT, esq, hib, lob, ex, row):
        """rows(row, row+1) of ex = NEGATED hi/lo fp16 pair of
        lhsT.T @ Square(src)."""
        n = sum(chunks)
        off = 0
        for cw in chunks:
            nc.scalar.activation(esq[:, off:off + cw],
                                 src[0:100, off:off + cw], AF.Square)
            ps = pconv.tile([1, cw], F32, tag="conv", name="sqps")
            nc.tensor.matmul(ps[:], lhsT, esq[:, off:off + cw],
                             start=True, stop=True)
            nc.scalar.activation(hib[0:1, off:off + cw], ps[:], AF.Copy,
                                 scale=-1.0)
            nc.vector.scalar_tensor_tensor(lob[0:1, off:off + cw], ps[:],
                                           -1.0, hib[0:1, off:off + cw],
                                           op0=ALU.mult, op1=ALU.subtract)
            off += cw
        nc.sync.dma_start(ex[row:row + 1, 0:n], hib[0:1, 0:n])
        nc.sync.dma_start(ex[row + 1:row + 2, 0:n], lob[0:1, 0:n])

    # --------------------------------------------- U-Net + emb interleave
    # enc1: im2col matmuls -> cat1[32:48] (tile_position col offset 32)
    r0 = 0
    for nr in (10, 10, 10, 8):
        ps = pconv.tile([48, nr * W], F32, tag="conv", name="convps")
        nc.tensor.matmul(ps[32:48, :], wt["enc1s"],
                         im27[:, r0 * W:(r0 + nr) * W],
                         start=True, stop=True, tile_position=(0, 32))
        nc.scalar.activation(cat13[32:48, 1 + r0:1 + r0 + nr, 1:1 + W],
                             _r3(ps[32:48, :], nr, W), AF.Relu,
                             bias=bia["enc1@32"])
        r0 += nr
    embconv(im2c12[0:10, 0:NREF], RECH, e1x[:], 2.0)       # PE filler
    pool2(cat13, p1p3, 19, 24, 32, 16)
    conv9(p1p3, wt["enc2"], 16, 32, [18], 24, AF.Relu, bia["enc2@64"],
          cat23, pbase=32, obase=64)
    embconv(im2c12[0:10, NREF:2 * NREF], RECH, e2x[:], 2.0)
    pool2(cat23, p2p3, 9, 12, 64, 32)
    conv9(p2p3, wt["bott"], 32, 64, [8], 12, AF.Relu, bia["bott"], btp3,
          pbase=64, obase=0)
    sqhilo(e1x[:], RECH, c025[:], esq1[:], hib1, lob1, e1x[:], 100)
    up2(btp3, 0, cat23, 0, 64, 8, 12)
    conv9(cat23, wt["dec2"], 96, 32, [14], 24, AF.Relu, bia["dec2"], d2p3)
    sqhilo(e2x[:], RECH, c025[:], esq2[:], hib2, lob2, e2x[:], 100)
    up2(d2p3, 0, cat13, 0, 32, 14, 24)
    # dec1 with the 1x1 output convs interleaved per row chunk so x3cp (and
    # the e3 shift DMAs it gates) completes as early as possible
    r0 = 0
    for nr in (10, 10, 6):
        psd = pconv.tile([16, nr * W], F32, tag="conv", name="convps")
        for s in range(9):
            dy, dx = s // 3, s % 3
            nc.tensor.matmul(psd[:], wt["dec1"][:, s * 16:(s + 1) * 16],
                             cat13[:, r0 + dy:r0 + dy + nr, dx:dx + W],
                             start=(s == 0), stop=(s == 8))
        nc.scalar.activation(d1p3[:, 1 + r0:1 + r0 + nr, 1:1 + W],
                             _r3(psd[:], nr, W), AF.Relu, bias=bia["dec1"])
        rhs = d1p3[:, 1 + r0:1 + r0 + nr, 1:1 + W]
        psc = pconv.tile([1, nr * W], F32, tag="conv", name="convps")
        nc.tensor.matmul(psc[:], wt["outc"], rhs, start=True, stop=True)
        nc.scalar.activation(x3cp3[0:1, 1 + r0:1 + r0 + nr, 1:1 + W],
                             _r3(psc[:], nr, W), AF.Identity,
                             bias=bia["outc"])
        ps = pconv.tile([4, nr * W], F32, tag="conv", name="convps")
        nc.tensor.matmul(ps[:], wt["out"], rhs, start=True, stop=True)
        nc.scalar.activation(xt3[0:4, 1 + r0:1 + r0 + nr, 1:1 + W],
                             _r3(ps[:], nr, W), AF.Identity, bias=bia["out"])
        r0 += nr

    # ------------------------------------------------------- embedding 3
    # shift DMAs split by query-row chunk: each piece only needs the x3cp
    # rows its outc chunk has produced, so they stream during dec1.
    for r0, r1 in ((0, 9), (9, 18), (18, 25)):
        for s in range(9):
            dy, dx = s // 3, s % 3
            eng = nc.sync if s % 2 == 0 else nc.gpsimd
            eng.dma_start(im2c3[s:s + 1, r0 * W:r1 * W],
                          x3cp3[0:1, r0 + dy:r1 + dy, dx:dx + W])
    embconv(im2c3[:], E3CH, e3x[:], 1.0)

    # ------------------------------------------------------- matching
    nc.vector.memset(accDG[:], -60000.0)
    nc.vector.memset(accAG[:], -60000.0)
    nc.vector.memset(accDL[:], -60000.0)
    nc.vector.memset(accAL[:], -60000.0)

    def match_chunks(ex, accD, accA, k_range):
        for k in k_range:
            lhsT = ex[:][:, k * 128:(k + 1) * 128]
            ps = pmain.tile([128, Q], F32, tag="main", name="mainps")
            for o, n in ((0, 512), (512, 512), (1024, 176)):
                nc.tensor.matmul(ps[:, o:o + n], lhsT,
                                 e3x[0:102, o:o + n], start=True, stop=True)
            if k % 4 == 0:          # DVE drains PSUM directly (1x rate)
                nc.vector.tensor_max(accD[:], ps[:], accD[:])
            else:                    # ACT copies; DVE merges all-fp16 (2x)
                sc = scr.tile([128, Q], F16, tag="scr", name="scrt")
                nc.scalar.copy(sc[:], ps[:])
                nc.vector.tensor_max(accA[:], sc[:], accA[:])

    def ref_finals(accD, accA, accM, qmax):
        nc.vector.tensor_max(accM[:], accD[:], accA[:])

    def ref_transposes(accM, qmax):
        for i in range(NQC):
            n = 128 if i < 9 else 48
            pst = pconv.tile([n, 128], F16, tag="conv", name="trps")
            nc.tensor.transpose(pst[:], accM[:, i * 128:i * 128 + n],
                                ident[:128, :128])
            nc.vector.tensor_reduce(qmax[0:n, i:i + 1], pst[:],
                                    axis=AX.X, op=ALU.max)

    def ref_plane(qmax, r, eng):
        dsub = small.tile([128, NQC], F32, tag="dsub", name="dsub")
        nc.vector.scalar_tensor_tensor(dsub[:], qmax[:], 1.0, b2T[:],
                                       op0=ALU.mult, op1=ALU.subtract)
        tneg = small.tile([128, NQC], F32, tag="tneg", name="tneg")
        nc.vector.tensor_scalar(tneg[:], dsub[:], 0.0, 0.0,
                                op0=ALU.min, op1=ALU.min)
        gcol = small.tile([128, NQC], F16, tag="gcol", name="gcol")
        nc.scalar.activation(gcol[:], tneg[:], AF.Tanh, scale=-0.5)
        pst = pconv.tile([NQC, 128], F16, tag="conv", name="gmtps")
        nc.tensor.transpose(pst[:], gcol[:], ident[:128, :128])
        gcolT = small.tile([NQC, 128], F16, tag="gcolT", name="gcolT")
        nc.scalar.copy(gcolT[:], pst[:])
        gflat = small.tile([1, Q], F16, tag="gflat", name="gflat")
        eng.dma_start(gflat[0:1, 0:1152], gcolT[0:9, :])
        eng.dma_start(gflat[0:1, 1152:1200], gcolT[9:10, 0:48])
        # plane in row pieces: the head's first chunks unblock early
        eng.dma_start(xt3[4 + r:5 + r, 1:11, 1:49], gflat[0:1, 0:480])
        eng.dma_start(xt3[4 + r:5 + r, 11:19, 1:49], gflat[0:1, 480:864])
        eng.dma_start(xt3[4 + r:5 + r, 19:26, 1:49], gflat[0:1, 864:1200])

    qmaxG = small.tile([128, NQC], F32, tag="qmaxG", name="qmaxG")
    qmaxL = small.tile([128, NQC], F32, tag="qmaxL", name="qmaxL")

    match_chunks(e1x, accDG, accAG, range(3))      # gm starts immediately
    # |b|^2 in transposed layout (commutes with the ref-max): Square + 10
    # small matmuls slot between gm chunks; only the qmax tail needs them.
    off = 0
    for cw in E3CH:
        nc.scalar.activation(esq3[:, off:off + cw], e3x[0:100, off:off + cw],
                             AF.Square)
        off += cw
    nc.gpsimd.memset(b2T[:], 0.0)
    for i in range(NQC):
        n = 128 if i < 9 else 48
        b2ps = pconv.tile([128, 2], F32, tag="conv", name="b2ps")
        nc.tensor.matmul(b2ps[0:n, :], esq3[:, i * 128:i * 128 + n], c1[:],
                         start=True, stop=True)
        nc.scalar.copy(b2T[0:n, i:i + 1], b2ps[0:n, 0:1])
    match_chunks(e1x, accDG, accAG, range(3, 18))
    ref_finals(accDG, accAG, accMG, qmaxG)         # DVE combine (overlaps lm)
    match_chunks(e2x, accDL, accAL, range(4))
    ref_transposes(accMG, qmaxG)                   # PE slots between lm MMs
    match_chunks(e2x, accDL, accAL, range(4, 18))
    ref_plane(qmaxG, 0, nc.gpsimd)
    ref_finals(accDL, accAL, accML, qmaxL)
    ref_transposes(accML, qmaxL)
    ref_plane(qmaxL, 1, nc.gpsimd)

    # ------------------------------------------------------- head conv
    r0 = 0
    for nr in (8, 8, 8):
        ps = pconv.tile([1, nr * W], F32, tag="conv", name="convps")
        for s in range(9):
            dy, dx = s // 3, s % 3
            nc.tensor.matmul(ps[:], wt["dshc"][:, s:s + 1],
                             xt3[:, r0 + dy:r0 + dy + nr, dx:dx + W],
                             start=(s == 0), stop=(s == 8))
        nc.scalar.activation(out_sb[0:1, r0 * W:(r0 + nr) * W],
                             _r3(ps[:], nr, W), AF.Identity, bias=bia["dsh"])
        r0 += nr
    nc.sync.dma_start(out_d, out_sb[:])


def build_program():
    import contextlib
    nc = bacc.Bacc("TRN2", target_bir_lowering=False, debug=False,
                   num_devices=8)
    with tile.TileContext(nc) as tc:
        with contextlib.ExitStack() as ctx:
            _emit(nc, tc, ctx)
    nc.compile()
    return nc


def _get_program():
    global _PROG
    if _PROG is None:
        _PROG = build_program()
    return _PROG


CORE_BC = [(0, 2), (0, 3), (1, 2), (1, 3)]


def _wT_flat(w):
    """[Cout, Cin, 3, 3] -> [Cin, 9*Cout]: col block s holds w[:, :, s//3, s%3].T"""
    cout, cin = w.shape[:2]
    out = np.zeros((cin, 9 * cout), np.float32)
    for s in range(9):
        out[:, s * cout:(s + 1) * cout] = w[:, :, s // 3, s % 3].T
    return out


def _pad50(img):
    out = np.zeros((50, 50), np.float32)
    out[1:49, 1:49] = img
    return out


def _im2col9(img, rows, ones_row=False):
    """padded 50x50 -> [9(+1), rows*48] rows ordered s=dy*3+dx."""
    p = _pad50(img)
    rws = [p[dy:dy + rows, dx:dx + W].ravel()
           for dy in range(3) for dx in range(3)]
    if ones_row:
        rws.append(np.ones(rows * W, np.float32))
    return np.stack(rws)


def _blobs(inp, flip, c):
    w = {k: (inp[k][:, :, ::-1, :] if flip else inp[k])
         for k in ["enc1_w", "enc2_w", "bott_w", "dec2_w", "dec1_w",
                   "emb_w", "dsh_w"]}
    seg = {}
    seg["enc1s"] = w["enc1_w"].reshape(16, 3, 9).transpose(2, 1, 0) \
                              .reshape(27, 16)
    seg["enc2"] = _wT_flat(w["enc2_w"])
    seg["bott"] = _wT_flat(w["bott_w"])
    seg["dec2"] = _wT_flat(w["dec2_w"])
    seg["dec1"] = _wT_flat(w["dec1_w"])
    seg["out"] = inp["out_w"][:, :, 0, 0].T
    seg["outc"] = inp["out_w"][c, :, 0, 0][:, None]
    seg["emb"] = np.vstack([w["emb_w"].reshape(100, 9).T,
                            inp["emb_b"][None, :]])
    seg["dshc"] = w["dsh_w"].reshape(7, 9)

    def pack(segs, ncols):
        blob = np.zeros((96, ncols), np.float16)
        off = 0
        for nm, r0, rows, cols in segs:
            blob[r0:r0 + rows, off:off + cols] = seg[nm].astype(np.float16)
            off += cols
        return blob

    blobbias = np.zeros((96, 10), np.float32)
    for nm, col in BIAS_COL.items():
        if nm == "outc":
            v = inp["out_b"][c:c + 1]
        else:
            v = inp[nm + "_b"]
        blobbias[0:len(v), col] = v
    blobbias[32:48, 8] = inp["enc1_b"]
    blobbias[64:96, 9] = inp["enc2_b"]
    return pack(ASEGS, ACOLS), pack(BSEGS, BCOLS), blobbias


def make_in_maps(inp):
    maps = []
    for k8 in range(8):
        n_idx, half = k8 // 2, k8 % 2
        b, c = CORE_BC[n_idx]
        x1c, x2c, x3b = inp["x1"][b, c], inp["x2"][b, c], inp["x3"][b]
        if half:
            x1c, x2c, x3b = x1c[::-1], x2c[::-1], x3b[:, ::-1]
        bx1 = np.zeros((27, 39 * W), np.float32)
        for ci in range(3):
            im9 = _im2col9(x3b[ci], 39)
            for s in range(9):
                bx1[s * 3 + ci] = im9[s]
        bx2 = np.concatenate([_im2col9(x1c, H, True),
                              _im2col9(x2c, H, True)], axis=1)
        bx3 = x2c[0:25, :].reshape(1, Q)
        blobA, blobB, blobbias = _blobs(inp, bool(half), c)
        maps.append({"blobA": blobA, "blobB": blobB,
                     "blobBias": blobbias,
                     "blobX1": np.ascontiguousarray(bx1.astype(np.float16)),
                     "blobX2": np.ascontiguousarray(bx2.astype(np.float16)),
                     "blobX3": np.ascontiguousarray(bx3.astype(np.float16))})
    return maps


def assemble(results):
    out = np.zeros((2, 2, H, W), np.float32)
    for k8, r in enumerate(results):
        n_idx, half = k8 // 2, k8 % 2
        b, c = CORE_BC[n_idx]
        y = r["out"].reshape(24, W)
        if half == 0:
            out[b, c - 2, 0:24] = y
        else:
            out[b, c - 2, 24:48] = y[::-1]
    return out


def kernel(**inputs):
    inp = {k: np.asarray(v) for k, v in inputs.items()}
    nc = _get_program()
    maps = make_in_maps(inp)
    res = run_bass_kernel_spmd(nc, maps, core_ids=list(range(8)), trace=False)
    return assemble(res.results)


# revision 26
# speedup vs baseline: 1.1037x; 1.0001x over previous
"""FEELVOS fused kernel for TRN2, 8-core SPMD — fp16 rev3.

Sharding: the reference only returns logits for classes C-2, C-1, so only 4 of
the 8 fused (batch, class) items matter. 8 cores = 4 (b, c) pairs x 2 frame
halves (top/bottom 24 rows). Bottom-half cores receive row-flipped inputs and
row-flipped conv kernels so every core runs the identical program computing
"top 25 rows" of its (possibly flipped) frame; the host un-flips on gather.

rev5: ref-major matching. The distance matmul puts 128 REF pixels on the
PSUM partitions and all 1200 queries on the free axis, NEGATED (m = 2ab -
|a|^2 - |b|^2 = -d^2, via sign flips host/device), so the reduction over
refs is a MAX. 18 ref chunks are folded by elementwise max-merges into two
fp16 accumulators: even chunks merge straight from PSUM on DVE; odd chunks
are copied PSUM->fp16 by ACT and merged by GPSIMD (SBUF-only engine). This
splits the PSUM-drain floor (~1.1 ns/elem/lane, dtype-independent on any
single engine) across three engines. The final 128-partition max per query
goes through PE transposes + short DVE reduces.

All matmul operands fp16; PSUM fp32. K=104 carries |a|^2 (rows 100/101 of
the ref operand, negated hi+lo pair, vs ones) and |b|^2 (rows 102/103 of
the query operand, negated hi+lo, vs ones), so -d^2 comes out of the
matmul complete.

U-Net decoder convs read single concatenated-K tiles: skip connections are
written at partition offsets via matmul tile_position (enc1 -> cat1[32:48],
enc2 -> cat2[64:96]), halving decoder matmuls. Embedding/square matmuls are
interleaved between U-Net layers to keep the PE busy (HAM warm). The head
conv runs directly on the padded [7,2500] xt plane (9 shifted K=7 matmuls).
"""
import numpy as np

import concourse.bass as bass
import concourse.bacc as bacc
import concourse.tile as tile
from concourse import mybir
from concourse.bass_utils import run_bass_kernel_spmd
from concourse.masks import make_identity

F32 = mybir.dt.float32
F16 = mybir.dt.float16
AF = mybir.ActivationFunctionType
ALU = mybir.AluOpType
AX = mybir.AxisListType

H = W = 48
NREF = H * W                 # 2304 ref pixels (full frame)
QROWS = 25
Q = QROWS * W                # 1200 query pixels
QCH, NQC = 120, 10           # query chunking for the distance matmul
RECH = [512, 512, 512, 512, 256]  # 2304 column chunking (PSUM bank)
E3CH = [432, 384, 384]       # 1200 column chunking
_PROG = None


def _r3(ap, h, w):
    return ap.rearrange("c (h w) -> c h w", h=h, w=w)


# blob column layouts: (name, row0, nrows, cols), fp16. Row offsets place
# weights at the partition base their matmul's contraction rows need.
ASEGS = [("enc1s", 0, 27, 16), ("enc2", 32, 16, 288), ("bott", 64, 32, 576)]
BSEGS = [("dec2", 0, 96, 288), ("dec1", 0, 48, 144), ("out", 0, 16, 4),
         ("outc", 0, 16, 1), ("emb", 0, 10, 100), ("dshc", 0, 7, 9)]


def _offsets(segs):
    off, o = {}, 0
    for nm, _r0, _r, c in segs:
        off[nm] = o
        o += c
    return off, o


AOFF, ACOLS = _offsets(ASEGS)
BOFF, BCOLS = _offsets(BSEGS)
BIAS_COL = {"enc1": 0, "enc2": 1, "bott": 2, "dec2": 3, "dec1": 4,
            "out": 5, "dsh": 6, "outc": 7}


def _emit(nc, tc, ctx):
    # ------------------------------------------------------------- dram io
    bA = nc.dram_tensor("blobA", [96, ACOLS], F16, kind="ExternalInput").ap()
    bB = nc.dram_tensor("blobB", [96, BCOLS], F16, kind="ExternalInput").ap()
    bBias = nc.dram_tensor("blobBias", [96, 10], F32,
                           kind="ExternalInput").ap()
    bX1 = nc.dram_tensor("blobX1", [27, 39 * W], F16,
                         kind="ExternalInput").ap()
    bX2 = nc.dram_tensor("blobX2", [10, 2 * NREF], F16,
                         kind="ExternalInput").ap()
    bX3 = nc.dram_tensor("blobX3", [1, Q], F16, kind="ExternalInput").ap()
    out_d = nc.dram_tensor("out", [1, 24 * W], F32, kind="ExternalOutput").ap()

    # ------------------------------------------------------------- sbuf
    sb = ctx.enter_context(tc.tile_pool(name="sb", bufs=1))

    def st(name, p, f, dt=F16):
        return sb.tile([p, f], dt, tag=name, name=name)

    bloba = st("bloba", 96, ACOLS)
    blobb = st("blobb", 96, BCOLS)
    blobbias = st("blobbias", 96, 10, F32)
    im27 = st("im27", 27, 39 * W)        # enc1 im2col (host-built)
    im2c12 = st("im2c12", 10, 2 * NREF)  # emb im2col e1|e2 (host-built)

    def wseg(blob, off, segs, nm):
        r0, rows, cols = next((a, b, c) for n, a, b, c in segs if n == nm)
        return blob[r0:r0 + rows, off[nm]:off[nm] + cols]

    wt = {nm: wseg(bloba, AOFF, ASEGS, nm) for nm, _, _, _ in ASEGS}
    wt.update({nm: wseg(blobb, BOFF, BSEGS, nm) for nm, _, _, _ in BSEGS})
    bia = {nm: blobbias[0:r, c:c + 1]
           for nm, (r, c) in {"enc1": (16, 0), "enc2": (32, 1),
                              "bott": (64, 2), "dec2": (32, 3),
                              "dec1": (16, 4), "out": (4, 5),
                              "dsh": (1, 6), "outc": (1, 7)}.items()}
    bia["enc1@32"] = blobbias[32:48, 8:9]
    bia["enc2@64"] = blobbias[64:96, 9:10]

    # device-written padded planes (fp16)
    x3cp = st("x3cp", 1, 2500)
    cat1 = st("cat1", 48, 2500)   # rows 0..31 up(d2), rows 32..47 e1
    p1p = st("p1p", 48, 676)      # rows 32..47 used
    cat2 = st("cat2", 96, 676)    # rows 0..63 up(bt), rows 64..95 e2
    p2p = st("p2p", 96, 196)      # rows 64..95 used
    btp = st("btp", 64, 196)
    d2p = st("d2p", 32, 676)
    d1p = st("d1p", 16, 2500)
    xt = st("xt", 7, 2500)

    im2c3 = st("im2c3", 10, Q)           # emb im2col (e3)
    e1x = st("e1x", 102, NREF)           # 0..99 +2*e1, 100/101 -|a|^2 hi/lo
    e2x = st("e2x", 102, NREF)
    e3x = st("e3x", 102, Q)              # 0..99 e3, 100/101 ones
    esq1 = st("esq1", 100, NREF)
    esq2 = st("esq2", 100, NREF)
    esq3 = st("esq3", 100, Q)
    ident = st("ident", 128, 128)

    c025 = st("c025", 100, 1)
    c1 = st("c1", 100, 2)
    hib1 = st("hib1", 1, NREF)
    lob1 = st("lob1", 1, NREF)
    hib2 = st("hib2", 1, NREF)
    lob2 = st("lob2", 1, NREF)
    b2T = st("b2T", 128, NQC, F32)       # |b|^2 in transposed query layout
    accDG = st("accDG", 128, Q)          # fp16 running-max accumulators
    accAG = st("accAG", 128, Q)
    accDL = st("accDL", 128, Q)
    accAL = st("accAL", 128, Q)
    accMG = st("accMG", 128, Q)
    accML = st("accML", 128, Q)
    out_sb = st("out_sb", 1, 24 * W, F32)

    small = ctx.enter_context(tc.tile_pool(name="small", bufs=8))
    tmp = ctx.enter_context(tc.tile_pool(name="tmp", bufs=2))
    scr = ctx.enter_context(tc.tile_pool(name="scr", bufs=3))

    # ------------------------------------------------------------- input dma
    # exact-row segment DMAs, hot-first
    def seg_dma(eng, blobt, blobd, off, segs, nm):
        r0, rows, cols = next((a, b, c) for n, a, b, c in segs if n == nm)
        eng.dma_start(blobt[r0:r0 + rows, off[nm]:off[nm] + cols],
                      blobd[r0:r0 + rows, off[nm]:off[nm] + cols])

    seg_dma(nc.sync, bloba, bA, AOFF, ASEGS, "enc1s")
    nc.gpsimd.dma_start(blobbias[:], bBias)
    nc.sync.dma_start(im27[:, 0:960], bX1[:, 0:960])        # enc1 rows 0..19
    seg_dma(nc.gpsimd, bloba, bA, AOFF, ASEGS, "enc2")
    seg_dma(nc.gpsimd, blobb, bB, BOFF, BSEGS, "emb")
    nc.sync.dma_start(im27[:, 960:39 * W], bX1[:, 960:39 * W])
    nc.sync.dma_start(im2c12[:, 0:NREF // 2], bX2[:, 0:NREF // 2])
    nc.gpsimd.dma_start(im2c12[:, NREF // 2:NREF], bX2[:, NREF // 2:NREF])
    seg_dma(nc.gpsimd, bloba, bA, AOFF, ASEGS, "bott")
    nc.sync.dma_start(im2c12[:, NREF:NREF + NREF // 2],
                      bX2[:, NREF:NREF + NREF // 2])
    nc.gpsimd.dma_start(im2c12[:, NREF + NREF // 2:2 * NREF],
                        bX2[:, NREF + NREF // 2:2 * NREF])
    seg_dma(nc.sync, blobb, bB, BOFF, BSEGS, "dec2")
    nc.sync.dma_start(blobb[0:48, BOFF["dec1"]:BCOLS],
                      bB[0:48, BOFF["dec1"]:BCOLS])  # dec1+out+outc+dshc

    # ------------------------------------------------------------- init
    make_identity(nc, ident[:])
    nc.gpsimd.memset(c025[:], 0.25)
    nc.gpsimd.memset(c1[:], 1.0)
    nc.gpsimd.memset(xt[:], 0.0)
    # engine partition starts must be 32-aligned; rows 96..99 / 0..8 are
    # overwritten later by the embconv ACT / shift DMAs; rows 100/101 of
    # e1x/e2x and 102/103 of e3x by the hi/lo DMAs.
    nc.vector.memset(e1x[96:102, :], 1.0)
    nc.vector.memset(e2x[96:102, :], 1.0)
    nc.vector.memset(e3x[96:102, :], 1.0)
    nc.vector.memset(im2c3[0:10, :], 1.0)

    xt3 = _r3(xt[:], 50, 50)
    x3cp3 = _r3(x3cp[:], 50, 50)

    def borders(eng, ap3, pw):
        eng.memset(ap3[:, 0:1, :], 0.0)
        eng.memset(ap3[:, pw - 1:pw, :], 0.0)
        eng.memset(ap3[:, 1:pw - 1, 0:1], 0.0)
        eng.memset(ap3[:, 1:pw - 1, pw - 1:pw], 0.0)

    cat13 = _r3(cat1[:], 50, 50)
    p1p3 = _r3(p1p[:], 26, 26)
    cat23 = _r3(cat2[:], 26, 26)
    p2p3 = _r3(p2p[:], 14, 14)
    btp3 = _r3(btp[:], 14, 14)
    d2p3 = _r3(d2p[:], 26, 26)
    d1p3 = _r3(d1p[:], 50, 50)

    borders(nc.gpsimd, x3cp3, 50)
    borders(nc.gpsimd, cat13, 50)
    borders(nc.gpsimd, p1p3[32:48], 26)
    borders(nc.vector, cat23, 26)
    borders(nc.vector, p2p3[64:96], 14)
    borders(nc.vector, btp3, 14)
    borders(nc.vector, d2p3, 26)
    borders(nc.gpsimd, d1p3, 50)
    # xt ch6 = x2 rows 0..24 straight from dram (after the xt memset)
    nc.gpsimd.dma_start(xt3[6:7, 1:26, 1:49], bX3)

    pconv = ctx.enter_context(tc.tile_pool(name="pconv", bufs=2, space="PSUM"))
    pmain = ctx.enter_context(tc.tile_pool(name="pmain", bufs=2, space="PSUM"))


    # ------------------------------------------------------------ helpers
    def conv9(src3, wtile, cin, cout, row_chunks, w_, func, bias_ap, dst3,
              pbase=0, obase=0):
        tp = (pbase, obase) if (pbase or obase) else None
        s3 = src3[pbase:pbase + cin]
        r0 = 0
        for nr in row_chunks:
            ps = pconv.tile([obase + cout, nr * w_], F32, tag="conv",
                            name="convps")
            for s in range(9):
                dy, dx = s // 3, s % 3
                nc.tensor.matmul(ps[obase:obase + cout, :],
                                 wtile[:, s * cout:(s + 1) * cout],
                                 s3[:, r0 + dy:r0 + dy + nr, dx:dx + w_],
                                 start=(s == 0), stop=(s == 8),
                                 tile_position=tp)
            nc.scalar.activation(dst3[obase:obase + cout,
                                      1 + r0:1 + r0 + nr, 1:1 + w_],
                                 _r3(ps[obase:obase + cout, :], nr, w_),
                                 func, bias=bias_ap)
            r0 += nr

    def pool2(src3, dst3, orows, ocols, pbase, cch):
        t1 = tmp.tile([pbase + cch, orows * ocols], F16, tag="pool_a",
                      name="poolt1")
        t2 = tmp.tile([pbase + cch, orows * ocols], F16, tag="pool_b",
                      name="poolt2")
        s3 = src3[pbase:pbase + cch]
        v = [s3[:, 1 + a:1 + a + 2 * orows:2, 1 + b:1 + b + 2 * ocols:2]
             for a, b in ((0, 0), (1, 1), (0, 1), (1, 0))]
        t13 = _r3(t1[pbase:pbase + cch, :], orows, ocols)
        t23 = _r3(t2[pbase:pbase + cch, :], orows, ocols)
        nc.vector.tensor_max(t13, v[0], v[1])
        nc.vector.tensor_max(t23, v[2], v[3])
        nc.vector.tensor_max(dst3[pbase:pbase + cch, 1:1 + orows,
                                  1:1 + ocols], t13, t23)

    def up2(src3, sbase, dst3, dbase, cch, irows, icols):
        s = src3[sbase:sbase + cch, 1:1 + irows, 1:1 + icols]
        for a in (0, 1):
            for b in (0, 1):
                nc.vector.tensor_copy(
                    dst3[dbase:dbase + cch, 1 + a:1 + a + 2 * irows:2,
                         1 + b:1 + b + 2 * icols:2], s)

    def embconv(imbuf, chunks, dst, scale):
        off = 0
        for cw in chunks:
            ps = pconv.tile([100, cw], F32, tag="conv", name="convps")
            nc.tensor.matmul(ps[:], wt["emb"], imbuf[:, off:off + cw],
                             start=True, stop=True)
            nc.scalar.activation(dst[0:100, off:off + cw], ps[:],
                                 AF.Copy, scale=scale)
            off += cw

    def sqhilo(src, chunks, lhsT, esq, hib, lob, ex, row):
        """rows(row, row+1) of ex = NEGATED hi/lo fp16 pair of
        lhsT.T @ Square(src)."""
        n = sum(chunks)
        off = 0
        for cw in chunks:
            nc.scalar.activation(esq[:, off:off + cw],
                                 src[0:100, off:off + cw], AF.Square)
            ps = pconv.tile([1, cw], F32, tag="conv", name="sqps")
            nc.tensor.matmul(ps[:], lhsT, esq[:, off:off + cw],
                             start=True, stop=True)
            nc.scalar.activation(hib[0:1, off:off + cw], ps[:], AF.Copy,
                                 scale=-1.0)
            nc.vector.scalar_tensor_tensor(lob[0:1, off:off + cw], ps[:],
                                           -1.0, hib[0:1, off:off + cw],
                                           op0=ALU.mult, op1=ALU.subtract)
            off += cw
        nc.sync.dma_start(ex[row:row + 1, 0:n], hib[0:1, 0:n])
        nc.sync.dma_start(ex[row + 1:row + 2, 0:n], lob[0:1, 0:n])

    # --------------------------------------------- U-Net + emb interleave
    # enc1: im2col matmuls -> cat1[32:48] (tile_position col offset 32)
    r0 = 0
    for nr in (10, 10, 10, 8):
        ps = pconv.tile([48, nr * W], F32, tag="conv", name="convps")
        nc.tensor.matmul(ps[32:48, :], wt["enc1s"],
                         im27[:, r0 * W:(r0 + nr) * W],
                         start=True, stop=True, tile_position=(0, 32))
        nc.scalar.activation(cat13[32:48, 1 + r0:1 + r0 + nr, 1:1 + W],
                             _r3(ps[32:48, :], nr, W), AF.Relu,
                             bias=bia["enc1@32"])
        r0 += nr
    embconv(im2c12[0:10, 0:NREF], RECH, e1x[:], 2.0)       # PE filler
    pool2(cat13, p1p3, 19, 24, 32, 16)
    conv9(p1p3, wt["enc2"], 16, 32, [18], 24, AF.Relu, bia["enc2@64"],
          cat23, pbase=32, obase=64)
    embconv(im2c12[0:10, NREF:2 * NREF], RECH, e2x[:], 2.0)
    pool2(cat23, p2p3, 9, 12, 64, 32)
    conv9(p2p3, wt["bott"], 32, 64, [8], 12, AF.Relu, bia["bott"], btp3,
          pbase=64, obase=0)
    sqhilo(e1x[:], RECH, c025[:], esq1[:], hib1, lob1, e1x[:], 100)
    up2(btp3, 0, cat23, 0, 64, 8, 12)
    conv9(cat23, wt["dec2"], 96, 32, [14], 24, AF.Relu, bia["dec2"], d2p3)
    sqhilo(e2x[:], RECH, c025[:], esq2[:], hib2, lob2, e2x[:], 100)
    up2(d2p3, 0, cat13, 0, 32, 14, 24)
    # dec1 with the 1x1 output convs interleaved per row chunk so x3cp (and
    # the e3 shift DMAs it gates) completes as early as possible
    r0 = 0
    for nr in (10, 10, 6):
        psd = pconv.tile([16, nr * W], F32, tag="conv", name="convps")
        for s in range(9):
            dy, dx = s // 3, s % 3
            nc.tensor.matmul(psd[:], wt["dec1"][:, s * 16:(s + 1) * 16],
                             cat13[:, r0 + dy:r0 + dy + nr, dx:dx + W],
                             start=(s == 0), stop=(s == 8))
        nc.scalar.activation(d1p3[:, 1 + r0:1 + r0 + nr, 1:1 + W],
                             _r3(psd[:], nr, W), AF.Relu, bias=bia["dec1"])
        rhs = d1p3[:, 1 + r0:1 + r0 + nr, 1:1 + W]
        psc = pconv.tile([1, nr * W], F32, tag="conv", name="convps")
        nc.tensor.matmul(psc[:], wt["outc"], rhs, start=True, stop=True)
        nc.scalar.activation(x3cp3[0:1, 1 + r0:1 + r0 + nr, 1:1 + W],
                             _r3(psc[:], nr, W), AF.Identity,
                             bias=bia["outc"])
        ps = pconv.tile([4, nr * W], F32, tag="conv", name="convps")
        nc.tensor.matmul(ps[:], wt["out"], rhs, start=True, stop=True)
        nc.scalar.activation(xt3[0:4, 1 + r0:1 + r0 + nr, 1:1 + W],
                             _r3(ps[:], nr, W), AF.Identity, bias=bia["out"])
        r0 += nr

    # ------------------------------------------------------- embedding 3
    # shift DMAs split by query-row chunk: each piece only needs the x3cp
    # rows its outc chunk has produced, so they stream during dec1.
    for r0, r1 in ((0, 9), (9, 18), (18, 25)):
        for s in range(9):
            dy, dx = s // 3, s % 3
            eng = nc.sync if s % 2 == 0 else nc.gpsimd
            eng.dma_start(im2c3[s:s + 1, r0 * W:r1 * W],
                          x3cp3[0:1, r0 + dy:r1 + dy, dx:dx + W])
    embconv(im2c3[:], E3CH, e3x[:], 1.0)

    # ------------------------------------------------------- matching
    nc.vector.memset(accDG[:], -60000.0)
    nc.vector.memset(accAG[:], -60000.0)
    nc.vector.memset(accDL[:], -60000.0)
    nc.vector.memset(accAL[:], -60000.0)

    def match_chunks(ex, accD, accA, k_range):
        for k in k_range:
            lhsT = ex[:][:, k * 128:(k + 1) * 128]
            ps = pmain.tile([128, Q], F32, tag="main", name="mainps")
            for o, n in ((0, 512), (512, 512), (1024, 176)):
                nc.tensor.matmul(ps[:, o:o + n], lhsT,
                                 e3x[0:102, o:o + n], start=True, stop=True)
            if k % 4 == 0:          # DVE drains PSUM directly (1x rate)
                nc.vector.tensor_max(accD[:], ps[:], accD[:])
            else:                    # ACT copies; DVE merges all-fp16 (2x)
                sc = scr.tile([128, Q], F16, tag="scr", name="scrt")
                nc.scalar.copy(sc[:], ps[:])
                nc.vector.tensor_max(accA[:], sc[:], accA[:])

    def ref_finals(accD, accA, accM, qmax):
        nc.vector.tensor_max(accM[:], accD[:], accA[:])

    def ref_transposes(accM, qmax):
        for i in range(NQC):
            n = 128 if i < 9 else 48
            pst = pconv.tile([n, 128], F16, tag="conv", name="trps")
            nc.tensor.transpose(pst[:], accM[:, i * 128:i * 128 + n],
                                ident[:128, :128])
            nc.vector.tensor_reduce(qmax[0:n, i:i + 1], pst[:],
                                    axis=AX.X, op=ALU.max)

    def ref_plane(qmax, r, eng):
        dsub = small.tile([128, NQC], F32, tag="dsub", name="dsub")
        nc.vector.scalar_tensor_tensor(dsub[:], qmax[:], 1.0, b2T[:],
                                       op0=ALU.mult, op1=ALU.subtract)
        tneg = small.tile([128, NQC], F32, tag="tneg", name="tneg")
        nc.vector.tensor_scalar(tneg[:], dsub[:], 0.0, 0.0,
                                op0=ALU.min, op1=ALU.min)
        gcol = small.tile([128, NQC], F16, tag="gcol", name="gcol")
        nc.scalar.activation(gcol[:], tneg[:], AF.Tanh, scale=-0.5)
        pst = pconv.tile([NQC, 128], F16, tag="conv", name="gmtps")
        nc.tensor.transpose(pst[:], gcol[:], ident[:128, :128])
        gcolT = small.tile([NQC, 128], F16, tag="gcolT", name="gcolT")
        nc.scalar.copy(gcolT[:], pst[:])
        gflat = small.tile([1, Q], F16, tag="gflat", name="gflat")
        eng2 = nc.sync if eng is nc.gpsimd else nc.gpsimd
        eng.dma_start(gflat[0:1, 0:1152], gcolT[0:9, :])
        eng2.dma_start(gflat[0:1, 1152:1200], gcolT[9:10, 0:48])
        # plane in row pieces on alternating queues: the head's first
        # chunks unblock early and the issue cost is split
        eng2.dma_start(xt3[4 + r:5 + r, 1:11, 1:49], gflat[0:1, 0:480])
        eng.dma_start(xt3[4 + r:5 + r, 11:19, 1:49], gflat[0:1, 480:864])
        eng2.dma_start(xt3[4 + r:5 + r, 19:26, 1:49], gflat[0:1, 864:1200])

    qmaxG = small.tile([128, NQC], F32, tag="qmaxG", name="qmaxG")
    qmaxL = small.tile([128, NQC], F32, tag="qmaxL", name="qmaxL")

    match_chunks(e1x, accDG, accAG, range(3))      # gm starts immediately
    # |b|^2 in transposed layout (commutes with the ref-max): Square + 10
    # small matmuls slot between gm chunks; only the qmax tail needs them.
    off = 0
    for cw in E3CH:
        nc.scalar.activation(esq3[:, off:off + cw], e3x[0:100, off:off + cw],
                             AF.Square)
        off += cw
    nc.gpsimd.memset(b2T[:], 0.0)
    for i in range(NQC):
        n = 128 if i < 9 else 48
        b2ps = pconv.tile([128, 2], F32, tag="conv", name="b2ps")
        nc.tensor.matmul(b2ps[0:n, :], esq3[:, i * 128:i * 128 + n], c1[:],
                         start=True, stop=True)
        nc.scalar.copy(b2T[0:n, i:i + 1], b2ps[0:n, 0:1])
    match_chunks(e1x, accDG, accAG, range(3, 18))
    ref_finals(accDG, accAG, accMG, qmaxG)         # DVE combine (overlaps lm)
    match_chunks(e2x, accDL, accAL, range(4))
    ref_transposes(accMG, qmaxG)                   # PE slots between lm MMs
    match_chunks(e2x, accDL, accAL, range(4, 18))
    ref_plane(qmaxG, 0, nc.gpsimd)
    ref_finals(accDL, accAL, accML, qmaxL)
    ref_transposes(accML, qmaxL)
    ref_plane(qmaxL, 1, nc.gpsimd)

    # ------------------------------------------------------- head conv
    r0 = 0
    for nr in (8, 8, 8):
        ps = pconv.tile([1, nr * W], F32, tag="conv", name="convps")
        for s in range(9):
            dy, dx = s // 3, s % 3
            nc.tensor.matmul(ps[:], wt["dshc"][:, s:s + 1],
                             xt3[:, r0 + dy:r0 + dy + nr, dx:dx + W],
                             start=(s == 0), stop=(s == 8))
        nc.scalar.activation(out_sb[0:1, r0 * W:(r0 + nr) * W],
                             _r3(ps[:], nr, W), AF.Identity, bias=bia["dsh"])
        r0 += nr
    nc.sync.dma_start(out_d, out_sb[:])


def build_program():
    import contextlib
    nc = bacc.Bacc("TRN2", target_bir_lowering=False, debug=False,
                   num_devices=8)
    with tile.TileContext(nc) as tc:
        with contextlib.ExitStack() as ctx:
            _emit(nc, tc, ctx)
    nc.compile()
    return nc


def _get_program():
    global _PROG
    if _PROG is None:
        _PROG = build_program()
    return _PROG


CORE_BC = [(0, 2), (0, 3), (1, 2), (1, 3)]


def _wT_flat(w):
    """[Cout, Cin, 3, 3] -> [Cin, 9*Cout]: col block s holds w[:, :, s//3, s%3].T"""
    cout, cin = w.shape[:2]
    out = np.zeros((cin, 9 * cout), np.float32)
    for s in range(9):
        out[:, s * cout:(s + 1) * cout] = w[:, :, s // 3, s % 3].T
    return out


def _pad50(img):
    out = np.zeros((50, 50), np.float32)
    out[1:49, 1:49] = img
    return out


def _im2col9(img, rows, ones_row=False):
    """padded 50x50 -> [9(+1), rows*48] rows ordered s=dy*3+dx."""
    p = _pad50(img)
    rws = [p[dy:dy + rows, dx:dx + W].ravel()
           for dy in range(3) for dx in range(3)]
    if ones_row:
        rws.append(np.ones(rows * W, np.float32))
    return np.stack(rws)


def _blobs(inp, flip, c):
    w = {k: (inp[k][:, :, ::-1, :] if flip else inp[k])
         for k in ["enc1_w", "enc2_w", "bott_w", "dec2_w", "dec1_w",
                   "emb_w", "dsh_w"]}
    seg = {}
    seg["enc1s"] = w["enc1_w"].reshape(16, 3, 9).transpose(2, 1, 0) \
                              .reshape(27, 16)
    seg["enc2"] = _wT_flat(w["enc2_w"])
    seg["bott"] = _wT_flat(w["bott_w"])
    seg["dec2"] = _wT_flat(w["dec2_w"])
    seg["dec1"] = _wT_flat(w["dec1_w"])
    seg["out"] = inp["out_w"][:, :, 0, 0].T
    seg["outc"] = inp["out_w"][c, :, 0, 0][:, None]
    seg["emb"] = np.vstack([w["emb_w"].reshape(100, 9).T,
                            inp["emb_b"][None, :]])
    seg["dshc"] = w["dsh_w"].reshape(7, 9)

    def pack(segs, ncols):
        blob = np.zeros((96, ncols), np.float16)
        off = 0
        for nm, r0, rows, cols in segs:
            blob[r0:r0 + rows, off:off + cols] = seg[nm].astype(np.float16)
            off += cols
        return blob

    blobbias = np.zeros((96, 10), np.float32)
    for nm, col in BIAS_COL.items():
        if nm == "outc":
            v = inp["out_b"][c:c + 1]
        else:
            v = inp[nm + "_b"]
        blobbias[0:len(v), col] = v
    blobbias[32:48, 8] = inp["enc1_b"]
    blobbias[64:96, 9] = inp["enc2_b"]
    return pack(ASEGS, ACOLS), pack(BSEGS, BCOLS), blobbias


def make_in_maps(inp):
    maps = []
    for k8 in range(8):
        n_idx, half = k8 // 2, k8 % 2
        b, c = CORE_BC[n_idx]
        x1c, x2c, x3b = inp["x1"][b, c], inp["x2"][b, c], inp["x3"][b]
        if half:
            x1c, x2c, x3b = x1c[::-1], x2c[::-1], x3b[:, ::-1]
        bx1 = np.zeros((27, 39 * W), np.float32)
        for ci in range(3):
            im9 = _im2col9(x3b[ci], 39)
            for s in range(9):
                bx1[s * 3 + ci] = im9[s]
        bx2 = np.concatenate([_im2col9(x1c, H, True),
                              _im2col9(x2c, H, True)], axis=1)
        bx3 = x2c[0:25, :].reshape(1, Q)
        blobA, blobB, blobbias = _blobs(inp, bool(half), c)
        maps.append({"blobA": blobA, "blobB": blobB,
                     "blobBias": blobbias,
                     "blobX1": np.ascontiguousarray(bx1.astype(np.float16)),
                     "blobX2": np.ascontiguousarray(bx2.astype(np.float16)),
                     "blobX3": np.ascontiguousarray(bx3.astype(np.float16))})
    return maps


def assemble(results):
    out = np.zeros((2, 2, H, W), np.float32)
    for k8, r in enumerate(results):
        n_idx, half = k8 // 2, k8 % 2
        b, c = CORE_BC[n_idx]
        y = r["out"].reshape(24, W)
        if half == 0:
            out[b, c - 2, 0:24] = y
        else:
            out[b, c - 2, 24:48] = y[::-1]
    return out


def kernel(**inputs):
    inp = {k: np.asarray(v) for k, v in inputs.items()}
    nc = _get_program()
    maps = make_in_maps(inp)
    res = run_bass_kernel_spmd(nc, maps, core_ids=list(range(8)), trace=False)
    return assemble(res.results)
